# revision 2
# baseline (speedup 1.0000x reference)
"""Trainium2 Bass kernel for a GPT-style transformer block (B=2, T=2048,
C=1024, 16 heads, MLP 4x), sharded across 8 NeuronCores.

v2 sharding: attention is HEAD-sharded. Core c = (b, i) with b = c//4,
i = c%4 owns heads [4i, 4i+4) of batch b for the whole 2048-token range:
it computes q/k/v for just those 256 channels (same FLOPs as a row shard
of all channels), runs causal attention with no cross-core kv exchange,
then produces token-major c_proj partials. Two ReduceScatters (one per
1024-token chunk) sum the partials over the 4-core batch group and hand
each core a contiguous 256-token strip per chunk; LN2+MLP then run
data-parallel on the core's two strips.

Host precomputes LN1 (inputs-only), folds LN scale/shift into the matmul
weights, folds 1/sqrt(D) into Wq, and folds the v-bias + proj-bias into
the residual (softmax weights sum to 1, so attn_out = av/den + bv and
proj(attn_out) = proj(av/den) + Wp@bv). Causality lives in the loop
bounds; only diagonal 128x128 tiles get a multiplicative triu mask.
"""
import numpy as np
import ml_dtypes

import concourse.bass as bass
import concourse.mybir as mybir
import concourse.tile as tile
import concourse.bacc as bacc
from concourse.bass_utils import run_bass_kernel_spmd

BF = ml_dtypes.bfloat16
P = 128
B, T, C, D, F = 2, 2048, 1024, 64, 4096
NCT = C // P            # 8   c-tiles
NFT = F // P            # 32  f-tiles
NTT = T // P            # 16  token tiles
HC = 4                  # heads per core
SPAN = 512              # q/token span
NSPAN = T // SPAN       # 4
STRIP = 256             # tokens owned per core per RS chunk
EPS = 1e-5
f32 = mybir.dt.float32
bf16 = mybir.dt.bfloat16
AF = mybir.ActivationFunctionType

_CACHED_NC = None
import os as _os
DBG = _os.environ.get("KDBG", "")


def _build_nc():
    nc = bacc.Bacc("TRN2", target_bir_lowering=False, debug=False)
    d = {}
    for name, shape, dt in [
        ("gT", [C, T], bf16),          # ln1(x) for the whole batch, ch-major
        ("WqkT", [C, 512], bf16),      # q(256) | k(256) out channels
        ("WvT", [C, 256], bf16),
        ("WpT", [256, C], bf16),       # [attn-ch, out-ch] for moving use
        ("WupT", [C, F], bf16),
        ("WdownT", [F, C], bf16),
        ("bqk", [P, 4], f32),
        ("bup", [P, 32], f32),
        ("bdown", [P, 8], f32),
        ("xbT", [C, 512], f32),        # residual for own tokens (biases folded)
        ("diagm", [P, P], bf16),       # triu causal mask for diagonal tiles
    ]:
        d[name] = nc.dram_tensor(name, shape, dt, kind="ExternalInput").ap()
    d["OUT"] = nc.dram_tensor("OUT", [C, 512], f32, kind="ExternalOutput").ap()

    with tile.TileContext(nc) as tc:
        _emit(nc, tc, d)
    nc.compile()
    return nc


def _emit(nc, tc, d):
    from contextlib import ExitStack

    with ExitStack() as ctx:
        # ---------------- long-lived tiles ----------------
        cpool = ctx.enter_context(tc.tile_pool(name="cpool", bufs=1))
        attnT = cpool.tile([P, 2, T], bf16, name="attnT")
        bqk = cpool.tile([P, 4], f32, name="bqk")
        bup = cpool.tile([P, 32], f32, name="bup")
        bdown = cpool.tile([P, 8], f32, name="bdown")
        diagm = cpool.tile([P, P], bf16, name="diagm")
        epsT = cpool.tile([P, 1], f32, name="epsT")
        onesb = cpool.tile([P, P], bf16, name="onesb")
        selp = cpool.tile([P, P], bf16, name="selp")
        gate = cpool.tile([P, 1], f32, name="gate")
        nc.vector.memset(epsT[:], EPS)
        nc.vector.memset(onesb[:], 1.0)
        nc.vector.memset(selp[:], 0.0)
        nc.vector.memset(selp[0:1, 0:64], 1.0)
        nc.vector.memset(selp[64:65, 64:128], 1.0)

        # DRAM scratch for the proj partials + RS outputs
        dramp = ctx.enter_context(tc.tile_pool(name="dramp", bufs=1,
                                               space="DRAM"))
        partials = [dramp.tile([1024, C], bf16, name=f"part{r}")
                    for r in range(2)]
        rsout = [dramp.tile([STRIP, C], bf16, name=f"rsout{r}")
                 for r in range(2)]

        # MLP weights / residual / strip tiles: allocated BEFORE the
        # attention pools so strip-0's MLP can run while attention finishes
        # (tiles in later pools would inherit waits on attention SBUF reuse).
        wmlp = ctx.enter_context(tc.tile_pool(name="wmlp", bufs=1))
        wd = wmlp.tile([P, NFT, C], bf16, name="wd")
        x1T = wmlp.tile([P, NCT, 512], f32, name="x1T")    # xb, then x1
        mep = ctx.enter_context(tc.tile_pool(name="mep", bufs=1))

        # attention operand tiles (freed after proj)
        bpool_cm = tc.tile_pool(name="bpool", bufs=1)
        bpool = bpool_cm.__enter__()
        qT = bpool.tile([P, 2, T], bf16, name="qT")
        kT = bpool.tile([P, 2, T], bf16, name="kT")
        v_aug = bpool.tile([P, HC, NTT, 65], bf16, name="v_aug")
        wp_sb = bpool.tile([P, 2, C], bf16, name="wp_sb")
        nc.vector.memset(v_aug[:, :, :, 64:65], 1.0)

        # ---------------- qkv projections ----------------
        with tc.tile_pool(name="gpool", bufs=2) as gpool, \
             tc.tile_pool(name="wqkp", bufs=1) as wqkp, \
             tc.tile_pool(name="qkps", bufs=3, space="PSUM") as qkps:
            wqk = wqkp.tile([P, NCT, 512], bf16, name="wqk")
            wv = wqkp.tile([P, NCT, 256], bf16, name="wv")
            wqksrc = d["WqkT"].rearrange("(ct p) o -> p ct o", p=P)
            gsrc = d["gT"].rearrange("(ct p) t -> p ct t", p=P)
            # startup-critical order: q weights, first g chunk, then the rest
            nc.sync.dma_start(wqk[:, :, 0:256], wqksrc[:, :, 0:256])
            g0 = gpool.tile([P, NCT, 256], bf16, name="g", tag="g")
            nc.sync.dma_start(g0[:], gsrc[:, :, 0:256])
            nc.sync.dma_start(wqk[:, :, 256:512], wqksrc[:, :, 256:512])
            nc.sync.dma_start(wv[:],
                              d["WvT"].rearrange("(ct p) o -> p ct o", p=P))
            for t, key in [(bqk, "bqk"), (bup, "bup"), (bdown, "bdown"),
                           (diagm, "diagm")]:
                nc.sync.dma_start(t[:], d[key])
            nc.sync.dma_start(wp_sb[:],
                              d["WpT"].rearrange("(j p) o -> p j o", p=P))
            for hh in range(2 * NSPAN):      # half-spans of 256 tokens
                c0 = hh * 256
                if hh == 0:
                    g = g0
                else:
                    g = gpool.tile([P, NCT, 256], bf16, name="g", tag="g")
                    nc.sync.dma_start(g[:], gsrc[:, :, c0:c0 + 256])
                for ot in range(4):          # q0 q1 k0 k1
                    ps = qkps.tile([P, 256], f32, name="ps", tag="qk")
                    for ct in range(NCT):
                        nc.tensor.matmul(ps[:], wqk[:, ct, ot * P:(ot + 1) * P],
                                         g[:, ct, :],
                                         start=(ct == 0), stop=(ct == NCT - 1))
                    dstT = qT if ot < 2 else kT
                    nc.scalar.add(dstT[:, ot % 2, c0:c0 + 256],
                                  ps[:], bqk[:, ot:ot + 1])
                for tl in range(2):          # v, token tile kt = 2*hh+tl
                    kt = 2 * hh + tl
                    pv = qkps.tile([P, 256], f32, name="pv", tag="qk")
                    for ct in range(NCT):
                        nc.tensor.matmul(pv[:], g[:, ct, tl * P:(tl + 1) * P],
                                         wv[:, ct, :],
                                         start=(ct == 0), stop=(ct == NCT - 1))
                    nc.scalar.copy(
                        v_aug[:, :, kt, 0:64],
                        pv[:].rearrange("p (h dd) -> p h dd", dd=64))
            # stage the big loads (SP queue, after the startup-critical ones)
            wdsrc = d["WdownT"].rearrange("(cf p) o -> p cf o", p=P)
            for cc in range(8):
                nc.sync.dma_start(wd[:, cc * 4:(cc + 1) * 4, :],
                                  wdsrc[:, cc * 4:(cc + 1) * 4, :])
            nc.sync.dma_start(x1T[:],
                              d["xbT"].rearrange("(ct p) t -> p ct t", p=P))

        # ----- strip-MLP emission helpers -----
        wusrc = d["WupT"].rearrange("(ct p) f -> p ct f", p=P)
        outdst = d["OUT"].rearrange("(ot p) t -> p ot t", p=P)

        def emit_wuw(w):
            t = mep.tile([P, NCT, 1024], bf16, name="wuw", tag="wuw", bufs=2)
            nc.sync.dma_start(
                t[:], wusrc[:, :, (w % 4) * 1024:((w % 4) + 1) * 1024])
            return t

        def emit_rs_transpose(r):
            rsT = mep.tile([P, NCT, STRIP], bf16, name="rsT", tag="rsT")
            nc.sync.dma_start_transpose(rsT[:], rsout[r][:])
            return rsT

        def emit_strip_pre(r, rsT, upps, gate_ex=None):
            cs = r * STRIP
            if gate_ex is not None:
                # scheduler-proofing: root the chain on a late attention exp
                # so its long RS-wait cannot head-block the DVE queue ahead
                # of ready attention work (the scheduler's internal sim
                # underestimates collective latency)
                gate2 = mep.tile([P, 1], f32, name="gate2", tag="gate2")
                nc.vector.tensor_scalar(gate2[:], gate_ex[:, 511:512],
                                        0.0, 1.0, mybir.AluOpType.mult,
                                        mybir.AluOpType.add)
                rsTg = mep.tile([P, NCT, STRIP], bf16, name="rsTg",
                                tag="x1b")
                nc.vector.tensor_scalar(rsTg[:], rsT[:], gate2[:, 0:1], None,
                                        mybir.AluOpType.mult)
                rsT = rsTg
            nc.vector.tensor_add(x1T[:, :, cs:cs + STRIP],
                                 x1T[:, :, cs:cs + STRIP], rsT[:])
            if DBG == "x1":
                nc.sync.dma_start(outdst[:, :, cs:cs + STRIP],
                                  x1T[:, :, cs:cs + STRIP])
                return None
            x1b = mep.tile([P, NCT, STRIP], bf16, name="x1b", tag="x1b")
            sqb = mep.tile([P, NCT, STRIP], bf16, name="sqb", tag="g28")
            for ct in range(NCT):
                nc.vector.tensor_copy(x1b[:, ct, :], x1T[:, ct, cs:cs + STRIP])
                nc.vector.tensor_mul(sqb[:, ct, :], x1T[:, ct, cs:cs + STRIP],
                                     x1T[:, ct, cs:cs + STRIP])
            psmu = upps.tile([P, STRIP], f32, name="psmu", tag="pu")
            for ct in range(NCT):
                nc.tensor.matmul(psmu[:], onesb[:], x1b[:, ct, :],
                                 start=(ct == 0), stop=(ct == NCT - 1))
            pssq = upps.tile([P, STRIP], f32, name="pssq", tag="pu")
            for ct in range(NCT):
                nc.tensor.matmul(pssq[:], onesb[:], sqb[:, ct, :],
                                 start=(ct == 0), stop=(ct == NCT - 1))
            mu = mep.tile([P, STRIP], f32, name="mu", tag="mu")
            e2 = mep.tile([P, STRIP], f32, name="e2", tag="e2")
            std = mep.tile([P, STRIP], f32, name="std", tag="stdt")
            nc.scalar.mul(mu[:], psmu[:], 1.0 / C)
            nc.scalar.mul(e2[:], pssq[:], 1.0 / C)
            musq = mep.tile([P, STRIP], f32, name="musq", tag="tmpc", bufs=2)
            nc.vector.tensor_mul(musq[:], mu[:], mu[:])
            nc.vector.tensor_sub(e2[:], e2[:], musq[:])
            nc.scalar.activation(std[:], e2[:], AF.Sqrt, bias=epsT[:])
            nc.vector.reciprocal(std[:], std[:])
            g2 = mep.tile([P, NCT, STRIP], bf16, name="g2", tag="g28")
            for ct in range(NCT):
                tmpc = mep.tile([P, STRIP], f32, name="tmpc", tag="tmpc",
                                bufs=2)
                nc.vector.tensor_sub(tmpc[:], x1T[:, ct, cs:cs + STRIP],
                                     mu[:])
                nc.vector.tensor_mul(g2[:, ct, :], tmpc[:], std[:])
            if DBG == "g2":
                g2f = mep.tile([P, NCT, STRIP], f32, name="g2f", tag="g2f")
                nc.vector.tensor_copy(g2f[:], g2[:])
                nc.sync.dma_start(outdst[:, :, cs:cs + STRIP], g2f[:])
                return None
            hT = mep.tile([P, NFT, STRIP], bf16, name="hT", tag="hT")
            return dict(cs=cs, g2=g2, hT=hT)

        def emit_up_group(st, f, wt, defer_gelu=False):
            pu = upps_ref[0].tile([P, STRIP], f32, name="pu", tag="pu")
            fl = f % 8
            for ct in range(NCT):
                nc.tensor.matmul(pu[:], wt[:, ct, fl * P:(fl + 1) * P],
                                 st["g2"][:, ct, :],
                                 start=(ct == 0), stop=(ct == NCT - 1))
            if defer_gelu:
                # keep Gelu out of the attention window: its act table
                # does not share a set with Exp, so inline gelus thrash
                # 1.28us table loads per switch.  Stage raw pre-act.
                nc.vector.tensor_copy(st["hT"][:, f, :], pu[:])
            else:
                nc.scalar.activation(st["hT"][:, f, :], pu[:], AF.Gelu,
                                     bias=bup[:, f:f + 1])

        def emit_down_out(st, dnps):
            for ot in range(8):
                pd = dnps.tile([P, STRIP], f32, name="pd", tag="dn")
                for cf in range(NFT):
                    nc.tensor.matmul(pd[:], wd[:, cf, ot * P:(ot + 1) * P],
                                     st["hT"][:, cf, :],
                                     start=(cf == 0), stop=(cf == NFT - 1))
                td = mep.tile([P, STRIP], f32, name="td", tag="td", bufs=2)
                nc.scalar.add(td[:], pd[:], bdown[:, ot:ot + 1])
                ox = mep.tile([P, STRIP], f32, name="ox", tag="ox", bufs=2)
                nc.vector.tensor_add(ox[:], td[:],
                                     x1T[:, ot, st["cs"]:st["cs"] + STRIP])
                nc.gpsimd.dma_start(outdst[:, ot, st["cs"]:st["cs"] + STRIP],
                                    ox[:])

        # ---------------- attention + proj + RS (+ strip0 MLP fill) --------
        wuw = {}
        if not DBG:
            wuw[0] = emit_wuw(0)
            wuw[1] = emit_wuw(1)
        st0 = None
        rsT0 = None
        upf = [0]        # next strip0 up f-group to emit

        with tc.tile_pool(name="scps", bufs=2, space="PSUM") as scps, \
             tc.tile_pool(name="avps", bufs=4, space="PSUM") as avps, \
             tc.tile_pool(name="upps", bufs=2, space="PSUM") as upps, \
             tc.tile_pool(name="expp", bufs=6) as expp, \
             tc.tile_pool(name="nrmp", bufs=2) as nrmp, \
             tc.tile_pool(name="prst", bufs=2) as prst:
            upps_ref = [upps]

            def fill_slot(Q, kt):
                # strip0 MLP work interleaved into spans 2-3's PE stream
                nonlocal st0
                if DBG or Q != 3:
                    return
                if kt == 4:
                    st0 = emit_strip_pre(0, rsT0, upps, gate_ex=last_ex)
                    return
                if st0 is None or upf[0] >= NFT:
                    return
                n = 2
                for _ in range(n):
                    if upf[0] >= NFT:
                        break
                    emit_up_group(st0, upf[0], wuw[upf[0] // 8],
                                  defer_gelu=True)
                    upf[0] += 1

            for Q in range(NSPAN):
                q0 = Q * SPAN
                avs = []
                for h in range(HC):
                    av = avps.tile([P, 512], f32, name=f"av{h}", tag="av")
                    nc.vector.memset(av[:], 0.0)
                    avs.append(av)
                nkt = 4 * Q + 4
                for kt in range(nkt):
                    p_ = kt - 4 * Q
                    c0 = 128 * p_ if p_ > 0 else 0
                    scs = []
                    for h in range(HC):
                        hb = (h % 2) * 64
                        j = h // 2
                        sc = scps.tile([P, 512], f32, name="sc", tag="sc")
                        nc.tensor.matmul(
                            sc[:, c0:512],
                            kT[hb:hb + 64, j, kt * P:(kt + 1) * P],
                            qT[hb:hb + 64, j, q0 + c0:q0 + 512],
                            start=True, stop=True)
                        scs.append(sc)
                    for h in range(HC):
                        ex = expp.tile([P, 512], bf16, name="ex", tag="ex")
                        last_ex = ex
                        nc.scalar.activation(ex[:, c0:512], scs[h][:, c0:512],
                                             AF.Exp)
                        if p_ >= 0:
                            nc.vector.tensor_mul(ex[:, c0:c0 + 128],
                                                 ex[:, c0:c0 + 128], diagm[:])
                        nc.tensor.matmul(
                            avs[h][0:65, c0:512],
                            v_aug[:, h, kt, 0:65],
                            ex[:, c0:512],
                            start=False, stop=(kt == nkt - 1),
                            skip_group_check=True)
                    fill_slot(Q, kt)
                # normalize: attnT[:, pr, span] = av / den  (head pair pr)
                for pr in range(2):
                    denb = nrmp.tile([P, 512], bf16, name="denb", tag="denb")
                    nc.vector.memset(denb[0:65, :], 0.0)
                    with nc.allow_low_precision(reason="bf16 softmax denom"):
                        nc.vector.reciprocal(denb[0:1, :],
                                             avs[2 * pr][64:65, :])
                        nc.vector.reciprocal(denb[64:65, :],
                                             avs[2 * pr + 1][64:65, :])
                    bc = scps.tile([P, 512], f32, name="bc", tag="sc")
                    nc.tensor.matmul(bc[:], selp[0:65, :], denb[0:65, :],
                                     start=True, stop=True)
                    bcs = nrmp.tile([P, 512], f32, name="bcs", tag="bcs")
                    nc.scalar.copy(bcs[:], bc[:])
                    nc.vector.tensor_mul(attnT[0:64, pr, q0:q0 + SPAN],
                                         avs[2 * pr][0:64, :], bcs[0:64, :])
                    nc.vector.tensor_mul(attnT[64:128, pr, q0:q0 + SPAN],
                                         avs[2 * pr + 1][0:64, :],
                                         bcs[64:128, :])
                # proj partials for this span, token-major
                for tl in range(4):
                    tt = 4 * Q + tl
                    stage = prst.tile([P, 1024], bf16, name="stage", tag="st")
                    for half in range(2):
                        pp = scps.tile([P, 512], f32, name="pp", tag="sc")
                        for j in range(2):
                            nc.tensor.matmul(
                                pp[:],
                                attnT[:, j, tt * P:(tt + 1) * P],
                                wp_sb[:, j, half * 512:(half + 1) * 512],
                                start=(j == 0), stop=(j == 1))
                        nc.vector.tensor_copy(
                            stage[:, half * 512:(half + 1) * 512], pp[:])
                    row = (Q % 2) * 512 + tl * P
                    nc.gpsimd.dma_start(partials[Q // 2][row:row + P, :],
                                        stage[:])
                if Q % 2 == 1:
                    r = Q // 2
                    nc.gpsimd.collective_compute(
                        "ReduceScatter", mybir.AluOpType.add,
                        ins=[partials[r].opt()], outs=[rsout[r].opt()],
                        replica_groups=[[0, 1, 2, 3], [4, 5, 6, 7]])
                    if r == 0:
                        rsT0 = emit_rs_transpose(0)
                        if not DBG:
                            wuw[2] = emit_wuw(2)
                            wuw[3] = emit_wuw(3)
                    else:
                        rsT1 = emit_rs_transpose(1)
                        if not DBG:
                            for w in range(4, 8):
                                wuw[w] = emit_wuw(w)
            # rest of strip0's up groups (overlaps RS2 on the collective cores)
            if not DBG and st0 is not None:
                while upf[0] < NFT:
                    emit_up_group(st0, upf[0], wuw[upf[0] // 8],
                                  defer_gelu=True)
                    upf[0] += 1
            # gate = 1.0, data-dependent on the last exp: ops scaled by it
            # cannot be scheduled into the attention exp stream (their act
            # tables would thrash the exp table set)
            nc.vector.tensor_scalar(gate[:], last_ex[:, 511:512], 0.0, 1.0,
                                    mybir.AluOpType.mult, mybir.AluOpType.add)
            if DBG:
                rsT0x = rsT0 if rsT0 is not None else emit_rs_transpose(0)
                emit_strip_pre(0, rsT0x, upps)
                emit_strip_pre(1, emit_rs_transpose(1), upps)

        bpool_cm.__exit__(None, None, None)   # free qT / kT / v_aug / wp_sb

        # ---------------- strip0 down + full strip1 ----------------
        if not DBG:
            with tc.tile_pool(name="dnps", bufs=2, space="PSUM") as dnps, \
                 tc.tile_pool(name="up2", bufs=2, space="PSUM") as up2:
                upps_ref[0] = up2
                # bulk gelu for strip0's staged pre-activations (one table
                # switch, after all attention exps are done; gate enforces it)
                for f in range(NFT):
                    nc.scalar.activation(st0["hT"][:, f, :], st0["hT"][:, f, :],
                                         AF.Gelu, bias=bup[:, f:f + 1],
                                         scale=gate[:, 0:1])
                emit_down_out(st0, dnps)
                st1 = emit_strip_pre(1, rsT1, up2)
                for f in range(NFT):
                    emit_up_group(st1, f, wuw[4 + f // 8])
                emit_down_out(st1, dnps)


def _prep_inputs(x, ln1_w, ln1_b, c_attn_w, c_attn_b, c_proj_w, c_proj_b,
                 ln2_w, ln2_b, up_w, up_b, down_w, down_b):
    """Host-side preprocessing -> list of 8 per-core input dicts."""
    x = np.asarray(x, np.float32)
    f64 = np.float64
    mu = x.mean(-1, keepdims=True, dtype=f64)
    var = np.asarray(x, f64).var(-1, keepdims=True)
    g = ((x - mu) / np.sqrt(var + EPS)).astype(np.float32)     # [B, T, C]

    ln1_w = np.asarray(ln1_w, np.float32); ln1_b = np.asarray(ln1_b, np.float32)
    ln2_w = np.asarray(ln2_w, np.float32); ln2_b = np.asarray(ln2_b, np.float32)
    c_attn_w = np.asarray(c_attn_w, np.float32)
    c_attn_b = np.asarray(c_attn_b, np.float32)
    c_proj_w = np.asarray(c_proj_w, np.float32)
    c_proj_b = np.asarray(c_proj_b, np.float32)
    up_w = np.asarray(up_w, np.float32); up_b = np.asarray(up_b, np.float32)
    down_w = np.asarray(down_w, np.float32)
    down_b = np.asarray(down_b, np.float32)

    Wa = c_attn_w * ln1_w[None, :]                  # fold LN1 scale
    ba = c_attn_b + c_attn_w @ ln1_b                # fold LN1 shift
    Wq, Wk, Wv = Wa[:C], Wa[C:2 * C], Wa[2 * C:]
    bq, bk, bv = ba[:C], ba[C:2 * C], ba[2 * C:]
    s = 1.0 / np.sqrt(D)
    Wq = Wq * s; bq = bq * s                        # fold attention scale

    Wup = up_w * ln2_w[None, :]
    bupv = up_b + up_w @ ln2_b

    def b2t(v, n):   # per-partition bias layout [128, n]
        return np.ascontiguousarray(v.reshape(n, P).T.astype(np.float32))

    diagm = np.triu(np.ones((P, P), np.float32))    # kv row <= q col

    shared = {
        "WupT": np.ascontiguousarray(Wup.T).astype(BF),
        "WdownT": np.ascontiguousarray(down_w.T).astype(BF),
        "bup": b2t(bupv, 32), "bdown": b2t(down_b, 8),
        "diagm": diagm.astype(BF),
    }

    # residual with proj bias and (v-bias pushed through proj) folded in
    xb = x + (c_proj_b + c_proj_w @ bv)[None, None, :]

    in_maps, tok_slices = [], []
    for core in range(8):
        b, i = core // 4, core % 4
        ch = slice(i * 256, (i + 1) * 256)          # this core's attn channels
        wqk = np.concatenate([Wq[ch], Wk[ch]], axis=0)      # [512, 1024]
        m = dict(shared)
        m["WqkT"] = np.ascontiguousarray(wqk.T).astype(BF)
        m["WvT"] = np.ascontiguousarray(Wv[ch].T).astype(BF)
        m["WpT"] = np.ascontiguousarray(c_proj_w[:, ch].T).astype(BF)
        m["bqk"] = b2t(np.concatenate([bq[ch], bk[ch]]), 4)
        m["gT"] = np.ascontiguousarray(g[b].T).astype(BF)
        strips = [slice(1024 * r + STRIP * i, 1024 * r + STRIP * (i + 1))
                  for r in range(2)]
        xbT = np.concatenate([xb[b, st].T for st in strips], axis=1)
        m["xbT"] = np.ascontiguousarray(xbT).astype(np.float32)
        in_maps.append(m)
        tok_slices.append((b, strips))
    return in_maps, tok_slices


def kernel(**inputs):
    global _CACHED_NC
    if _CACHED_NC is None:
        _CACHED_NC = _build_nc()
    nc = _CACHED_NC
    in_maps, tok_slices = _prep_inputs(**inputs)
    try:
        res = run_bass_kernel_spmd(nc, in_maps, list(range(8)))
    except Exception:
        # one retry: transient NRT device faults are recoverable on re-run
        res = run_bass_kernel_spmd(nc, in_maps, list(range(8)))
    out = np.empty((B, T, C), np.float32)
    for core in range(8):
        o = res.results[core]["OUT"]                # [C, 512]
        b, strips = tok_slices[core]
        for r, st in enumerate(strips):
            out[b, st, :] = o[:, r * STRIP:(r + 1) * STRIP].T
    return out


# revision 3
# speedup vs baseline: 1.0296x; 1.0296x over previous
"""Trainium2 Bass kernel for a GPT-style transformer block (B=2, T=2048,
C=1024, 16 heads, MLP 4x), sharded across 8 NeuronCores.

Sharding: attention is HEAD-sharded. Core c = (b, i) with b = c//4,
i = c%4 owns heads [4i, 4i+4) of batch b for the whole 2048-token range:
it computes q/k/v for just those 256 channels (same FLOPs as a row shard
of all channels), runs causal attention with no cross-core kv exchange,
then produces token-major c_proj partials. Two ReduceScatters (one per
1024-token chunk) sum the partials over the 4-core batch group and hand
each core a contiguous 256-token strip per chunk; LN2+MLP then run
data-parallel on the core's two strips.

Host precomputes LN1 (inputs-only), folds LN scale/shift into the matmul
weights, folds 1/sqrt(D) into Wq, and folds the v-bias + proj-bias into
the residual (softmax weights sum to 1, so attn_out = av/den + bv and
proj(attn_out) = proj(av/den) + Wp@bv). Causality lives in the loop
bounds; only diagonal 128x128 tiles get a multiplicative triu mask.
"""
import numpy as np
import ml_dtypes

import concourse.bass as bass
import concourse.mybir as mybir
import concourse.tile as tile
import concourse.bacc as bacc
from concourse.bass_utils import run_bass_kernel_spmd

BF = ml_dtypes.bfloat16
P = 128
B, T, C, D, F = 2, 2048, 1024, 64, 4096
NCT = C // P            # 8   c-tiles
NFT = F // P            # 32  f-tiles
NTT = T // P            # 16  token tiles
HC = 4                  # heads per core
SPAN = 512              # q/token span
NSPAN = T // SPAN       # 4
STRIP = 256             # tokens owned per core per RS chunk
EPS = 1e-5
f32 = mybir.dt.float32
bf16 = mybir.dt.bfloat16
AF = mybir.ActivationFunctionType

_CACHED_NC = None
import os as _os
DBG = _os.environ.get("KDBG", "")


def _build_nc():
    nc = bacc.Bacc("TRN2", target_bir_lowering=False, debug=False)
    d = {}
    for name, shape, dt in [
        ("gT", [C, T], bf16),          # ln1(x) for the whole batch, ch-major
        ("WqkT", [C, 512], bf16),      # q(256) | k(256) out channels
        ("WvT", [C, 256], bf16),
        ("WpT", [256, C], bf16),       # [attn-ch, out-ch] for moving use
        ("WupT", [C, F], bf16),
        ("WdownT", [F, C], bf16),
        ("bqk", [P, 4], f32),
        ("bup", [P, 32], f32),
        ("bdown", [P, 8], f32),
        ("xbT", [C, 512], f32),        # residual for own tokens (biases folded)
        ("diagm", [P, P], bf16),       # triu causal mask for diagonal tiles
    ]:
        d[name] = nc.dram_tensor(name, shape, dt, kind="ExternalInput").ap()
    d["OUT"] = nc.dram_tensor("OUT", [C, 512], f32, kind="ExternalOutput").ap()

    with tile.TileContext(nc) as tc:
        _emit(nc, tc, d)
    nc.compile()
    return nc


def _emit(nc, tc, d):
    from contextlib import ExitStack

    with ExitStack() as ctx:
        # ---------------- long-lived tiles ----------------
        cpool = ctx.enter_context(tc.tile_pool(name="cpool", bufs=1))
        attnT = cpool.tile([P, 2, T], bf16, name="attnT")
        bqk = cpool.tile([P, 4], f32, name="bqk")
        bup = cpool.tile([P, 32], f32, name="bup")
        bdown = cpool.tile([P, 8], f32, name="bdown")
        diagm = cpool.tile([P, P], bf16, name="diagm")
        epsT = cpool.tile([P, 1], f32, name="epsT")
        onesb = cpool.tile([P, P], bf16, name="onesb")
        selp = cpool.tile([P, P], bf16, name="selp")
        gate = cpool.tile([P, 1], f32, name="gate")
        nc.vector.memset(epsT[:], EPS)
        nc.vector.memset(onesb[:], 1.0)
        nc.vector.memset(selp[:], 0.0)
        nc.vector.memset(selp[0:1, 0:64], 1.0)
        nc.vector.memset(selp[64:65, 64:128], 1.0)

        # DRAM scratch for the proj partials + RS outputs
        dramp = ctx.enter_context(tc.tile_pool(name="dramp", bufs=1,
                                               space="DRAM"))
        partials = [dramp.tile([1024, C], bf16, name=f"part{r}")
                    for r in range(2)]
        rsout = [dramp.tile([STRIP, C], bf16, name=f"rsout{r}")
                 for r in range(2)]

        # MLP weights / residual / strip tiles: allocated BEFORE the
        # attention pools so strip-0's MLP can run while attention finishes
        # (tiles in later pools would inherit waits on attention SBUF reuse).
        wmlp = ctx.enter_context(tc.tile_pool(name="wmlp", bufs=1))
        wd = wmlp.tile([P, NFT, C], bf16, name="wd")
        x1T = wmlp.tile([P, NCT, 512], f32, name="x1T")    # xb, then x1
        mep = ctx.enter_context(tc.tile_pool(name="mep", bufs=1))

        # attention operand tiles (freed after proj)
        bpool_cm = tc.tile_pool(name="bpool", bufs=1)
        bpool = bpool_cm.__enter__()
        qT = bpool.tile([P, 2, T], bf16, name="qT")
        kT = bpool.tile([P, 2, T], bf16, name="kT")
        v_aug = bpool.tile([P, HC, NTT, 65], bf16, name="v_aug")
        wp_sb = bpool.tile([P, 2, C], bf16, name="wp_sb")
        nc.vector.memset(v_aug[:, :, :, 64:65], 1.0)

        # ---------------- qkv projections ----------------
        with tc.tile_pool(name="gpool", bufs=2) as gpool, \
             tc.tile_pool(name="wqkp", bufs=1) as wqkp, \
             tc.tile_pool(name="qkps", bufs=3, space="PSUM") as qkps:
            wqk = wqkp.tile([P, NCT, 512], bf16, name="wqk")
            wv = wqkp.tile([P, NCT, 256], bf16, name="wv")
            wqksrc = d["WqkT"].rearrange("(ct p) o -> p ct o", p=P)
            gsrc = d["gT"].rearrange("(ct p) t -> p ct t", p=P)
            # startup-critical order: q weights, first g chunk, then the rest
            nc.sync.dma_start(wqk[:, 0:4, 0:256], wqksrc[:, 0:4, 0:256])
            g0 = gpool.tile([P, NCT, 256], bf16, name="g", tag="g")
            nc.sync.dma_start(g0[:, 0:4, :], gsrc[:, 0:4, 0:256])
            nc.sync.dma_start(wqk[:, 4:8, 0:256], wqksrc[:, 4:8, 0:256])
            nc.sync.dma_start(g0[:, 4:8, :], gsrc[:, 4:8, 0:256])
            nc.sync.dma_start(wqk[:, :, 256:512], wqksrc[:, :, 256:512])
            nc.sync.dma_start(wv[:],
                              d["WvT"].rearrange("(ct p) o -> p ct o", p=P))
            for t, key in [(bqk, "bqk"), (bup, "bup"), (bdown, "bdown"),
                           (diagm, "diagm")]:
                nc.sync.dma_start(t[:], d[key])
            nc.sync.dma_start(wp_sb[:],
                              d["WpT"].rearrange("(j p) o -> p j o", p=P))
            for hh in range(2 * NSPAN):      # half-spans of 256 tokens
                c0 = hh * 256
                if hh == 0:
                    g = g0
                else:
                    g = gpool.tile([P, NCT, 256], bf16, name="g", tag="g")
                    nc.sync.dma_start(g[:], gsrc[:, :, c0:c0 + 256])
                for ot in range(4):          # q0 q1 k0 k1
                    ps = qkps.tile([P, 256], f32, name="ps", tag="qk")
                    for ct in range(NCT):
                        nc.tensor.matmul(ps[:], wqk[:, ct, ot * P:(ot + 1) * P],
                                         g[:, ct, :],
                                         start=(ct == 0), stop=(ct == NCT - 1))
                    dstT = qT if ot < 2 else kT
                    nc.scalar.add(dstT[:, ot % 2, c0:c0 + 256],
                                  ps[:], bqk[:, ot:ot + 1])
                for tl in range(2):          # v, token tile kt = 2*hh+tl
                    kt = 2 * hh + tl
                    pv = qkps.tile([P, 256], f32, name="pv", tag="qk")
                    for ct in range(NCT):
                        nc.tensor.matmul(pv[:], g[:, ct, tl * P:(tl + 1) * P],
                                         wv[:, ct, :],
                                         start=(ct == 0), stop=(ct == NCT - 1))
                    nc.scalar.copy(
                        v_aug[:, :, kt, 0:64],
                        pv[:].rearrange("p (h dd) -> p h dd", dd=64))
            # stage the big loads (SP queue, after the startup-critical ones)
            wdsrc = d["WdownT"].rearrange("(cf p) o -> p cf o", p=P)
            for cc in range(8):
                nc.sync.dma_start(wd[:, cc * 4:(cc + 1) * 4, :],
                                  wdsrc[:, cc * 4:(cc + 1) * 4, :])
            nc.sync.dma_start(x1T[:],
                              d["xbT"].rearrange("(ct p) t -> p ct t", p=P))

        # ----- strip-MLP emission helpers -----
        wusrc = d["WupT"].rearrange("(ct p) f -> p ct f", p=P)
        outdst = d["OUT"].rearrange("(ot p) t -> p ot t", p=P)

        def emit_wuw(w):
            t = mep.tile([P, NCT, 1024], bf16, name="wuw", tag="wuw", bufs=2)
            nc.sync.dma_start(
                t[:], wusrc[:, :, (w % 4) * 1024:((w % 4) + 1) * 1024])
            return t

        def emit_rs_transpose(r):
            rsT = mep.tile([P, NCT, STRIP], bf16, name="rsT", tag="rsT")
            nc.sync.dma_start_transpose(rsT[:], rsout[r][:])
            return rsT

        def emit_strip_pre(r, rsT, upps, gate_ex=None):
            cs = r * STRIP
            if gate_ex is not None:
                # scheduler-proofing: root the chain on a late attention exp
                # so its long RS-wait cannot head-block the DVE queue ahead
                # of ready attention work (the scheduler's internal sim
                # underestimates collective latency)
                gate2 = mep.tile([P, 1], f32, name="gate2", tag="gate2")
                nc.vector.tensor_scalar(gate2[:], gate_ex[:, 511:512],
                                        0.0, 1.0, mybir.AluOpType.mult,
                                        mybir.AluOpType.add)
                rsTg = mep.tile([P, NCT, STRIP], bf16, name="rsTg",
                                tag="x1b")
                nc.vector.tensor_scalar(rsTg[:], rsT[:], gate2[:, 0:1], None,
                                        mybir.AluOpType.mult)
                rsT = rsTg
            nc.vector.tensor_add(x1T[:, :, cs:cs + STRIP],
                                 x1T[:, :, cs:cs + STRIP], rsT[:])
            if DBG == "x1":
                nc.sync.dma_start(outdst[:, :, cs:cs + STRIP],
                                  x1T[:, :, cs:cs + STRIP])
                return None
            x1b = mep.tile([P, NCT, STRIP], bf16, name="x1b", tag="x1b")
            sqb = mep.tile([P, NCT, STRIP], bf16, name="sqb", tag="g28")
            for ct in range(NCT):
                nc.vector.tensor_copy(x1b[:, ct, :], x1T[:, ct, cs:cs + STRIP])
                nc.vector.tensor_mul(sqb[:, ct, :], x1T[:, ct, cs:cs + STRIP],
                                     x1T[:, ct, cs:cs + STRIP])
            psmu = upps.tile([P, STRIP], f32, name="psmu", tag="pu")
            for ct in range(NCT):
                nc.tensor.matmul(psmu[:], onesb[:], x1b[:, ct, :],
                                 start=(ct == 0), stop=(ct == NCT - 1))
            pssq = upps.tile([P, STRIP], f32, name="pssq", tag="pu")
            for ct in range(NCT):
                nc.tensor.matmul(pssq[:], onesb[:], sqb[:, ct, :],
                                 start=(ct == 0), stop=(ct == NCT - 1))
            mu = mep.tile([P, STRIP], f32, name="mu", tag="mu")
            e2 = mep.tile([P, STRIP], f32, name="e2", tag="e2")
            std = mep.tile([P, STRIP], f32, name="std", tag="stdt")
            nc.scalar.mul(mu[:], psmu[:], 1.0 / C)
            nc.scalar.mul(e2[:], pssq[:], 1.0 / C)
            musq = mep.tile([P, STRIP], f32, name="musq", tag="tmpc", bufs=2)
            nc.vector.tensor_mul(musq[:], mu[:], mu[:])
            nc.vector.tensor_sub(e2[:], e2[:], musq[:])
            nc.scalar.activation(std[:], e2[:], AF.Sqrt, bias=epsT[:])
            nc.vector.reciprocal(std[:], std[:])
            g2 = mep.tile([P, NCT, STRIP], bf16, name="g2", tag="g28")
            for ct in range(NCT):
                tmpc = mep.tile([P, STRIP], f32, name="tmpc", tag="tmpc",
                                bufs=2)
                nc.vector.tensor_sub(tmpc[:], x1T[:, ct, cs:cs + STRIP],
                                     mu[:])
                nc.vector.tensor_mul(g2[:, ct, :], tmpc[:], std[:])
            if DBG == "g2":
                g2f = mep.tile([P, NCT, STRIP], f32, name="g2f", tag="g2f")
                nc.vector.tensor_copy(g2f[:], g2[:])
                nc.sync.dma_start(outdst[:, :, cs:cs + STRIP], g2f[:])
                return None
            hT = mep.tile([P, NFT, STRIP], bf16, name="hT", tag="hT")
            return dict(cs=cs, g2=g2, hT=hT)

        def emit_up_group(st, f, wt, defer_gelu=False):
            pu = upps_ref[0].tile([P, STRIP], f32, name="pu", tag="pu")
            fl = f % 8
            for ct in range(NCT):
                nc.tensor.matmul(pu[:], wt[:, ct, fl * P:(fl + 1) * P],
                                 st["g2"][:, ct, :],
                                 start=(ct == 0), stop=(ct == NCT - 1))
            if defer_gelu:
                # keep Gelu out of the attention window: its act table
                # does not share a set with Exp, so inline gelus thrash
                # 1.28us table loads per switch.  Stage raw pre-act.
                nc.vector.tensor_copy(st["hT"][:, f, :], pu[:])
            else:
                nc.scalar.activation(st["hT"][:, f, :], pu[:], AF.Gelu,
                                     bias=bup[:, f:f + 1])

        def emit_down_out(st, dnps):
            for ot in range(8):
                pd = dnps.tile([P, STRIP], f32, name="pd", tag="dn")
                for cf in range(NFT):
                    nc.tensor.matmul(pd[:], wd[:, cf, ot * P:(ot + 1) * P],
                                     st["hT"][:, cf, :],
                                     start=(cf == 0), stop=(cf == NFT - 1))
                td = mep.tile([P, STRIP], f32, name="td", tag="td", bufs=2)
                nc.scalar.add(td[:], pd[:], bdown[:, ot:ot + 1])
                ox = mep.tile([P, STRIP], f32, name="ox", tag="ox", bufs=2)
                nc.vector.tensor_add(ox[:], td[:],
                                     x1T[:, ot, st["cs"]:st["cs"] + STRIP])
                nc.gpsimd.dma_start(outdst[:, ot, st["cs"]:st["cs"] + STRIP],
                                    ox[:])

        # ---------------- attention + proj + RS (+ strip0 MLP fill) --------
        wuw = {}
        if not DBG:
            wuw[0] = emit_wuw(0)
            wuw[1] = emit_wuw(1)
        st0 = None
        rsT0 = None
        rsT1 = None
        last_ex = None
        upf = [0]        # next strip0 up f-group to emit

        with tc.tile_pool(name="avps", bufs=4, space="PSUM") as avps, \
             tc.tile_pool(name="expp", bufs=3) as expp, \
             tc.tile_pool(name="nrmp", bufs=2) as nrmp, \
             tc.tile_pool(name="prst", bufs=2) as prst:

            def span_tail(Q, avs, mkpsum):
                # normalize: attnT[:, pr, span] = av / den, then proj + RS
                q0 = Q * SPAN
                for pr in range(2):
                    denb = nrmp.tile([P, 512], bf16, name="denb", tag="denb")
                    nc.vector.memset(denb[0:65, :], 0.0)
                    with nc.allow_low_precision(reason="bf16 softmax denom"):
                        nc.vector.reciprocal(denb[0:1, :],
                                             avs[2 * pr][64:65, :])
                        nc.vector.reciprocal(denb[64:65, :],
                                             avs[2 * pr + 1][64:65, :])
                    bc = mkpsum()
                    nc.tensor.matmul(bc[:], selp[0:65, :], denb[0:65, :],
                                     start=True, stop=True)
                    bcs = nrmp.tile([P, 512], f32, name="bcs", tag="bcs")
                    nc.scalar.copy(bcs[:], bc[:])
                    nc.vector.tensor_mul(attnT[0:64, pr, q0:q0 + SPAN],
                                         avs[2 * pr][0:64, :], bcs[0:64, :])
                    nc.vector.tensor_mul(attnT[64:128, pr, q0:q0 + SPAN],
                                         avs[2 * pr + 1][0:64, :],
                                         bcs[64:128, :])
                for tl in range(4):
                    tt = 4 * Q + tl
                    stage = prst.tile([P, 1024], bf16, name="stage", tag="st")
                    for half in range(2):
                        pp = mkpsum()
                        for j in range(2):
                            nc.tensor.matmul(
                                pp[:],
                                attnT[:, j, tt * P:(tt + 1) * P],
                                wp_sb[:, j, half * 512:(half + 1) * 512],
                                start=(j == 0), stop=(j == 1))
                        nc.vector.tensor_copy(
                            stage[:, half * 512:(half + 1) * 512], pp[:])
                    row = (Q % 2) * 512 + tl * P
                    nc.gpsimd.dma_start(partials[Q // 2][row:row + P, :],
                                        stage[:])
                if Q % 2 == 1:
                    r = Q // 2
                    nc.gpsimd.collective_compute(
                        "ReduceScatter", mybir.AluOpType.add,
                        ins=[partials[r].opt()], outs=[rsout[r].opt()],
                        replica_groups=[[0, 1, 2, 3], [4, 5, 6, 7]])
                    return emit_rs_transpose(r)
                return None

            # ---- spans 0-2: paired scores, one exp per head pair ----
            scp_cm = tc.tile_pool(name="scp", bufs=2, space="PSUM")
            scp = scp_cm.__enter__()

            def mkpsum_pair():
                t = scp.tile([P, 2, 512], f32, name="sc2", tag="sc")
                return t[:, 0, :]

            for Q in range(3):
                q0 = Q * SPAN
                avs = []
                for h in range(HC):
                    av = avps.tile([P, 512], f32, name=f"av{h}", tag="av")
                    avs.append(av)
                nkt = 4 * Q + 4
                for kt in range(nkt):
                    p_ = kt - 4 * Q
                    c0 = 128 * p_ if p_ > 0 else 0
                    sps = []
                    for pr in range(2):
                        sc2 = scp.tile([P, 2, 512], f32, name="sc2", tag="sc")
                        for i in range(2):
                            h = 2 * pr + i
                            hb = (h % 2) * 64
                            j = h // 2
                            nc.tensor.matmul(
                                sc2[:, i, c0:512],
                                kT[hb:hb + 64, j, kt * P:(kt + 1) * P],
                                qT[hb:hb + 64, j, q0 + c0:q0 + 512],
                                start=True, stop=True)
                        sps.append(sc2)
                    for pr in range(2):
                        ex2 = expp.tile([P, 2, 512], bf16, name="ex2",
                                        tag="ex")
                        nc.scalar.activation(ex2[:, :, c0:512],
                                             sps[pr][:, :, c0:512], AF.Exp)
                        for i in range(2):
                            h = 2 * pr + i
                            if p_ >= 0:
                                nc.vector.tensor_mul(ex2[:, i, c0:c0 + 128],
                                                     ex2[:, i, c0:c0 + 128],
                                                     diagm[:])
                            nc.tensor.matmul(
                                avs[h][0:65, c0:512],
                                v_aug[:, h, kt, 0:65],
                                ex2[:, i, c0:512],
                                start=(kt == 0), stop=(kt == nkt - 1),
                                skip_group_check=True)
                rsT = span_tail(Q, avs, mkpsum_pair)
                if rsT is not None:
                    rsT0 = rsT
                    if not DBG:
                        wuw[2] = emit_wuw(2)
                        wuw[3] = emit_wuw(3)

            scp_cm.__exit__(None, None, None)

            # ---- span 3: single scores + strip0 MLP fill ----
            with tc.tile_pool(name="sc3p", bufs=2, space="PSUM") as scps, \
                 tc.tile_pool(name="upps", bufs=2, space="PSUM") as upps:
                upps_ref = [upps]

                def fill_slot(kt):
                    nonlocal st0
                    if DBG:
                        return
                    if kt == 4:
                        st0 = emit_strip_pre(0, rsT0, upps, gate_ex=last_ex)
                        return
                    if st0 is None or upf[0] >= NFT:
                        return
                    for _ in range(2):
                        if upf[0] >= NFT:
                            break
                        emit_up_group(st0, upf[0], wuw[upf[0] // 8],
                                      defer_gelu=True)
                        upf[0] += 1

                Q = 3
                q0 = Q * SPAN
                avs = []
                for h in range(HC):
                    av = avps.tile([P, 512], f32, name=f"av{h}", tag="av")
                    avs.append(av)
                nkt = 16
                for kt in range(nkt):
                    p_ = kt - 4 * Q
                    c0 = 128 * p_ if p_ > 0 else 0
                    scs = []
                    for h in range(HC):
                        hb = (h % 2) * 64
                        j = h // 2
                        sc = scps.tile([P, 512], f32, name="sc", tag="sc")
                        nc.tensor.matmul(
                            sc[:, c0:512],
                            kT[hb:hb + 64, j, kt * P:(kt + 1) * P],
                            qT[hb:hb + 64, j, q0 + c0:q0 + 512],
                            start=True, stop=True)
                        scs.append(sc)
                    for h in range(HC):
                        ex = expp.tile([P, 512], bf16, name="ex", tag="ex")
                        last_ex = ex
                        nc.scalar.activation(ex[:, c0:512], scs[h][:, c0:512],
                                             AF.Exp)
                        if p_ >= 0:
                            nc.vector.tensor_mul(ex[:, c0:c0 + 128],
                                                 ex[:, c0:c0 + 128], diagm[:])
                        nc.tensor.matmul(
                            avs[h][0:65, c0:512],
                            v_aug[:, h, kt, 0:65],
                            ex[:, c0:512],
                            start=(kt == 0), stop=(kt == nkt - 1),
                            skip_group_check=True)
                    fill_slot(kt)

                def mkpsum_single():
                    return scps.tile([P, 512], f32, name="pp", tag="sc")

                rsT1 = span_tail(3, avs, mkpsum_single)
                if not DBG:
                    for w in range(4, 8):
                        wuw[w] = emit_wuw(w)
                # rest of strip0's ups (overlap RS2 on the collective cores)
                if not DBG and st0 is not None:
                    while upf[0] < NFT:
                        emit_up_group(st0, upf[0], wuw[upf[0] // 8],
                                      defer_gelu=True)
                        upf[0] += 1
                # gate = 1.0, data-dependent on the last exp: ops scaled by
                # it cannot be scheduled into the attention exp stream
                nc.vector.tensor_scalar(gate[:], last_ex[:, 511:512], 0.0,
                                        1.0, mybir.AluOpType.mult,
                                        mybir.AluOpType.add)
                if DBG:
                    rsT0x = rsT0 if rsT0 is not None else emit_rs_transpose(0)
                    emit_strip_pre(0, rsT0x, upps)
                    emit_strip_pre(1, rsT1, upps)

        bpool_cm.__exit__(None, None, None)   # free qT / kT / v_aug / wp_sb

        # ---------------- strip0 down + full strip1 ----------------
        if not DBG:
            with tc.tile_pool(name="dnps", bufs=2, space="PSUM") as dnps, \
                 tc.tile_pool(name="up2", bufs=2, space="PSUM") as up2:
                upps_ref[0] = up2
                # bulk gelu for strip0's staged pre-activations (one table
                # switch, after all attention exps are done; gate enforces it)
                for f in range(NFT):
                    nc.scalar.activation(st0["hT"][:, f, :], st0["hT"][:, f, :],
                                         AF.Gelu, bias=bup[:, f:f + 1],
                                         scale=gate[:, 0:1])
                emit_down_out(st0, dnps)
                st1 = emit_strip_pre(1, rsT1, up2)
                for f in range(NFT):
                    emit_up_group(st1, f, wuw[4 + f // 8])
                emit_down_out(st1, dnps)


def _prep_inputs(x, ln1_w, ln1_b, c_attn_w, c_attn_b, c_proj_w, c_proj_b,
                 ln2_w, ln2_b, up_w, up_b, down_w, down_b):
    """Host-side preprocessing -> list of 8 per-core input dicts."""
    x = np.asarray(x, np.float32)
    f64 = np.float64
    mu = x.mean(-1, keepdims=True, dtype=f64)
    var = np.asarray(x, f64).var(-1, keepdims=True)
    g = ((x - mu) / np.sqrt(var + EPS)).astype(np.float32)     # [B, T, C]

    ln1_w = np.asarray(ln1_w, np.float32); ln1_b = np.asarray(ln1_b, np.float32)
    ln2_w = np.asarray(ln2_w, np.float32); ln2_b = np.asarray(ln2_b, np.float32)
    c_attn_w = np.asarray(c_attn_w, np.float32)
    c_attn_b = np.asarray(c_attn_b, np.float32)
    c_proj_w = np.asarray(c_proj_w, np.float32)
    c_proj_b = np.asarray(c_proj_b, np.float32)
    up_w = np.asarray(up_w, np.float32); up_b = np.asarray(up_b, np.float32)
    down_w = np.asarray(down_w, np.float32)
    down_b = np.asarray(down_b, np.float32)

    Wa = c_attn_w * ln1_w[None, :]                  # fold LN1 scale
    ba = c_attn_b + c_attn_w @ ln1_b                # fold LN1 shift
    Wq, Wk, Wv = Wa[:C], Wa[C:2 * C], Wa[2 * C:]
    bq, bk, bv = ba[:C], ba[C:2 * C], ba[2 * C:]
    s = 1.0 / np.sqrt(D)
    Wq = Wq * s; bq = bq * s                        # fold attention scale

    Wup = up_w * ln2_w[None, :]
    bupv = up_b + up_w @ ln2_b

    def b2t(v, n):   # per-partition bias layout [128, n]
        return np.ascontiguousarray(v.reshape(n, P).T.astype(np.float32))

    diagm = np.triu(np.ones((P, P), np.float32))    # kv row <= q col

    shared = {
        "WupT": np.ascontiguousarray(Wup.T).astype(BF),
        "WdownT": np.ascontiguousarray(down_w.T).astype(BF),
        "bup": b2t(bupv, 32), "bdown": b2t(down_b, 8),
        "diagm": diagm.astype(BF),
    }

    # residual with proj bias and (v-bias pushed through proj) folded in
    xb = x + (c_proj_b + c_proj_w @ bv)[None, None, :]

    in_maps, tok_slices = [], []
    for core in range(8):
        b, i = core // 4, core % 4
        ch = slice(i * 256, (i + 1) * 256)          # this core's attn channels
        wqk = np.concatenate([Wq[ch], Wk[ch]], axis=0)      # [512, 1024]
        m = dict(shared)
        m["WqkT"] = np.ascontiguousarray(wqk.T).astype(BF)
        m["WvT"] = np.ascontiguousarray(Wv[ch].T).astype(BF)
        m["WpT"] = np.ascontiguousarray(c_proj_w[:, ch].T).astype(BF)
        m["bqk"] = b2t(np.concatenate([bq[ch], bk[ch]]), 4)
        m["gT"] = np.ascontiguousarray(g[b].T).astype(BF)
        strips = [slice(1024 * r + STRIP * i, 1024 * r + STRIP * (i + 1))
                  for r in range(2)]
        xbT = np.concatenate([xb[b, st].T for st in strips], axis=1)
        m["xbT"] = np.ascontiguousarray(xbT).astype(np.float32)
        in_maps.append(m)
        tok_slices.append((b, strips))
    return in_maps, tok_slices


def kernel(**inputs):
    global _CACHED_NC
    if _CACHED_NC is None:
        _CACHED_NC = _build_nc()
    nc = _CACHED_NC
    in_maps, tok_slices = _prep_inputs(**inputs)
    try:
        res = run_bass_kernel_spmd(nc, in_maps, list(range(8)))
    except Exception:
        # one retry: transient NRT device faults are recoverable on re-run
        res = run_bass_kernel_spmd(nc, in_maps, list(range(8)))
    out = np.empty((B, T, C), np.float32)
    for core in range(8):
        o = res.results[core]["OUT"]                # [C, 512]
        b, strips = tok_slices[core]
        for r, st in enumerate(strips):
            out[b, st, :] = o[:, r * STRIP:(r + 1) * STRIP].T
    return out


# revision 4
# speedup vs baseline: 1.0304x; 1.0008x over previous
"""Trainium2 Bass kernel for a GPT-style transformer block (B=2, T=2048,
C=1024, 16 heads, MLP 4x), sharded across 8 NeuronCores.

Sharding: attention is HEAD-sharded. Core c = (b, i) with b = c//4,
i = c%4 owns heads [4i, 4i+4) of batch b for the whole 2048-token range:
it computes q/k/v for just those 256 channels (same FLOPs as a row shard
of all channels), runs causal attention with no cross-core kv exchange,
then produces token-major c_proj partials. Two ReduceScatters (one per
1024-token chunk) sum the partials over the 4-core batch group and hand
each core a contiguous 256-token strip per chunk; LN2+MLP then run
data-parallel on the core's two strips.

Host precomputes LN1 (inputs-only), folds LN scale/shift into the matmul
weights, folds 1/sqrt(D) into Wq, and folds the v-bias + proj-bias into
the residual (softmax weights sum to 1, so attn_out = av/den + bv and
proj(attn_out) = proj(av/den) + Wp@bv). Causality lives in the loop
bounds; only diagonal 128x128 tiles get a multiplicative triu mask.
"""
import numpy as np
import ml_dtypes

import concourse.bass as bass
import concourse.mybir as mybir
import concourse.tile as tile
import concourse.bacc as bacc
from concourse.bass_utils import run_bass_kernel_spmd

BF = ml_dtypes.bfloat16
P = 128
B, T, C, D, F = 2, 2048, 1024, 64, 4096
NCT = C // P            # 8   c-tiles
NFT = F // P            # 32  f-tiles
NTT = T // P            # 16  token tiles
HC = 4                  # heads per core
SPAN = 512              # q/token span
NSPAN = T // SPAN       # 4
STRIP = 256             # tokens owned per core per RS chunk
EPS = 1e-5
f32 = mybir.dt.float32
bf16 = mybir.dt.bfloat16
AF = mybir.ActivationFunctionType

_CACHED_NC = None
import os as _os
DBG = _os.environ.get("KDBG", "")


def _build_nc():
    nc = bacc.Bacc("TRN2", target_bir_lowering=False, debug=False)
    d = {}
    for name, shape, dt in [
        ("gT", [C, T], bf16),          # ln1(x) for the whole batch, ch-major
        ("WqkT", [C, 512], bf16),      # q(256) | k(256) out channels
        ("WvT", [C, 256], bf16),
        ("WpT", [256, C], bf16),       # [attn-ch, out-ch] for moving use
        ("WupT", [C, F], bf16),
        ("WdownT", [F, C], bf16),
        ("bqk", [P, 4], f32),
        ("bup", [P, 32], f32),
        ("bdown", [P, 8], f32),
        ("xbT", [C, 512], f32),        # residual for own tokens (biases folded)
        ("diagm", [P, P], bf16),       # triu causal mask for diagonal tiles
    ]:
        d[name] = nc.dram_tensor(name, shape, dt, kind="ExternalInput").ap()
    d["OUT"] = nc.dram_tensor("OUT", [C, 512], f32, kind="ExternalOutput").ap()

    with tile.TileContext(nc) as tc:
        _emit(nc, tc, d)
    nc.compile()
    return nc


def _emit(nc, tc, d):
    from contextlib import ExitStack

    with ExitStack() as ctx:
        # ---------------- long-lived tiles ----------------
        cpool = ctx.enter_context(tc.tile_pool(name="cpool", bufs=1))
        attnT = cpool.tile([P, 2, T], bf16, name="attnT")
        bqk = cpool.tile([P, 4], f32, name="bqk")
        bup = cpool.tile([P, 32], f32, name="bup")
        bdown = cpool.tile([P, 8], f32, name="bdown")
        diagm = cpool.tile([P, P], bf16, name="diagm")
        epsT = cpool.tile([P, 1], f32, name="epsT")
        onesb = cpool.tile([P, P], bf16, name="onesb")
        selp = cpool.tile([P, P], bf16, name="selp")
        gate = cpool.tile([P, 1], f32, name="gate")
        nc.vector.memset(epsT[:], EPS)
        nc.vector.memset(onesb[:], 1.0)
        nc.vector.memset(selp[:], 0.0)
        nc.vector.memset(selp[0:1, 0:64], 1.0)
        nc.vector.memset(selp[64:65, 64:128], 1.0)

        # DRAM scratch for the proj partials + RS outputs
        dramp = ctx.enter_context(tc.tile_pool(name="dramp", bufs=1,
                                               space="DRAM"))
        partials = [dramp.tile([1024, C], bf16, name=f"part{r}")
                    for r in range(2)]
        rsout = [dramp.tile([STRIP, C], bf16, name=f"rsout{r}")
                 for r in range(2)]

        # MLP weights / residual / strip tiles: allocated BEFORE the
        # attention pools so strip-0's MLP can run while attention finishes
        # (tiles in later pools would inherit waits on attention SBUF reuse).
        wmlp = ctx.enter_context(tc.tile_pool(name="wmlp", bufs=1))
        wd = wmlp.tile([P, NFT, C], bf16, name="wd")
        x1T = wmlp.tile([P, NCT, 512], f32, name="x1T")    # xb, then x1
        mep = ctx.enter_context(tc.tile_pool(name="mep", bufs=1))

        # attention operand tiles (freed after proj)
        bpool_cm = tc.tile_pool(name="bpool", bufs=1)
        bpool = bpool_cm.__enter__()
        qT = bpool.tile([P, 2, T], bf16, name="qT")
        kT = bpool.tile([P, 2, T], bf16, name="kT")
        v_aug = bpool.tile([P, HC, NTT, 65], bf16, name="v_aug")
        wp_sb = bpool.tile([P, 2, C], bf16, name="wp_sb")
        nc.vector.memset(v_aug[:, :, :, 64:65], 1.0)

        # ---------------- qkv projections ----------------
        with tc.tile_pool(name="gpool", bufs=2) as gpool, \
             tc.tile_pool(name="wqkp", bufs=1) as wqkp, \
             tc.tile_pool(name="qkps", bufs=3, space="PSUM") as qkps:
            wqk = wqkp.tile([P, NCT, 512], bf16, name="wqk")
            wv = wqkp.tile([P, NCT, 256], bf16, name="wv")
            wqksrc = d["WqkT"].rearrange("(ct p) o -> p ct o", p=P)
            gsrc = d["gT"].rearrange("(ct p) t -> p ct t", p=P)
            # startup-critical order: q weights, first g chunk, then the rest
            nc.sync.dma_start(wqk[:, 0:4, 0:256], wqksrc[:, 0:4, 0:256])
            g0 = gpool.tile([P, NCT, 256], bf16, name="g", tag="g")
            nc.sync.dma_start(g0[:, 0:4, :], gsrc[:, 0:4, 0:256])
            nc.sync.dma_start(wqk[:, 4:8, 0:256], wqksrc[:, 4:8, 0:256])
            nc.sync.dma_start(g0[:, 4:8, :], gsrc[:, 4:8, 0:256])
            nc.sync.dma_start(wqk[:, :, 256:512], wqksrc[:, :, 256:512])
            nc.sync.dma_start(wv[:],
                              d["WvT"].rearrange("(ct p) o -> p ct o", p=P))
            for t, key in [(bqk, "bqk"), (bup, "bup"), (bdown, "bdown"),
                           (diagm, "diagm")]:
                nc.sync.dma_start(t[:], d[key])
            nc.sync.dma_start(wp_sb[:],
                              d["WpT"].rearrange("(j p) o -> p j o", p=P))
            for hh in range(2 * NSPAN):      # half-spans of 256 tokens
                c0 = hh * 256
                if hh == 0:
                    g = g0
                else:
                    g = gpool.tile([P, NCT, 256], bf16, name="g", tag="g")
                    nc.sync.dma_start(g[:], gsrc[:, :, c0:c0 + 256])
                for ot in range(4):          # q0 q1 k0 k1
                    ps = qkps.tile([P, 256], f32, name="ps", tag="qk")
                    for ct in range(NCT):
                        nc.tensor.matmul(ps[:], wqk[:, ct, ot * P:(ot + 1) * P],
                                         g[:, ct, :],
                                         start=(ct == 0), stop=(ct == NCT - 1))
                    dstT = qT if ot < 2 else kT
                    nc.scalar.add(dstT[:, ot % 2, c0:c0 + 256],
                                  ps[:], bqk[:, ot:ot + 1])
                for tl in range(2):          # v, token tile kt = 2*hh+tl
                    kt = 2 * hh + tl
                    pv = qkps.tile([P, 256], f32, name="pv", tag="qk")
                    for ct in range(NCT):
                        nc.tensor.matmul(pv[:], g[:, ct, tl * P:(tl + 1) * P],
                                         wv[:, ct, :],
                                         start=(ct == 0), stop=(ct == NCT - 1))
                    nc.scalar.copy(
                        v_aug[:, :, kt, 0:64],
                        pv[:].rearrange("p (h dd) -> p h dd", dd=64))
            # stage the big loads (SP queue, after the startup-critical ones)
            wdsrc = d["WdownT"].rearrange("(cf p) o -> p cf o", p=P)
            for cc in range(8):
                nc.sync.dma_start(wd[:, cc * 4:(cc + 1) * 4, :],
                                  wdsrc[:, cc * 4:(cc + 1) * 4, :])
            nc.sync.dma_start(x1T[:],
                              d["xbT"].rearrange("(ct p) t -> p ct t", p=P))

        # ----- strip-MLP emission helpers -----
        wusrc = d["WupT"].rearrange("(ct p) f -> p ct f", p=P)
        outdst = d["OUT"].rearrange("(ot p) t -> p ot t", p=P)

        def emit_wuw(w):
            t = mep.tile([P, NCT, 1024], bf16, name="wuw", tag="wuw", bufs=2)
            nc.sync.dma_start(
                t[:], wusrc[:, :, (w % 4) * 1024:((w % 4) + 1) * 1024])
            return t

        def emit_rs_transpose(r):
            rsT = mep.tile([P, NCT, STRIP], bf16, name="rsT", tag="rsT")
            nc.sync.dma_start_transpose(rsT[:], rsout[r][:])
            return rsT

        def emit_strip_pre(r, rsT, upps, gate_ex=None):
            cs = r * STRIP
            if gate_ex is not None:
                # scheduler-proofing: root the chain on a late attention exp
                # so its long RS-wait cannot head-block the DVE queue ahead
                # of ready attention work (the scheduler's internal sim
                # underestimates collective latency)
                gate2 = mep.tile([P, 1], f32, name="gate2", tag="gate2")
                nc.vector.tensor_scalar(gate2[:], gate_ex[:, 511:512],
                                        0.0, 1.0, mybir.AluOpType.mult,
                                        mybir.AluOpType.add)
                rsTg = mep.tile([P, NCT, STRIP], bf16, name="rsTg",
                                tag="x1b")
                nc.vector.tensor_scalar(rsTg[:], rsT[:], gate2[:, 0:1], None,
                                        mybir.AluOpType.mult)
                rsT = rsTg
            nc.vector.tensor_add(x1T[:, :, cs:cs + STRIP],
                                 x1T[:, :, cs:cs + STRIP], rsT[:])
            if DBG == "x1":
                nc.sync.dma_start(outdst[:, :, cs:cs + STRIP],
                                  x1T[:, :, cs:cs + STRIP])
                return None
            x1b = mep.tile([P, NCT, STRIP], bf16, name="x1b", tag="x1b")
            sqb = mep.tile([P, NCT, STRIP], bf16, name="sqb", tag="g28")
            for ct in range(NCT):
                nc.vector.tensor_copy(x1b[:, ct, :], x1T[:, ct, cs:cs + STRIP])
                nc.vector.tensor_mul(sqb[:, ct, :], x1T[:, ct, cs:cs + STRIP],
                                     x1T[:, ct, cs:cs + STRIP])
            psmu = upps.tile([P, STRIP], f32, name="psmu", tag="pu")
            for ct in range(NCT):
                nc.tensor.matmul(psmu[:], onesb[:], x1b[:, ct, :],
                                 start=(ct == 0), stop=(ct == NCT - 1))
            pssq = upps.tile([P, STRIP], f32, name="pssq", tag="pu")
            for ct in range(NCT):
                nc.tensor.matmul(pssq[:], onesb[:], sqb[:, ct, :],
                                 start=(ct == 0), stop=(ct == NCT - 1))
            mu = mep.tile([P, STRIP], f32, name="mu", tag="mu")
            e2 = mep.tile([P, STRIP], f32, name="e2", tag="e2")
            std = mep.tile([P, STRIP], f32, name="std", tag="stdt")
            nc.scalar.mul(mu[:], psmu[:], 1.0 / C)
            nc.scalar.mul(e2[:], pssq[:], 1.0 / C)
            musq = mep.tile([P, STRIP], f32, name="musq", tag="tmpc", bufs=2)
            nc.vector.tensor_mul(musq[:], mu[:], mu[:])
            nc.vector.tensor_sub(e2[:], e2[:], musq[:])
            nc.scalar.activation(std[:], e2[:], AF.Sqrt, bias=epsT[:])
            nc.vector.reciprocal(std[:], std[:])
            g2 = mep.tile([P, NCT, STRIP], bf16, name="g2", tag="g28")
            for ct in range(NCT):
                tmpc = mep.tile([P, STRIP], f32, name="tmpc", tag="tmpc",
                                bufs=2)
                nc.vector.tensor_sub(tmpc[:], x1T[:, ct, cs:cs + STRIP],
                                     mu[:])
                nc.vector.tensor_mul(g2[:, ct, :], tmpc[:], std[:])
            if DBG == "g2":
                g2f = mep.tile([P, NCT, STRIP], f32, name="g2f", tag="g2f")
                nc.vector.tensor_copy(g2f[:], g2[:])
                nc.sync.dma_start(outdst[:, :, cs:cs + STRIP], g2f[:])
                return None
            hT = mep.tile([P, NFT, STRIP], bf16, name="hT", tag="hT")
            return dict(cs=cs, g2=g2, hT=hT)

        def emit_up_group(st, f, wt, defer_gelu=False):
            pu = upps_ref[0].tile([P, STRIP], f32, name="pu", tag="pu")
            fl = f % 8
            for ct in range(NCT):
                nc.tensor.matmul(pu[:], wt[:, ct, fl * P:(fl + 1) * P],
                                 st["g2"][:, ct, :],
                                 start=(ct == 0), stop=(ct == NCT - 1))
            if defer_gelu:
                # keep Gelu out of the attention window: its act table
                # does not share a set with Exp, so inline gelus thrash
                # 1.28us table loads per switch.  Stage raw pre-act.
                nc.vector.tensor_copy(st["hT"][:, f, :], pu[:])
            else:
                nc.scalar.activation(st["hT"][:, f, :], pu[:], AF.Gelu,
                                     bias=bup[:, f:f + 1])

        def emit_down_out(st, dnps):
            for ot in range(8):
                pd = dnps.tile([P, STRIP], f32, name="pd", tag="dn")
                for cf in range(NFT):
                    nc.tensor.matmul(pd[:], wd[:, cf, ot * P:(ot + 1) * P],
                                     st["hT"][:, cf, :],
                                     start=(cf == 0), stop=(cf == NFT - 1))
                td = mep.tile([P, STRIP], f32, name="td", tag="td", bufs=2)
                nc.scalar.add(td[:], pd[:], bdown[:, ot:ot + 1])
                ox = mep.tile([P, STRIP], f32, name="ox", tag="ox", bufs=2)
                nc.vector.tensor_add(ox[:], td[:],
                                     x1T[:, ot, st["cs"]:st["cs"] + STRIP])
                nc.gpsimd.dma_start(outdst[:, ot, st["cs"]:st["cs"] + STRIP],
                                    ox[:])

        # ---------------- attention + proj + RS (+ strip0 MLP fill) --------
        wuw = {}
        if not DBG:
            wuw[0] = emit_wuw(0)
            wuw[1] = emit_wuw(1)
        st0 = None
        rsT0 = None
        rsT1 = None
        last_ex = None
        upf = [0]        # next strip0 up f-group to emit

        with tc.tile_pool(name="avps", bufs=4, space="PSUM") as avps, \
             tc.tile_pool(name="expp", bufs=3) as expp, \
             tc.tile_pool(name="nrmp", bufs=2) as nrmp, \
             tc.tile_pool(name="prst", bufs=2) as prst:

            def span_tail(Q, avs, mkpsum):
                # normalize: attnT[:, pr, span] = av / den, then proj + RS
                q0 = Q * SPAN
                for pr in range(2):
                    denb = nrmp.tile([P, 512], bf16, name="denb", tag="denb")
                    nc.vector.memset(denb[0:65, :], 0.0)
                    with nc.allow_low_precision(reason="bf16 softmax denom"):
                        nc.vector.reciprocal(denb[0:1, :],
                                             avs[2 * pr][64:65, :])
                        nc.vector.reciprocal(denb[64:65, :],
                                             avs[2 * pr + 1][64:65, :])
                    bc = mkpsum()
                    nc.tensor.matmul(bc[:], selp[0:65, :], denb[0:65, :],
                                     start=True, stop=True)
                    bcs = nrmp.tile([P, 512], f32, name="bcs", tag="bcs")
                    nc.scalar.copy(bcs[:], bc[:])
                    nc.vector.tensor_mul(attnT[0:64, pr, q0:q0 + SPAN],
                                         avs[2 * pr][0:64, :], bcs[0:64, :])
                    nc.vector.tensor_mul(attnT[64:128, pr, q0:q0 + SPAN],
                                         avs[2 * pr + 1][0:64, :],
                                         bcs[64:128, :])
                for tl in range(4):
                    tt = 4 * Q + tl
                    stage = prst.tile([P, 1024], bf16, name="stage", tag="st")
                    for half in range(2):
                        pp = avps.tile([P, 512], f32, name="pp", tag="av")
                        for j in range(2):
                            nc.tensor.matmul(
                                pp[:],
                                attnT[:, j, tt * P:(tt + 1) * P],
                                wp_sb[:, j, half * 512:(half + 1) * 512],
                                start=(j == 0), stop=(j == 1))
                        nc.vector.tensor_copy(
                            stage[:, half * 512:(half + 1) * 512], pp[:])
                    row = (Q % 2) * 512 + tl * P
                    nc.gpsimd.dma_start(partials[Q // 2][row:row + P, :],
                                        stage[:])
                if Q % 2 == 1:
                    r = Q // 2
                    nc.gpsimd.collective_compute(
                        "ReduceScatter", mybir.AluOpType.add,
                        ins=[partials[r].opt()], outs=[rsout[r].opt()],
                        replica_groups=[[0, 1, 2, 3], [4, 5, 6, 7]])
                    return emit_rs_transpose(r)
                return None

            # ---- spans 0-2: paired scores, one exp per head pair ----
            scp_cm = tc.tile_pool(name="scp", bufs=2, space="PSUM")
            scp = scp_cm.__enter__()

            def mkpsum_pair():
                t = scp.tile([P, 2, 512], f32, name="sc2", tag="sc")
                return t[:, 0, :]

            for Q in range(3):
                q0 = Q * SPAN
                avs = []
                for h in range(HC):
                    av = avps.tile([P, 512], f32, name=f"av{h}", tag="av")
                    avs.append(av)
                nkt = 4 * Q + 4
                for kt in range(nkt):
                    p_ = kt - 4 * Q
                    c0 = 128 * p_ if p_ > 0 else 0
                    sps = []
                    for pr in range(2):
                        sc2 = scp.tile([P, 2, 512], f32, name="sc2", tag="sc")
                        for i in range(2):
                            h = 2 * pr + i
                            hb = (h % 2) * 64
                            j = h // 2
                            nc.tensor.matmul(
                                sc2[:, i, c0:512],
                                kT[hb:hb + 64, j, kt * P:(kt + 1) * P],
                                qT[hb:hb + 64, j, q0 + c0:q0 + 512],
                                start=True, stop=True)
                        sps.append(sc2)
                    for pr in range(2):
                        ex2 = expp.tile([P, 2, 512], bf16, name="ex2",
                                        tag="ex")
                        nc.scalar.activation(ex2[:, :, c0:512],
                                             sps[pr][:, :, c0:512], AF.Exp)
                        for i in range(2):
                            h = 2 * pr + i
                            if p_ >= 0:
                                nc.vector.tensor_mul(ex2[:, i, c0:c0 + 128],
                                                     ex2[:, i, c0:c0 + 128],
                                                     diagm[:])
                            nc.tensor.matmul(
                                avs[h][0:65, c0:512],
                                v_aug[:, h, kt, 0:65],
                                ex2[:, i, c0:512],
                                start=(kt == 0), stop=(kt == nkt - 1),
                                skip_group_check=True)
                rsT = span_tail(Q, avs, mkpsum_pair)
                if rsT is not None:
                    rsT0 = rsT
                    if not DBG:
                        wuw[2] = emit_wuw(2)
                        wuw[3] = emit_wuw(3)

            scp_cm.__exit__(None, None, None)

            # ---- span 3: single scores + strip0 MLP fill ----
            with tc.tile_pool(name="sc3p", bufs=2, space="PSUM") as scps, \
                 tc.tile_pool(name="upps", bufs=2, space="PSUM") as upps:
                upps_ref = [upps]

                def fill_slot(kt):
                    nonlocal st0
                    if DBG:
                        return
                    if kt == 4:
                        st0 = emit_strip_pre(0, rsT0, upps, gate_ex=last_ex)
                        return
                    if st0 is None or upf[0] >= NFT:
                        return
                    for _ in range(2):
                        if upf[0] >= NFT:
                            break
                        emit_up_group(st0, upf[0], wuw[upf[0] // 8],
                                      defer_gelu=True)
                        upf[0] += 1

                Q = 3
                q0 = Q * SPAN
                avs = []
                for h in range(HC):
                    av = avps.tile([P, 512], f32, name=f"av{h}", tag="av")
                    avs.append(av)
                nkt = 16
                for kt in range(nkt):
                    p_ = kt - 4 * Q
                    c0 = 128 * p_ if p_ > 0 else 0
                    scs = []
                    for h in range(HC):
                        hb = (h % 2) * 64
                        j = h // 2
                        sc = scps.tile([P, 512], f32, name="sc", tag="sc")
                        nc.tensor.matmul(
                            sc[:, c0:512],
                            kT[hb:hb + 64, j, kt * P:(kt + 1) * P],
                            qT[hb:hb + 64, j, q0 + c0:q0 + 512],
                            start=True, stop=True)
                        scs.append(sc)
                    for h in range(HC):
                        ex = expp.tile([P, 512], bf16, name="ex", tag="ex")
                        last_ex = ex
                        nc.scalar.activation(ex[:, c0:512], scs[h][:, c0:512],
                                             AF.Exp)
                        if p_ >= 0:
                            nc.vector.tensor_mul(ex[:, c0:c0 + 128],
                                                 ex[:, c0:c0 + 128], diagm[:])
                        nc.tensor.matmul(
                            avs[h][0:65, c0:512],
                            v_aug[:, h, kt, 0:65],
                            ex[:, c0:512],
                            start=(kt == 0), stop=(kt == nkt - 1),
                            skip_group_check=True)
                    fill_slot(kt)

                def mkpsum_single():
                    return scps.tile([P, 512], f32, name="pp", tag="sc")

                rsT1 = span_tail(3, avs, mkpsum_single)
                if not DBG:
                    for w in range(4, 8):
                        wuw[w] = emit_wuw(w)
                # rest of strip0's ups (overlap RS2 on the collective cores)
                if not DBG and st0 is not None:
                    while upf[0] < NFT:
                        emit_up_group(st0, upf[0], wuw[upf[0] // 8],
                                      defer_gelu=True)
                        upf[0] += 1
                # gate = 1.0, data-dependent on the last exp: ops scaled by
                # it cannot be scheduled into the attention exp stream
                nc.vector.tensor_scalar(gate[:], last_ex[:, 511:512], 0.0,
                                        1.0, mybir.AluOpType.mult,
                                        mybir.AluOpType.add)
                if DBG:
                    rsT0x = rsT0 if rsT0 is not None else emit_rs_transpose(0)
                    emit_strip_pre(0, rsT0x, upps)
                    emit_strip_pre(1, rsT1, upps)

        bpool_cm.__exit__(None, None, None)   # free qT / kT / v_aug / wp_sb

        # ---------------- strip0 down + full strip1 ----------------
        if not DBG:
            with tc.tile_pool(name="dnps", bufs=2, space="PSUM") as dnps, \
                 tc.tile_pool(name="up2", bufs=2, space="PSUM") as up2:
                upps_ref[0] = up2
                # bulk gelu for strip0's staged pre-activations (one table
                # switch, after all attention exps are done; gate enforces it)
                for f in range(NFT):
                    nc.scalar.activation(st0["hT"][:, f, :], st0["hT"][:, f, :],
                                         AF.Gelu, bias=bup[:, f:f + 1],
                                         scale=gate[:, 0:1])
                emit_down_out(st0, dnps)
                st1 = emit_strip_pre(1, rsT1, up2)
                for f in range(NFT):
                    emit_up_group(st1, f, wuw[4 + f // 8])
                emit_down_out(st1, dnps)


def _prep_inputs(x, ln1_w, ln1_b, c_attn_w, c_attn_b, c_proj_w, c_proj_b,
                 ln2_w, ln2_b, up_w, up_b, down_w, down_b):
    """Host-side preprocessing -> list of 8 per-core input dicts."""
    x = np.asarray(x, np.float32)
    f64 = np.float64
    mu = x.mean(-1, keepdims=True, dtype=f64)
    var = np.asarray(x, f64).var(-1, keepdims=True)
    g = ((x - mu) / np.sqrt(var + EPS)).astype(np.float32)     # [B, T, C]

    ln1_w = np.asarray(ln1_w, np.float32); ln1_b = np.asarray(ln1_b, np.float32)
    ln2_w = np.asarray(ln2_w, np.float32); ln2_b = np.asarray(ln2_b, np.float32)
    c_attn_w = np.asarray(c_attn_w, np.float32)
    c_attn_b = np.asarray(c_attn_b, np.float32)
    c_proj_w = np.asarray(c_proj_w, np.float32)
    c_proj_b = np.asarray(c_proj_b, np.float32)
    up_w = np.asarray(up_w, np.float32); up_b = np.asarray(up_b, np.float32)
    down_w = np.asarray(down_w, np.float32)
    down_b = np.asarray(down_b, np.float32)

    Wa = c_attn_w * ln1_w[None, :]                  # fold LN1 scale
    ba = c_attn_b + c_attn_w @ ln1_b                # fold LN1 shift
    Wq, Wk, Wv = Wa[:C], Wa[C:2 * C], Wa[2 * C:]
    bq, bk, bv = ba[:C], ba[C:2 * C], ba[2 * C:]
    s = 1.0 / np.sqrt(D)
    Wq = Wq * s; bq = bq * s                        # fold attention scale

    Wup = up_w * ln2_w[None, :]
    bupv = up_b + up_w @ ln2_b

    def b2t(v, n):   # per-partition bias layout [128, n]
        return np.ascontiguousarray(v.reshape(n, P).T.astype(np.float32))

    diagm = np.triu(np.ones((P, P), np.float32))    # kv row <= q col

    shared = {
        "WupT": np.ascontiguousarray(Wup.T).astype(BF),
        "WdownT": np.ascontiguousarray(down_w.T).astype(BF),
        "bup": b2t(bupv, 32), "bdown": b2t(down_b, 8),
        "diagm": diagm.astype(BF),
    }

    # residual with proj bias and (v-bias pushed through proj) folded in
    xb = x + (c_proj_b + c_proj_w @ bv)[None, None, :]

    in_maps, tok_slices = [], []
    for core in range(8):
        b, i = core // 4, core % 4
        ch = slice(i * 256, (i + 1) * 256)          # this core's attn channels
        wqk = np.concatenate([Wq[ch], Wk[ch]], axis=0)      # [512, 1024]
        m = dict(shared)
        m["WqkT"] = np.ascontiguousarray(wqk.T).astype(BF)
        m["WvT"] = np.ascontiguousarray(Wv[ch].T).astype(BF)
        m["WpT"] = np.ascontiguousarray(c_proj_w[:, ch].T).astype(BF)
        m["bqk"] = b2t(np.concatenate([bq[ch], bk[ch]]), 4)
        m["gT"] = np.ascontiguousarray(g[b].T).astype(BF)
        strips = [slice(1024 * r + STRIP * i, 1024 * r + STRIP * (i + 1))
                  for r in range(2)]
        xbT = np.concatenate([xb[b, st].T for st in strips], axis=1)
        m["xbT"] = np.ascontiguousarray(xbT).astype(np.float32)
        in_maps.append(m)
        tok_slices.append((b, strips))
    return in_maps, tok_slices


def kernel(**inputs):
    global _CACHED_NC
    if _CACHED_NC is None:
        _CACHED_NC = _build_nc()
    nc = _CACHED_NC
    in_maps, tok_slices = _prep_inputs(**inputs)
    try:
        res = run_bass_kernel_spmd(nc, in_maps, list(range(8)))
    except Exception:
        # one retry: transient NRT device faults are recoverable on re-run
        res = run_bass_kernel_spmd(nc, in_maps, list(range(8)))
    out = np.empty((B, T, C), np.float32)
    for core in range(8):
        o = res.results[core]["OUT"]                # [C, 512]
        b, strips = tok_slices[core]
        for r, st in enumerate(strips):
            out[b, st, :] = o[:, r * STRIP:(r + 1) * STRIP].T
    return out


# revision 5
# speedup vs baseline: 1.0346x; 1.0041x over previous
"""Trainium2 Bass kernel for a GPT-style transformer block (B=2, T=2048,
C=1024, 16 heads, MLP 4x), sharded across 8 NeuronCores.

Sharding: attention is HEAD-sharded. Core c = (b, i) with b = c//4,
i = c%4 owns heads [4i, 4i+4) of batch b for the whole 2048-token range:
it computes q/k/v for just those 256 channels (same FLOPs as a row shard
of all channels), runs causal attention with no cross-core kv exchange,
then produces token-major c_proj partials. Two ReduceScatters (one per
1024-token chunk) sum the partials over the 4-core batch group and hand
each core a contiguous 256-token strip per chunk; LN2+MLP then run
data-parallel on the core's two strips.

Host precomputes LN1 (inputs-only), folds LN scale/shift into the matmul
weights, folds 1/sqrt(D) into Wq, and folds the v-bias + proj-bias into
the residual (softmax weights sum to 1, so attn_out = av/den + bv and
proj(attn_out) = proj(av/den) + Wp@bv). Causality lives in the loop
bounds; only diagonal 128x128 tiles get a multiplicative triu mask.
"""
import numpy as np
import ml_dtypes

import concourse.bass as bass
import concourse.mybir as mybir
import concourse.tile as tile
import concourse.bacc as bacc
from concourse.bass_utils import run_bass_kernel_spmd

BF = ml_dtypes.bfloat16
P = 128
B, T, C, D, F = 2, 2048, 1024, 64, 4096
NCT = C // P            # 8   c-tiles
NFT = F // P            # 32  f-tiles
NTT = T // P            # 16  token tiles
HC = 4                  # heads per core
SPAN = 512              # q/token span
NSPAN = T // SPAN       # 4
STRIP = 256             # tokens owned per core per RS chunk
EPS = 1e-5
f32 = mybir.dt.float32
bf16 = mybir.dt.bfloat16
AF = mybir.ActivationFunctionType

_CACHED_NC = None
import os as _os
DBG = _os.environ.get("KDBG", "")


def _build_nc():
    nc = bacc.Bacc("TRN2", target_bir_lowering=False, debug=False)
    d = {}
    for name, shape, dt in [
        ("gT", [C, T], bf16),          # ln1(x) for the whole batch, ch-major
        ("WqkT", [C, 512], bf16),      # q(256) | k(256) out channels
        ("WvT", [C, 256], bf16),
        ("WpT", [256, C], bf16),       # [attn-ch, out-ch] for moving use
        ("WupT", [C, F], bf16),
        ("WdownT", [F, C], bf16),
        ("bqk", [P, 4], f32),
        ("bup", [P, 32], f32),
        ("bdown", [P, 8], f32),
        ("xbT", [C, 512], f32),        # residual for own tokens (biases folded)
        ("diagm", [P, P], bf16),       # triu causal mask for diagonal tiles
    ]:
        d[name] = nc.dram_tensor(name, shape, dt, kind="ExternalInput").ap()
    d["OUT"] = nc.dram_tensor("OUT", [C, 512], f32, kind="ExternalOutput").ap()

    with tile.TileContext(nc) as tc:
        _emit(nc, tc, d)
    nc.compile()
    return nc


def _emit(nc, tc, d):
    from contextlib import ExitStack

    with ExitStack() as ctx:
        # ---------------- long-lived tiles ----------------
        cpool = ctx.enter_context(tc.tile_pool(name="cpool", bufs=1))
        attnT = cpool.tile([P, 2, T], bf16, name="attnT")
        bqk = cpool.tile([P, 4], f32, name="bqk")
        bup = cpool.tile([P, 32], f32, name="bup")
        bdown = cpool.tile([P, 8], f32, name="bdown")
        diagm = cpool.tile([P, P], bf16, name="diagm")
        epsT = cpool.tile([P, 1], f32, name="epsT")
        onesb = cpool.tile([P, P], bf16, name="onesb")
        selp = cpool.tile([P, P], bf16, name="selp")
        gate = cpool.tile([P, 1], f32, name="gate")
        nc.vector.memset(epsT[:], EPS)
        nc.vector.memset(onesb[:], 1.0)
        nc.vector.memset(selp[:], 0.0)
        nc.vector.memset(selp[0:1, 0:64], 1.0)
        nc.vector.memset(selp[64:65, 64:128], 1.0)

        # DRAM scratch for the proj partials + RS outputs
        dramp = ctx.enter_context(tc.tile_pool(name="dramp", bufs=1,
                                               space="DRAM"))
        partials = [dramp.tile([1024, C], bf16, name=f"part{r}")
                    for r in range(2)]
        rsout = [dramp.tile([STRIP, C], bf16, name=f"rsout{r}")
                 for r in range(2)]

        # MLP weights / residual / strip tiles: allocated BEFORE the
        # attention pools so strip-0's MLP can run while attention finishes
        # (tiles in later pools would inherit waits on attention SBUF reuse).
        wmlp = ctx.enter_context(tc.tile_pool(name="wmlp", bufs=1))
        wd = wmlp.tile([P, NFT, C], bf16, name="wd")
        x1T = wmlp.tile([P, NCT, 512], f32, name="x1T")    # xb, then x1
        mep = ctx.enter_context(tc.tile_pool(name="mep", bufs=1))

        # attention operand tiles (freed after proj)
        bpool_cm = tc.tile_pool(name="bpool", bufs=1)
        bpool = bpool_cm.__enter__()
        qT = bpool.tile([P, 2, T], bf16, name="qT")
        kT = bpool.tile([P, 2, T], bf16, name="kT")
        v_aug = bpool.tile([P, HC, NTT, 65], bf16, name="v_aug")
        wp_sb = bpool.tile([P, 2, C], bf16, name="wp_sb")
        nc.vector.memset(v_aug[:, :, :, 64:65], 1.0)

        # ---------------- qkv projections ----------------
        with tc.tile_pool(name="gpool", bufs=2) as gpool, \
             tc.tile_pool(name="wqkp", bufs=1) as wqkp, \
             tc.tile_pool(name="qkps", bufs=3, space="PSUM") as qkps:
            wqk = wqkp.tile([P, NCT, 512], bf16, name="wqk")
            wv = wqkp.tile([P, NCT, 256], bf16, name="wv")
            wqksrc = d["WqkT"].rearrange("(ct p) o -> p ct o", p=P)
            gsrc = d["gT"].rearrange("(ct p) t -> p ct t", p=P)
            # startup-critical order: q weights, first g chunk, then the rest
            nc.sync.dma_start(wqk[:, 0:4, 0:256], wqksrc[:, 0:4, 0:256])
            g0 = gpool.tile([P, NCT, 256], bf16, name="g", tag="g")
            nc.sync.dma_start(g0[:, 0:4, :], gsrc[:, 0:4, 0:256])
            nc.sync.dma_start(wqk[:, 4:8, 0:256], wqksrc[:, 4:8, 0:256])
            nc.sync.dma_start(g0[:, 4:8, :], gsrc[:, 4:8, 0:256])
            nc.sync.dma_start(wqk[:, :, 256:512], wqksrc[:, :, 256:512])
            nc.sync.dma_start(wv[:],
                              d["WvT"].rearrange("(ct p) o -> p ct o", p=P))
            for t, key in [(bqk, "bqk"), (bup, "bup"), (bdown, "bdown"),
                           (diagm, "diagm")]:
                nc.sync.dma_start(t[:], d[key])
            nc.sync.dma_start(wp_sb[:],
                              d["WpT"].rearrange("(j p) o -> p j o", p=P))
            for hh in range(2 * NSPAN):      # half-spans of 256 tokens
                c0 = hh * 256
                if hh == 0:
                    g = g0
                else:
                    g = gpool.tile([P, NCT, 256], bf16, name="g", tag="g")
                    nc.sync.dma_start(g[:], gsrc[:, :, c0:c0 + 256])
                for ot in range(4):          # q0 q1 k0 k1
                    ps = qkps.tile([P, 256], f32, name="ps", tag="qk")
                    for ct in range(NCT):
                        nc.tensor.matmul(ps[:], wqk[:, ct, ot * P:(ot + 1) * P],
                                         g[:, ct, :],
                                         start=(ct == 0), stop=(ct == NCT - 1))
                    dstT = qT if ot < 2 else kT
                    nc.scalar.add(dstT[:, ot % 2, c0:c0 + 256],
                                  ps[:], bqk[:, ot:ot + 1])
                for tl in range(2):          # v, token tile kt = 2*hh+tl
                    kt = 2 * hh + tl
                    pv = qkps.tile([P, 256], f32, name="pv", tag="qk")
                    for ct in range(NCT):
                        nc.tensor.matmul(pv[:], g[:, ct, tl * P:(tl + 1) * P],
                                         wv[:, ct, :],
                                         start=(ct == 0), stop=(ct == NCT - 1))
                    nc.scalar.copy(
                        v_aug[:, :, kt, 0:64],
                        pv[:].rearrange("p (h dd) -> p h dd", dd=64))
            # stage the big loads (SP queue, after the startup-critical ones)
            wdsrc = d["WdownT"].rearrange("(cf p) o -> p cf o", p=P)
            for cc in range(8):
                nc.sync.dma_start(wd[:, cc * 4:(cc + 1) * 4, :],
                                  wdsrc[:, cc * 4:(cc + 1) * 4, :])
            nc.sync.dma_start(x1T[:],
                              d["xbT"].rearrange("(ct p) t -> p ct t", p=P))

        # ----- strip-MLP emission helpers -----
        wusrc = d["WupT"].rearrange("(ct p) f -> p ct f", p=P)
        outdst = d["OUT"].rearrange("(ot p) t -> p ot t", p=P)

        def emit_wuw(w):
            t = mep.tile([P, NCT, 1024], bf16, name="wuw", tag="wuw", bufs=2)
            nc.sync.dma_start(
                t[:], wusrc[:, :, (w % 4) * 1024:((w % 4) + 1) * 1024])
            return t

        def emit_rs_transpose(r):
            rsT = mep.tile([P, NCT, STRIP], bf16, name="rsT", tag="rsT")
            nc.sync.dma_start_transpose(rsT[:], rsout[r][:])
            return rsT

        def emit_strip_pre(r, rsT, upps, gate_ex=None):
            cs = r * STRIP
            if gate_ex is not None:
                # scheduler-proofing: root the chain on a late attention exp
                # so its long RS-wait cannot head-block the DVE queue ahead
                # of ready attention work (the scheduler's internal sim
                # underestimates collective latency)
                gate2 = mep.tile([P, 1], f32, name="gate2", tag="gate2")
                nc.vector.tensor_scalar(gate2[:], gate_ex[:, 511:512],
                                        0.0, 1.0, mybir.AluOpType.mult,
                                        mybir.AluOpType.add)
                rsTg = mep.tile([P, NCT, STRIP], bf16, name="rsTg",
                                tag="x1b")
                nc.vector.tensor_scalar(rsTg[:], rsT[:], gate2[:, 0:1], None,
                                        mybir.AluOpType.mult)
                rsT = rsTg
            x1b = mep.tile([P, NCT, STRIP], bf16, name="x1b", tag="x1b")
            if r == 1 and DBG == "":
                # post-attention strip: bf16 x1 first so the LN2 stat
                # matmuls start one DVE op earlier; fp32 residual update
                # follows off the critical path
                nc.vector.tensor_add(x1b[:], x1T[:, :, cs:cs + STRIP],
                                     rsT[:])
                nc.vector.tensor_add(x1T[:, :, cs:cs + STRIP],
                                     x1T[:, :, cs:cs + STRIP], rsT[:])
            else:
                nc.vector.tensor_add(x1T[:, :, cs:cs + STRIP],
                                     x1T[:, :, cs:cs + STRIP], rsT[:])
                if DBG == "x1":
                    nc.sync.dma_start(outdst[:, :, cs:cs + STRIP],
                                      x1T[:, :, cs:cs + STRIP])
                    return None
                nc.vector.tensor_copy(x1b[:], x1T[:, :, cs:cs + STRIP])
            sqb = mep.tile([P, NCT, STRIP], bf16, name="sqb", tag="g28")
            for ct in range(NCT):
                nc.vector.tensor_mul(sqb[:, ct, :], x1b[:, ct, :],
                                     x1b[:, ct, :])
            psmu = upps.tile([P, STRIP], f32, name="psmu", tag="pu")
            for ct in range(NCT):
                nc.tensor.matmul(psmu[:], onesb[:], x1b[:, ct, :],
                                 start=(ct == 0), stop=(ct == NCT - 1))
            pssq = upps.tile([P, STRIP], f32, name="pssq", tag="pu")
            for ct in range(NCT):
                nc.tensor.matmul(pssq[:], onesb[:], sqb[:, ct, :],
                                 start=(ct == 0), stop=(ct == NCT - 1))
            mu = mep.tile([P, STRIP], f32, name="mu", tag="mu")
            e2 = mep.tile([P, STRIP], f32, name="e2", tag="e2")
            std = mep.tile([P, STRIP], f32, name="std", tag="stdt")
            nc.scalar.mul(mu[:], psmu[:], 1.0 / C)
            nc.scalar.mul(e2[:], pssq[:], 1.0 / C)
            musq = mep.tile([P, STRIP], f32, name="musq", tag="tmpc", bufs=2)
            nc.vector.tensor_mul(musq[:], mu[:], mu[:])
            nc.vector.tensor_sub(e2[:], e2[:], musq[:])
            nc.scalar.activation(std[:], e2[:], AF.Sqrt, bias=epsT[:])
            nc.vector.reciprocal(std[:], std[:])
            g2 = mep.tile([P, NCT, STRIP], bf16, name="g2", tag="g28")
            for ct in range(NCT):
                tmpc = mep.tile([P, STRIP], f32, name="tmpc", tag="tmpc",
                                bufs=2)
                nc.vector.tensor_sub(tmpc[:], x1T[:, ct, cs:cs + STRIP],
                                     mu[:])
                nc.vector.tensor_mul(g2[:, ct, :], tmpc[:], std[:])
            if DBG == "g2":
                g2f = mep.tile([P, NCT, STRIP], f32, name="g2f", tag="g2f")
                nc.vector.tensor_copy(g2f[:], g2[:])
                nc.sync.dma_start(outdst[:, :, cs:cs + STRIP], g2f[:])
                return None
            hT = mep.tile([P, NFT, STRIP], bf16, name="hT", tag="hT")
            return dict(cs=cs, g2=g2, hT=hT)

        def emit_up_group(st, f, wt, defer_gelu=False):
            pu = upps_ref[0].tile([P, STRIP], f32, name="pu", tag="pu")
            fl = f % 8
            for ct in range(NCT):
                nc.tensor.matmul(pu[:], wt[:, ct, fl * P:(fl + 1) * P],
                                 st["g2"][:, ct, :],
                                 start=(ct == 0), stop=(ct == NCT - 1))
            if defer_gelu:
                # keep Gelu out of the attention window: its act table
                # does not share a set with Exp, so inline gelus thrash
                # 1.28us table loads per switch.  Stage raw pre-act.
                nc.vector.tensor_copy(st["hT"][:, f, :], pu[:])
            else:
                nc.scalar.activation(st["hT"][:, f, :], pu[:], AF.Gelu,
                                     bias=bup[:, f:f + 1])

        def emit_down_out(st, dnps):
            for ot in range(8):
                pd = dnps.tile([P, STRIP], f32, name="pd", tag="dn")
                for cf in range(NFT):
                    nc.tensor.matmul(pd[:], wd[:, cf, ot * P:(ot + 1) * P],
                                     st["hT"][:, cf, :],
                                     start=(cf == 0), stop=(cf == NFT - 1))
                td = mep.tile([P, STRIP], f32, name="td", tag="td", bufs=2)
                nc.scalar.add(td[:], pd[:], bdown[:, ot:ot + 1])
                ox = mep.tile([P, STRIP], f32, name="ox", tag="ox", bufs=2)
                nc.vector.tensor_add(ox[:], td[:],
                                     x1T[:, ot, st["cs"]:st["cs"] + STRIP])
                nc.gpsimd.dma_start(outdst[:, ot, st["cs"]:st["cs"] + STRIP],
                                    ox[:])

        # ---------------- attention + proj + RS (+ strip0 MLP fill) --------
        wuw = {}
        if not DBG:
            wuw[0] = emit_wuw(0)
            wuw[1] = emit_wuw(1)
        st0 = None
        rsT0 = None
        rsT1 = None
        last_ex = None
        upf = [0]        # next strip0 up f-group to emit

        with tc.tile_pool(name="avps", bufs=4, space="PSUM") as avps, \
             tc.tile_pool(name="expp", bufs=3) as expp, \
             tc.tile_pool(name="nrmp", bufs=2) as nrmp, \
             tc.tile_pool(name="prst", bufs=2) as prst:

            def span_tail(Q, avs, mkpsum):
                # normalize: attnT[:, pr, span] = av / den, then proj + RS
                q0 = Q * SPAN
                for pr in range(2):
                    denb = nrmp.tile([P, 512], bf16, name="denb", tag="denb")
                    nc.vector.memset(denb[0:65, :], 0.0)
                    with nc.allow_low_precision(reason="bf16 softmax denom"):
                        nc.vector.reciprocal(denb[0:1, :],
                                             avs[2 * pr][64:65, :])
                        nc.vector.reciprocal(denb[64:65, :],
                                             avs[2 * pr + 1][64:65, :])
                    bc = mkpsum()
                    nc.tensor.matmul(bc[:], selp[0:65, :], denb[0:65, :],
                                     start=True, stop=True)
                    bcs = nrmp.tile([P, 512], f32, name="bcs", tag="bcs")
                    nc.scalar.copy(bcs[:], bc[:])
                    nc.vector.tensor_mul(attnT[0:64, pr, q0:q0 + SPAN],
                                         avs[2 * pr][0:64, :], bcs[0:64, :])
                    nc.vector.tensor_mul(attnT[64:128, pr, q0:q0 + SPAN],
                                         avs[2 * pr + 1][0:64, :],
                                         bcs[64:128, :])
                for tl in range(4):
                    tt = 4 * Q + tl
                    stage = prst.tile([P, 1024], bf16, name="stage", tag="st")
                    for half in range(2):
                        pp = avps.tile([P, 512], f32, name="pp", tag="av")
                        for j in range(2):
                            nc.tensor.matmul(
                                pp[:],
                                attnT[:, j, tt * P:(tt + 1) * P],
                                wp_sb[:, j, half * 512:(half + 1) * 512],
                                start=(j == 0), stop=(j == 1))
                        nc.vector.tensor_copy(
                            stage[:, half * 512:(half + 1) * 512], pp[:])
                    row = (Q % 2) * 512 + tl * P
                    nc.gpsimd.dma_start(partials[Q // 2][row:row + P, :],
                                        stage[:])
                if Q % 2 == 1:
                    r = Q // 2
                    nc.gpsimd.collective_compute(
                        "ReduceScatter", mybir.AluOpType.add,
                        ins=[partials[r].opt()], outs=[rsout[r].opt()],
                        replica_groups=[[0, 1, 2, 3], [4, 5, 6, 7]])
                    return emit_rs_transpose(r)
                return None

            # ---- spans 0-2: paired scores, one exp per head pair ----
            scp_cm = tc.tile_pool(name="scp", bufs=2, space="PSUM")
            scp = scp_cm.__enter__()

            def mkpsum_pair():
                t = scp.tile([P, 2, 512], f32, name="sc2", tag="sc")
                return t[:, 0, :]

            for Q in range(3):
                q0 = Q * SPAN
                avs = []
                for h in range(HC):
                    av = avps.tile([P, 512], f32, name=f"av{h}", tag="av")
                    avs.append(av)
                nkt = 4 * Q + 4
                for kt in range(nkt):
                    p_ = kt - 4 * Q
                    c0 = 128 * p_ if p_ > 0 else 0
                    sps = []
                    for pr in range(2):
                        sc2 = scp.tile([P, 2, 512], f32, name="sc2", tag="sc")
                        for i in range(2):
                            h = 2 * pr + i
                            hb = (h % 2) * 64
                            j = h // 2
                            nc.tensor.matmul(
                                sc2[:, i, c0:512],
                                kT[hb:hb + 64, j, kt * P:(kt + 1) * P],
                                qT[hb:hb + 64, j, q0 + c0:q0 + 512],
                                start=True, stop=True)
                        sps.append(sc2)
                    for pr in range(2):
                        ex2 = expp.tile([P, 2, 512], bf16, name="ex2",
                                        tag="ex")
                        nc.scalar.activation(ex2[:, :, c0:512],
                                             sps[pr][:, :, c0:512], AF.Exp)
                        for i in range(2):
                            h = 2 * pr + i
                            if p_ >= 0:
                                nc.vector.tensor_mul(ex2[:, i, c0:c0 + 128],
                                                     ex2[:, i, c0:c0 + 128],
                                                     diagm[:])
                            nc.tensor.matmul(
                                avs[h][0:65, c0:512],
                                v_aug[:, h, kt, 0:65],
                                ex2[:, i, c0:512],
                                start=(kt == 0), stop=(kt == nkt - 1),
                                skip_group_check=True)
                rsT = span_tail(Q, avs, mkpsum_pair)
                if rsT is not None:
                    rsT0 = rsT
                    if not DBG:
                        wuw[2] = emit_wuw(2)
                        wuw[3] = emit_wuw(3)

            scp_cm.__exit__(None, None, None)

            # ---- span 3: single scores + strip0 MLP fill ----
            with tc.tile_pool(name="sc3p", bufs=2, space="PSUM") as scps, \
                 tc.tile_pool(name="upps", bufs=2, space="PSUM") as upps:
                upps_ref = [upps]

                def fill_slot(kt):
                    nonlocal st0
                    if DBG:
                        return
                    if kt == 4:
                        st0 = emit_strip_pre(0, rsT0, upps, gate_ex=last_ex)
                        return
                    if st0 is None or upf[0] >= NFT:
                        return
                    for _ in range(2):
                        if upf[0] >= NFT:
                            break
                        emit_up_group(st0, upf[0], wuw[upf[0] // 8],
                                      defer_gelu=True)
                        upf[0] += 1

                Q = 3
                q0 = Q * SPAN
                avs = []
                for h in range(HC):
                    av = avps.tile([P, 512], f32, name=f"av{h}", tag="av")
                    avs.append(av)
                nkt = 16
                for kt in range(nkt):
                    p_ = kt - 4 * Q
                    c0 = 128 * p_ if p_ > 0 else 0
                    scs = []
                    for h in range(HC):
                        hb = (h % 2) * 64
                        j = h // 2
                        sc = scps.tile([P, 512], f32, name="sc", tag="sc")
                        nc.tensor.matmul(
                            sc[:, c0:512],
                            kT[hb:hb + 64, j, kt * P:(kt + 1) * P],
                            qT[hb:hb + 64, j, q0 + c0:q0 + 512],
                            start=True, stop=True)
                        scs.append(sc)
                    for h in range(HC):
                        ex = expp.tile([P, 512], bf16, name="ex", tag="ex")
                        last_ex = ex
                        nc.scalar.activation(ex[:, c0:512], scs[h][:, c0:512],
                                             AF.Exp)
                        if p_ >= 0:
                            nc.vector.tensor_mul(ex[:, c0:c0 + 128],
                                                 ex[:, c0:c0 + 128], diagm[:])
                        nc.tensor.matmul(
                            avs[h][0:65, c0:512],
                            v_aug[:, h, kt, 0:65],
                            ex[:, c0:512],
                            start=(kt == 0), stop=(kt == nkt - 1),
                            skip_group_check=True)
                    fill_slot(kt)

                def mkpsum_single():
                    return scps.tile([P, 512], f32, name="pp", tag="sc")

                rsT1 = span_tail(3, avs, mkpsum_single)
                if not DBG:
                    for w in range(4, 8):
                        wuw[w] = emit_wuw(w)
                # rest of strip0's ups (overlap RS2 on the collective cores)
                if not DBG and st0 is not None:
                    while upf[0] < NFT:
                        emit_up_group(st0, upf[0], wuw[upf[0] // 8],
                                      defer_gelu=True)
                        upf[0] += 1
                # gate = 1.0, data-dependent on the last exp: ops scaled by
                # it cannot be scheduled into the attention exp stream
                nc.vector.tensor_scalar(gate[:], last_ex[:, 511:512], 0.0,
                                        1.0, mybir.AluOpType.mult,
                                        mybir.AluOpType.add)
                if DBG:
                    rsT0x = rsT0 if rsT0 is not None else emit_rs_transpose(0)
                    emit_strip_pre(0, rsT0x, upps)
                    emit_strip_pre(1, rsT1, upps)

        bpool_cm.__exit__(None, None, None)   # free qT / kT / v_aug / wp_sb

        # ---------------- strip0 down + full strip1 ----------------
        if not DBG:
            with tc.tile_pool(name="dnps", bufs=2, space="PSUM") as dnps, \
                 tc.tile_pool(name="up2", bufs=2, space="PSUM") as up2:
                upps_ref[0] = up2
                # bulk gelu for strip0's staged pre-activations (one table
                # switch, after all attention exps are done; gate enforces it)
                for f in range(NFT):
                    nc.scalar.activation(st0["hT"][:, f, :], st0["hT"][:, f, :],
                                         AF.Gelu, bias=bup[:, f:f + 1],
                                         scale=gate[:, 0:1])
                emit_down_out(st0, dnps)
                st1 = emit_strip_pre(1, rsT1, up2)
                for f in range(NFT):
                    emit_up_group(st1, f, wuw[4 + f // 8])
                emit_down_out(st1, dnps)


def _prep_inputs(x, ln1_w, ln1_b, c_attn_w, c_attn_b, c_proj_w, c_proj_b,
                 ln2_w, ln2_b, up_w, up_b, down_w, down_b):
    """Host-side preprocessing -> list of 8 per-core input dicts."""
    x = np.asarray(x, np.float32)
    f64 = np.float64
    mu = x.mean(-1, keepdims=True, dtype=f64)
    var = np.asarray(x, f64).var(-1, keepdims=True)
    g = ((x - mu) / np.sqrt(var + EPS)).astype(np.float32)     # [B, T, C]

    ln1_w = np.asarray(ln1_w, np.float32); ln1_b = np.asarray(ln1_b, np.float32)
    ln2_w = np.asarray(ln2_w, np.float32); ln2_b = np.asarray(ln2_b, np.float32)
    c_attn_w = np.asarray(c_attn_w, np.float32)
    c_attn_b = np.asarray(c_attn_b, np.float32)
    c_proj_w = np.asarray(c_proj_w, np.float32)
    c_proj_b = np.asarray(c_proj_b, np.float32)
    up_w = np.asarray(up_w, np.float32); up_b = np.asarray(up_b, np.float32)
    down_w = np.asarray(down_w, np.float32)
    down_b = np.asarray(down_b, np.float32)

    Wa = c_attn_w * ln1_w[None, :]                  # fold LN1 scale
    ba = c_attn_b + c_attn_w @ ln1_b                # fold LN1 shift
    Wq, Wk, Wv = Wa[:C], Wa[C:2 * C], Wa[2 * C:]
    bq, bk, bv = ba[:C], ba[C:2 * C], ba[2 * C:]
    s = 1.0 / np.sqrt(D)
    Wq = Wq * s; bq = bq * s                        # fold attention scale

    Wup = up_w * ln2_w[None, :]
    bupv = up_b + up_w @ ln2_b

    def b2t(v, n):   # per-partition bias layout [128, n]
        return np.ascontiguousarray(v.reshape(n, P).T.astype(np.float32))

    diagm = np.triu(np.ones((P, P), np.float32))    # kv row <= q col

    shared = {
        "WupT": np.ascontiguousarray(Wup.T).astype(BF),
        "WdownT": np.ascontiguousarray(down_w.T).astype(BF),
        "bup": b2t(bupv, 32), "bdown": b2t(down_b, 8),
        "diagm": diagm.astype(BF),
    }

    # residual with proj bias and (v-bias pushed through proj) folded in
    xb = x + (c_proj_b + c_proj_w @ bv)[None, None, :]

    in_maps, tok_slices = [], []
    for core in range(8):
        b, i = core // 4, core % 4
        ch = slice(i * 256, (i + 1) * 256)          # this core's attn channels
        wqk = np.concatenate([Wq[ch], Wk[ch]], axis=0)      # [512, 1024]
        m = dict(shared)
        m["WqkT"] = np.ascontiguousarray(wqk.T).astype(BF)
        m["WvT"] = np.ascontiguousarray(Wv[ch].T).astype(BF)
        m["WpT"] = np.ascontiguousarray(c_proj_w[:, ch].T).astype(BF)
        m["bqk"] = b2t(np.concatenate([bq[ch], bk[ch]]), 4)
        m["gT"] = np.ascontiguousarray(g[b].T).astype(BF)
        strips = [slice(1024 * r + STRIP * i, 1024 * r + STRIP * (i + 1))
                  for r in range(2)]
        xbT = np.concatenate([xb[b, st].T for st in strips], axis=1)
        m["xbT"] = np.ascontiguousarray(xbT).astype(np.float32)
        in_maps.append(m)
        tok_slices.append((b, strips))
    return in_maps, tok_slices


def kernel(**inputs):
    global _CACHED_NC
    if _CACHED_NC is None:
        _CACHED_NC = _build_nc()
    nc = _CACHED_NC
    in_maps, tok_slices = _prep_inputs(**inputs)
    try:
        res = run_bass_kernel_spmd(nc, in_maps, list(range(8)))
    except Exception:
        # one retry: transient NRT device faults are recoverable on re-run
        res = run_bass_kernel_spmd(nc, in_maps, list(range(8)))
    out = np.empty((B, T, C), np.float32)
    for core in range(8):
        o = res.results[core]["OUT"]                # [C, 512]
        b, strips = tok_slices[core]
        for r, st in enumerate(strips):
            out[b, st, :] = o[:, r * STRIP:(r + 1) * STRIP].T
    return out


# revision 6
# speedup vs baseline: 1.0378x; 1.0031x over previous
"""Trainium2 Bass kernel for a GPT-style transformer block (B=2, T=2048,
C=1024, 16 heads, MLP 4x), sharded across 8 NeuronCores.

Sharding: attention is HEAD-sharded. Core c = (b, i) with b = c//4,
i = c%4 owns heads [4i, 4i+4) of batch b for the whole 2048-token range:
it computes q/k/v for just those 256 channels (same FLOPs as a row shard
of all channels), runs causal attention with no cross-core kv exchange,
then produces token-major c_proj partials. Two ReduceScatters (one per
1024-token chunk) sum the partials over the 4-core batch group and hand
each core a contiguous 256-token strip per chunk; LN2+MLP then run
data-parallel on the core's two strips.

Host precomputes LN1 (inputs-only), folds LN scale/shift into the matmul
weights, folds 1/sqrt(D) into Wq, and folds the v-bias + proj-bias into
the residual (softmax weights sum to 1, so attn_out = av/den + bv and
proj(attn_out) = proj(av/den) + Wp@bv). Causality lives in the loop
bounds; only diagonal 128x128 tiles get a multiplicative triu mask.
"""
import numpy as np
import ml_dtypes

import concourse.bass as bass
import concourse.mybir as mybir
import concourse.tile as tile
import concourse.bacc as bacc
from concourse.bass_utils import run_bass_kernel_spmd

BF = ml_dtypes.bfloat16
P = 128
B, T, C, D, F = 2, 2048, 1024, 64, 4096
NCT = C // P            # 8   c-tiles
NFT = F // P            # 32  f-tiles
NTT = T // P            # 16  token tiles
HC = 4                  # heads per core
SPAN = 512              # q/token span
NSPAN = T // SPAN       # 4
STRIP = 256             # tokens owned per core per RS chunk
EPS = 1e-5
f32 = mybir.dt.float32
bf16 = mybir.dt.bfloat16
AF = mybir.ActivationFunctionType

_CACHED_NC = None
import os as _os
DBG = _os.environ.get("KDBG", "")


def _build_nc():
    nc = bacc.Bacc("TRN2", target_bir_lowering=False, debug=False)
    d = {}
    for name, shape, dt in [
        ("gT", [C, T], bf16),          # ln1(x) for the whole batch, ch-major
        ("WqkT", [C, 512], bf16),      # q(256) | k(256) out channels
        ("WvT", [C, 256], bf16),
        ("WpT", [256, C], bf16),       # [attn-ch, out-ch] for moving use
        ("WupT", [C, F], bf16),
        ("WdownT", [F, C], bf16),
        ("bqk", [P, 4], f32),
        ("bup", [P, 32], f32),
        ("bdown", [P, 8], f32),
        ("xbT", [C, 512], f32),        # residual for own tokens (biases folded)
        ("diagm", [P, P], bf16),       # triu causal mask for diagonal tiles
    ]:
        d[name] = nc.dram_tensor(name, shape, dt, kind="ExternalInput").ap()
    d["OUT"] = nc.dram_tensor("OUT", [C, 512], f32, kind="ExternalOutput").ap()

    with tile.TileContext(nc) as tc:
        _emit(nc, tc, d)
    nc.compile()
    return nc


def _emit(nc, tc, d):
    from contextlib import ExitStack

    with ExitStack() as ctx:
        # ---------------- long-lived tiles ----------------
        cpool = ctx.enter_context(tc.tile_pool(name="cpool", bufs=1))
        attnT = cpool.tile([P, 2, T], bf16, name="attnT")
        bqk = cpool.tile([P, 4], f32, name="bqk")
        bup = cpool.tile([P, 32], f32, name="bup")
        bdown = cpool.tile([P, 8], f32, name="bdown")
        diagm = cpool.tile([P, P], bf16, name="diagm")
        epsT = cpool.tile([P, 1], f32, name="epsT")
        onesb = cpool.tile([P, P], bf16, name="onesb")
        selp = cpool.tile([P, P], bf16, name="selp")
        gate = cpool.tile([P, 1], f32, name="gate")
        nc.vector.memset(epsT[:], EPS)
        nc.vector.memset(onesb[:], 1.0)
        nc.vector.memset(selp[:], 0.0)
        nc.vector.memset(selp[0:1, 0:64], 1.0)
        nc.vector.memset(selp[64:65, 64:128], 1.0)

        # DRAM scratch for the proj partials + RS outputs
        dramp = ctx.enter_context(tc.tile_pool(name="dramp", bufs=1,
                                               space="DRAM"))
        partials = [dramp.tile([1024, C], bf16, name=f"part{r}")
                    for r in range(2)]
        rsout = [dramp.tile([STRIP, C], bf16, name=f"rsout{r}")
                 for r in range(2)]

        # MLP weights / residual / strip tiles: allocated BEFORE the
        # attention pools so strip-0's MLP can run while attention finishes
        # (tiles in later pools would inherit waits on attention SBUF reuse).
        wmlp = ctx.enter_context(tc.tile_pool(name="wmlp", bufs=1))
        wd = wmlp.tile([P, NFT, C], bf16, name="wd")
        x1T = wmlp.tile([P, NCT, 512], f32, name="x1T")    # xb, then x1
        mep = ctx.enter_context(tc.tile_pool(name="mep", bufs=1))

        # attention operand tiles (freed after proj)
        bpool_cm = tc.tile_pool(name="bpool", bufs=1)
        bpool = bpool_cm.__enter__()
        qT = bpool.tile([P, 2, T], bf16, name="qT")
        kT = bpool.tile([P, 2, T], bf16, name="kT")
        v_aug = bpool.tile([P, HC, NTT, 65], bf16, name="v_aug")
        wp_sb = bpool.tile([P, 2, C], bf16, name="wp_sb")
        nc.vector.memset(v_aug[:, :, :, 64:65], 1.0)

        # ---------------- qkv projections ----------------
        with tc.tile_pool(name="gpool", bufs=2) as gpool, \
             tc.tile_pool(name="wqkp", bufs=1) as wqkp, \
             tc.tile_pool(name="qkps", bufs=3, space="PSUM") as qkps:
            wqk = wqkp.tile([P, NCT, 512], bf16, name="wqk")
            wv = wqkp.tile([P, NCT, 256], bf16, name="wv")
            wqksrc = d["WqkT"].rearrange("(ct p) o -> p ct o", p=P)
            gsrc = d["gT"].rearrange("(ct p) t -> p ct t", p=P)
            # startup-critical order: q weights, first g chunk, then the rest
            nc.sync.dma_start(wqk[:, 0:4, 0:256], wqksrc[:, 0:4, 0:256])
            g0 = gpool.tile([P, NCT, 256], bf16, name="g", tag="g")
            nc.sync.dma_start(g0[:, 0:4, :], gsrc[:, 0:4, 0:256])
            nc.sync.dma_start(wqk[:, 4:8, 0:256], wqksrc[:, 4:8, 0:256])
            nc.sync.dma_start(g0[:, 4:8, :], gsrc[:, 4:8, 0:256])
            nc.sync.dma_start(wqk[:, :, 256:512], wqksrc[:, :, 256:512])
            nc.sync.dma_start(wv[:],
                              d["WvT"].rearrange("(ct p) o -> p ct o", p=P))
            for t, key in [(bqk, "bqk"), (bup, "bup"), (bdown, "bdown"),
                           (diagm, "diagm")]:
                nc.sync.dma_start(t[:], d[key])
            nc.sync.dma_start(wp_sb[:],
                              d["WpT"].rearrange("(j p) o -> p j o", p=P))
            for hh in range(2 * NSPAN):      # half-spans of 256 tokens
                c0 = hh * 256
                if hh == 0:
                    g = g0
                else:
                    g = gpool.tile([P, NCT, 256], bf16, name="g", tag="g")
                    nc.sync.dma_start(g[:], gsrc[:, :, c0:c0 + 256])
                for ot in range(4):          # q0 q1 k0 k1
                    ps = qkps.tile([P, 256], f32, name="ps", tag="qk")
                    for ct in range(NCT):
                        nc.tensor.matmul(ps[:], wqk[:, ct, ot * P:(ot + 1) * P],
                                         g[:, ct, :],
                                         start=(ct == 0), stop=(ct == NCT - 1))
                    dstT = qT if ot < 2 else kT
                    nc.scalar.add(dstT[:, ot % 2, c0:c0 + 256],
                                  ps[:], bqk[:, ot:ot + 1])
                for tl in range(2):          # v, token tile kt = 2*hh+tl
                    kt = 2 * hh + tl
                    pv = qkps.tile([P, 256], f32, name="pv", tag="qk")
                    for ct in range(NCT):
                        nc.tensor.matmul(pv[:], g[:, ct, tl * P:(tl + 1) * P],
                                         wv[:, ct, :],
                                         start=(ct == 0), stop=(ct == NCT - 1))
                    nc.scalar.copy(
                        v_aug[:, :, kt, 0:64],
                        pv[:].rearrange("p (h dd) -> p h dd", dd=64))
            # stage the big loads (SP queue, after the startup-critical ones)
            wdsrc = d["WdownT"].rearrange("(cf p) o -> p cf o", p=P)
            for cc in range(8):
                nc.sync.dma_start(wd[:, cc * 4:(cc + 1) * 4, :],
                                  wdsrc[:, cc * 4:(cc + 1) * 4, :])
            nc.sync.dma_start(x1T[:],
                              d["xbT"].rearrange("(ct p) t -> p ct t", p=P))

        # ----- strip-MLP emission helpers -----
        wusrc = d["WupT"].rearrange("(ct p) f -> p ct f", p=P)
        outdst = d["OUT"].rearrange("(ot p) t -> p ot t", p=P)

        def emit_wuw(w):
            t = mep.tile([P, NCT, 1024], bf16, name="wuw", tag="wuw", bufs=2)
            nc.sync.dma_start(
                t[:], wusrc[:, :, (w % 4) * 1024:((w % 4) + 1) * 1024])
            return t

        def emit_rs_transpose(r):
            rsT = mep.tile([P, NCT, STRIP], bf16, name="rsT", tag="rsT")
            nc.sync.dma_start_transpose(rsT[:], rsout[r][:])
            return rsT

        def emit_strip_pre(r, rsT, upps, gate_ex=None):
            cs = r * STRIP
            if gate_ex is not None:
                # scheduler-proofing: root the chain on a late attention exp
                # so its long RS-wait cannot head-block the DVE queue ahead
                # of ready attention work (the scheduler's internal sim
                # underestimates collective latency)
                gate2 = mep.tile([P, 1], f32, name="gate2", tag="gate2")
                nc.vector.tensor_scalar(gate2[:], gate_ex[:, 511:512],
                                        0.0, 1.0, mybir.AluOpType.mult,
                                        mybir.AluOpType.add)
                rsTg = mep.tile([P, NCT, STRIP], bf16, name="rsTg",
                                tag="x1b")
                nc.vector.tensor_scalar(rsTg[:], rsT[:], gate2[:, 0:1], None,
                                        mybir.AluOpType.mult)
                rsT = rsTg
            x1b = mep.tile([P, NCT, STRIP], bf16, name="x1b", tag="x1b")
            if r == 1 and DBG == "":
                # post-attention strip: bf16 x1 first so the LN2 stat
                # matmuls start one DVE op earlier; fp32 residual update
                # follows off the critical path
                nc.vector.tensor_add(x1b[:], x1T[:, :, cs:cs + STRIP],
                                     rsT[:])
                nc.vector.tensor_add(x1T[:, :, cs:cs + STRIP],
                                     x1T[:, :, cs:cs + STRIP], rsT[:])
            else:
                nc.vector.tensor_add(x1T[:, :, cs:cs + STRIP],
                                     x1T[:, :, cs:cs + STRIP], rsT[:])
                if DBG == "x1":
                    nc.sync.dma_start(outdst[:, :, cs:cs + STRIP],
                                      x1T[:, :, cs:cs + STRIP])
                    return None
                nc.vector.tensor_copy(x1b[:], x1T[:, :, cs:cs + STRIP])
            sqb = mep.tile([P, NCT, STRIP], bf16, name="sqb", tag="g28")
            for ct in range(NCT):
                nc.vector.tensor_mul(sqb[:, ct, :], x1b[:, ct, :],
                                     x1b[:, ct, :])
            psmu = upps.tile([P, STRIP], f32, name="psmu", tag="pu")
            for ct in range(NCT):
                nc.tensor.matmul(psmu[:], onesb[:], x1b[:, ct, :],
                                 start=(ct == 0), stop=(ct == NCT - 1))
            pssq = upps.tile([P, STRIP], f32, name="pssq", tag="pu")
            for ct in range(NCT):
                nc.tensor.matmul(pssq[:], onesb[:], sqb[:, ct, :],
                                 start=(ct == 0), stop=(ct == NCT - 1))
            mu = mep.tile([P, STRIP], f32, name="mu", tag="mu")
            e2 = mep.tile([P, STRIP], f32, name="e2", tag="e2")
            std = mep.tile([P, STRIP], f32, name="std", tag="stdt")
            nc.scalar.mul(mu[:], psmu[:], 1.0 / C)
            nc.scalar.mul(e2[:], pssq[:], 1.0 / C)
            musq = mep.tile([P, STRIP], f32, name="musq", tag="tmpc", bufs=2)
            nc.vector.tensor_mul(musq[:], mu[:], mu[:])
            nc.vector.tensor_sub(e2[:], e2[:], musq[:])
            nc.scalar.activation(std[:], e2[:], AF.Sqrt, bias=epsT[:])
            nc.vector.reciprocal(std[:], std[:])
            g2 = mep.tile([P, NCT, STRIP], bf16, name="g2", tag="g28")
            for ct in range(NCT):
                tmpc = mep.tile([P, STRIP], f32, name="tmpc", tag="tmpc",
                                bufs=2)
                nc.vector.tensor_sub(tmpc[:], x1T[:, ct, cs:cs + STRIP],
                                     mu[:])
                nc.vector.tensor_mul(g2[:, ct, :], tmpc[:], std[:])
            if DBG == "g2":
                g2f = mep.tile([P, NCT, STRIP], f32, name="g2f", tag="g2f")
                nc.vector.tensor_copy(g2f[:], g2[:])
                nc.sync.dma_start(outdst[:, :, cs:cs + STRIP], g2f[:])
                return None
            hT = mep.tile([P, NFT, STRIP], bf16, name="hT", tag="hT")
            return dict(cs=cs, g2=g2, hT=hT)

        def emit_up_group(st, f, wt, defer_gelu=False):
            pu = upps_ref[0].tile([P, STRIP], f32, name="pu", tag="pu")
            fl = f % 8
            for ct in range(NCT):
                nc.tensor.matmul(pu[:], wt[:, ct, fl * P:(fl + 1) * P],
                                 st["g2"][:, ct, :],
                                 start=(ct == 0), stop=(ct == NCT - 1))
            if defer_gelu:
                # keep Gelu out of the attention window: its act table
                # does not share a set with Exp, so inline gelus thrash
                # 1.28us table loads per switch.  Stage raw pre-act.
                nc.vector.tensor_copy(st["hT"][:, f, :], pu[:])
            else:
                nc.scalar.activation(st["hT"][:, f, :], pu[:], AF.Gelu,
                                     bias=bup[:, f:f + 1])

        def emit_down_out(st, dnps):
            for ot in range(8):
                pd = dnps.tile([P, STRIP], f32, name="pd", tag="dn")
                for cf in range(NFT):
                    nc.tensor.matmul(pd[:], wd[:, cf, ot * P:(ot + 1) * P],
                                     st["hT"][:, cf, :],
                                     start=(cf == 0), stop=(cf == NFT - 1))
                td = mep.tile([P, STRIP], f32, name="td", tag="td", bufs=2)
                nc.scalar.add(td[:], pd[:], bdown[:, ot:ot + 1])
                ox = mep.tile([P, STRIP], f32, name="ox", tag="ox", bufs=2)
                nc.vector.tensor_add(ox[:], td[:],
                                     x1T[:, ot, st["cs"]:st["cs"] + STRIP])
                nc.gpsimd.dma_start(outdst[:, ot, st["cs"]:st["cs"] + STRIP],
                                    ox[:])

        # ---------------- attention + proj + RS (+ strip0 MLP fill) --------
        wuw = {}
        if not DBG:
            wuw[0] = emit_wuw(0)
            wuw[1] = emit_wuw(1)
        st0 = None
        rsT0 = None
        rsT1 = None
        last_ex = None
        upf = [0]        # next strip0 up f-group to emit

        with tc.tile_pool(name="avps", bufs=4, space="PSUM") as avps, \
             tc.tile_pool(name="expp", bufs=4) as expp, \
             tc.tile_pool(name="nrmp", bufs=2) as nrmp, \
             tc.tile_pool(name="prst", bufs=2) as prst:

            def span_tail(Q, avs, mkpsum):
                # normalize: attnT[:, pr, span] = av / den, then proj + RS
                q0 = Q * SPAN
                for pr in range(2):
                    denb = nrmp.tile([P, 512], bf16, name="denb", tag="denb")
                    nc.vector.memset(denb[0:65, :], 0.0)
                    with nc.allow_low_precision(reason="bf16 softmax denom"):
                        nc.vector.reciprocal(denb[0:1, :],
                                             avs[2 * pr][64:65, :])
                        nc.vector.reciprocal(denb[64:65, :],
                                             avs[2 * pr + 1][64:65, :])
                    bc = mkpsum()
                    nc.tensor.matmul(bc[:], selp[0:65, :], denb[0:65, :],
                                     start=True, stop=True)
                    bcs = nrmp.tile([P, 512], f32, name="bcs", tag="bcs")
                    nc.scalar.copy(bcs[:], bc[:])
                    nc.vector.tensor_mul(attnT[0:64, pr, q0:q0 + SPAN],
                                         avs[2 * pr][0:64, :], bcs[0:64, :])
                    nc.vector.tensor_mul(attnT[64:128, pr, q0:q0 + SPAN],
                                         avs[2 * pr + 1][0:64, :],
                                         bcs[64:128, :])
                for tl in range(4):
                    tt = 4 * Q + tl
                    stage = prst.tile([P, 1024], bf16, name="stage", tag="st")
                    for half in range(2):
                        pp = avps.tile([P, 512], f32, name="pp", tag="av")
                        for j in range(2):
                            nc.tensor.matmul(
                                pp[:],
                                attnT[:, j, tt * P:(tt + 1) * P],
                                wp_sb[:, j, half * 512:(half + 1) * 512],
                                start=(j == 0), stop=(j == 1))
                        nc.vector.tensor_copy(
                            stage[:, half * 512:(half + 1) * 512], pp[:])
                    row = (Q % 2) * 512 + tl * P
                    nc.gpsimd.dma_start(partials[Q // 2][row:row + P, :],
                                        stage[:])
                if Q % 2 == 1:
                    r = Q // 2
                    nc.gpsimd.collective_compute(
                        "ReduceScatter", mybir.AluOpType.add,
                        ins=[partials[r].opt()], outs=[rsout[r].opt()],
                        replica_groups=[[0, 1, 2, 3], [4, 5, 6, 7]])
                    return emit_rs_transpose(r)
                return None

            # ---- spans 0-2: paired scores, one exp per head pair ----
            scp_cm = tc.tile_pool(name="scp", bufs=2, space="PSUM")
            scp = scp_cm.__enter__()

            def mkpsum_pair():
                t = scp.tile([P, 2, 512], f32, name="sc2", tag="sc")
                return t[:, 0, :]

            for Q in range(3):
                q0 = Q * SPAN
                avs = []
                for h in range(HC):
                    av = avps.tile([P, 512], f32, name=f"av{h}", tag="av")
                    avs.append(av)
                nkt = 4 * Q + 4
                for kt in range(nkt):
                    p_ = kt - 4 * Q
                    c0 = 128 * p_ if p_ > 0 else 0
                    sps = []
                    for pr in range(2):
                        sc2 = scp.tile([P, 2, 512], f32, name="sc2", tag="sc")
                        for i in range(2):
                            h = 2 * pr + i
                            hb = (h % 2) * 64
                            j = h // 2
                            nc.tensor.matmul(
                                sc2[:, i, c0:512],
                                kT[hb:hb + 64, j, kt * P:(kt + 1) * P],
                                qT[hb:hb + 64, j, q0 + c0:q0 + 512],
                                start=True, stop=True)
                        sps.append(sc2)
                    for pr in range(2):
                        ex2 = expp.tile([P, 2, 512], bf16, name="ex2",
                                        tag="ex")
                        nc.scalar.activation(ex2[:, :, c0:512],
                                             sps[pr][:, :, c0:512], AF.Exp)
                        for i in range(2):
                            h = 2 * pr + i
                            if p_ >= 0:
                                nc.vector.tensor_mul(ex2[:, i, c0:c0 + 128],
                                                     ex2[:, i, c0:c0 + 128],
                                                     diagm[:])
                            nc.tensor.matmul(
                                avs[h][0:65, c0:512],
                                v_aug[:, h, kt, 0:65],
                                ex2[:, i, c0:512],
                                start=(kt == 0), stop=(kt == nkt - 1),
                                skip_group_check=True)
                rsT = span_tail(Q, avs, mkpsum_pair)
                if rsT is not None:
                    rsT0 = rsT
                    if not DBG:
                        wuw[2] = emit_wuw(2)
                        wuw[3] = emit_wuw(3)

            scp_cm.__exit__(None, None, None)

            # ---- span 3: single scores + strip0 MLP fill ----
            with tc.tile_pool(name="sc3p", bufs=2, space="PSUM") as scps, \
                 tc.tile_pool(name="upps", bufs=2, space="PSUM") as upps:
                upps_ref = [upps]

                def fill_slot(kt):
                    nonlocal st0
                    if DBG:
                        return
                    if kt == 4:
                        st0 = emit_strip_pre(0, rsT0, upps, gate_ex=last_ex)
                        return
                    if st0 is None or upf[0] >= NFT:
                        return
                    for _ in range(2):
                        if upf[0] >= NFT:
                            break
                        emit_up_group(st0, upf[0], wuw[upf[0] // 8],
                                      defer_gelu=True)
                        upf[0] += 1

                Q = 3
                q0 = Q * SPAN
                avs = []
                for h in range(HC):
                    av = avps.tile([P, 512], f32, name=f"av{h}", tag="av")
                    avs.append(av)
                nkt = 16
                for kt in range(nkt):
                    p_ = kt - 4 * Q
                    c0 = 128 * p_ if p_ > 0 else 0
                    scs = []
                    for h in range(HC):
                        hb = (h % 2) * 64
                        j = h // 2
                        sc = scps.tile([P, 512], f32, name="sc", tag="sc")
                        nc.tensor.matmul(
                            sc[:, c0:512],
                            kT[hb:hb + 64, j, kt * P:(kt + 1) * P],
                            qT[hb:hb + 64, j, q0 + c0:q0 + 512],
                            start=True, stop=True)
                        scs.append(sc)
                    for h in range(HC):
                        ex = expp.tile([P, 512], bf16, name="ex", tag="ex")
                        last_ex = ex
                        nc.scalar.activation(ex[:, c0:512], scs[h][:, c0:512],
                                             AF.Exp)
                        if p_ >= 0:
                            nc.vector.tensor_mul(ex[:, c0:c0 + 128],
                                                 ex[:, c0:c0 + 128], diagm[:])
                        nc.tensor.matmul(
                            avs[h][0:65, c0:512],
                            v_aug[:, h, kt, 0:65],
                            ex[:, c0:512],
                            start=(kt == 0), stop=(kt == nkt - 1),
                            skip_group_check=True)
                    fill_slot(kt)

                def mkpsum_single():
                    return scps.tile([P, 512], f32, name="pp", tag="sc")

                rsT1 = span_tail(3, avs, mkpsum_single)
                if not DBG:
                    for w in range(4, 8):
                        wuw[w] = emit_wuw(w)
                # rest of strip0's ups (overlap RS2 on the collective cores)
                if not DBG and st0 is not None:
                    while upf[0] < NFT:
                        emit_up_group(st0, upf[0], wuw[upf[0] // 8],
                                      defer_gelu=True)
                        upf[0] += 1
                # gate = 1.0, data-dependent on the last exp: ops scaled by
                # it cannot be scheduled into the attention exp stream
                nc.vector.tensor_scalar(gate[:], last_ex[:, 511:512], 0.0,
                                        1.0, mybir.AluOpType.mult,
                                        mybir.AluOpType.add)
                if DBG:
                    rsT0x = rsT0 if rsT0 is not None else emit_rs_transpose(0)
                    emit_strip_pre(0, rsT0x, upps)
                    emit_strip_pre(1, rsT1, upps)

        bpool_cm.__exit__(None, None, None)   # free qT / kT / v_aug / wp_sb

        # ---------------- strip0 down + full strip1 ----------------
        if not DBG:
            with tc.tile_pool(name="dnps", bufs=2, space="PSUM") as dnps, \
                 tc.tile_pool(name="up2", bufs=2, space="PSUM") as up2:
                upps_ref[0] = up2
                # bulk gelu for strip0's staged pre-activations (one table
                # switch, after all attention exps are done; gate enforces it)
                for f in range(NFT):
                    nc.scalar.activation(st0["hT"][:, f, :], st0["hT"][:, f, :],
                                         AF.Gelu, bias=bup[:, f:f + 1],
                                         scale=gate[:, 0:1])
                emit_down_out(st0, dnps)
                st1 = emit_strip_pre(1, rsT1, up2)
                for f in range(NFT):
                    emit_up_group(st1, f, wuw[4 + f // 8])
                emit_down_out(st1, dnps)


def _prep_inputs(x, ln1_w, ln1_b, c_attn_w, c_attn_b, c_proj_w, c_proj_b,
                 ln2_w, ln2_b, up_w, up_b, down_w, down_b):
    """Host-side preprocessing -> list of 8 per-core input dicts."""
    x = np.asarray(x, np.float32)
    f64 = np.float64
    mu = x.mean(-1, keepdims=True, dtype=f64)
    var = np.asarray(x, f64).var(-1, keepdims=True)
    g = ((x - mu) / np.sqrt(var + EPS)).astype(np.float32)     # [B, T, C]

    ln1_w = np.asarray(ln1_w, np.float32); ln1_b = np.asarray(ln1_b, np.float32)
    ln2_w = np.asarray(ln2_w, np.float32); ln2_b = np.asarray(ln2_b, np.float32)
    c_attn_w = np.asarray(c_attn_w, np.float32)
    c_attn_b = np.asarray(c_attn_b, np.float32)
    c_proj_w = np.asarray(c_proj_w, np.float32)
    c_proj_b = np.asarray(c_proj_b, np.float32)
    up_w = np.asarray(up_w, np.float32); up_b = np.asarray(up_b, np.float32)
    down_w = np.asarray(down_w, np.float32)
    down_b = np.asarray(down_b, np.float32)

    Wa = c_attn_w * ln1_w[None, :]                  # fold LN1 scale
    ba = c_attn_b + c_attn_w @ ln1_b                # fold LN1 shift
    Wq, Wk, Wv = Wa[:C], Wa[C:2 * C], Wa[2 * C:]
    bq, bk, bv = ba[:C], ba[C:2 * C], ba[2 * C:]
    s = 1.0 / np.sqrt(D)
    Wq = Wq * s; bq = bq * s                        # fold attention scale

    Wup = up_w * ln2_w[None, :]
    bupv = up_b + up_w @ ln2_b

    def b2t(v, n):   # per-partition bias layout [128, n]
        return np.ascontiguousarray(v.reshape(n, P).T.astype(np.float32))

    diagm = np.triu(np.ones((P, P), np.float32))    # kv row <= q col

    shared = {
        "WupT": np.ascontiguousarray(Wup.T).astype(BF),
        "WdownT": np.ascontiguousarray(down_w.T).astype(BF),
        "bup": b2t(bupv, 32), "bdown": b2t(down_b, 8),
        "diagm": diagm.astype(BF),
    }

    # residual with proj bias and (v-bias pushed through proj) folded in
    xb = x + (c_proj_b + c_proj_w @ bv)[None, None, :]

    in_maps, tok_slices = [], []
    for core in range(8):
        b, i = core // 4, core % 4
        ch = slice(i * 256, (i + 1) * 256)          # this core's attn channels
        wqk = np.concatenate([Wq[ch], Wk[ch]], axis=0)      # [512, 1024]
        m = dict(shared)
        m["WqkT"] = np.ascontiguousarray(wqk.T).astype(BF)
        m["WvT"] = np.ascontiguousarray(Wv[ch].T).astype(BF)
        m["WpT"] = np.ascontiguousarray(c_proj_w[:, ch].T).astype(BF)
        m["bqk"] = b2t(np.concatenate([bq[ch], bk[ch]]), 4)
        m["gT"] = np.ascontiguousarray(g[b].T).astype(BF)
        strips = [slice(1024 * r + STRIP * i, 1024 * r + STRIP * (i + 1))
                  for r in range(2)]
        xbT = np.concatenate([xb[b, st].T for st in strips], axis=1)
        m["xbT"] = np.ascontiguousarray(xbT).astype(np.float32)
        in_maps.append(m)
        tok_slices.append((b, strips))
    return in_maps, tok_slices


def kernel(**inputs):
    global _CACHED_NC
    if _CACHED_NC is None:
        _CACHED_NC = _build_nc()
    nc = _CACHED_NC
    in_maps, tok_slices = _prep_inputs(**inputs)
    try:
        res = run_bass_kernel_spmd(nc, in_maps, list(range(8)))
    except Exception:
        # one retry: transient NRT device faults are recoverable on re-run
        res = run_bass_kernel_spmd(nc, in_maps, list(range(8)))
    out = np.empty((B, T, C), np.float32)
    for core in range(8):
        o = res.results[core]["OUT"]                # [C, 512]
        b, strips = tok_slices[core]
        for r, st in enumerate(strips):
            out[b, st, :] = o[:, r * STRIP:(r + 1) * STRIP].T
    return out


# revision 7
# speedup vs baseline: 1.0406x; 1.0027x over previous
"""Trainium2 Bass kernel for a GPT-style transformer block (B=2, T=2048,
C=1024, 16 heads, MLP 4x), sharded across 8 NeuronCores.

Sharding: attention is HEAD-sharded. Core c = (b, i) with b = c//4,
i = c%4 owns heads [4i, 4i+4) of batch b for the whole 2048-token range:
it computes q/k/v for just those 256 channels (same FLOPs as a row shard
of all channels), runs causal attention with no cross-core kv exchange,
then produces token-major c_proj partials. Two ReduceScatters (one per
1024-token chunk) sum the partials over the 4-core batch group and hand
each core a contiguous 256-token strip per chunk; LN2+MLP then run
data-parallel on the core's two strips.

Host precomputes LN1 (inputs-only), folds LN scale/shift into the matmul
weights, folds 1/sqrt(D) into Wq, and folds the v-bias + proj-bias into
the residual (softmax weights sum to 1, so attn_out = av/den + bv and
proj(attn_out) = proj(av/den) + Wp@bv). Causality lives in the loop
bounds; only diagonal 128x128 tiles get a multiplicative triu mask.
"""
import numpy as np
import ml_dtypes

import concourse.bass as bass
import concourse.mybir as mybir
import concourse.tile as tile
import concourse.bacc as bacc
from concourse.bass_utils import run_bass_kernel_spmd

BF = ml_dtypes.bfloat16
P = 128
B, T, C, D, F = 2, 2048, 1024, 64, 4096
NCT = C // P            # 8   c-tiles
NFT = F // P            # 32  f-tiles
NTT = T // P            # 16  token tiles
HC = 4                  # heads per core
SPAN = 512              # q/token span
NSPAN = T // SPAN       # 4
STRIP = 256             # tokens owned per core per RS chunk
EPS = 1e-5
f32 = mybir.dt.float32
bf16 = mybir.dt.bfloat16
AF = mybir.ActivationFunctionType

_CACHED_NC = None
import os as _os
DBG = _os.environ.get("KDBG", "")


def _build_nc():
    nc = bacc.Bacc("TRN2", target_bir_lowering=False, debug=False)
    d = {}
    for name, shape, dt in [
        ("gT", [C, T], bf16),          # ln1(x) for the whole batch, ch-major
        ("WqkT", [C, 512], bf16),      # q(256) | k(256) out channels
        ("WvT", [C, 256], bf16),
        ("WpT", [256, C], bf16),       # [attn-ch, out-ch] for moving use
        ("WupT", [C, F], bf16),
        ("WdownT", [F, C], bf16),
        ("bqk", [P, 4], f32),
        ("bup", [P, 32], f32),
        ("bdown", [P, 8], f32),
        ("xbT", [C, 512], f32),        # residual for own tokens (biases folded)
        ("diagm", [P, P], bf16),       # triu causal mask for diagonal tiles
    ]:
        d[name] = nc.dram_tensor(name, shape, dt, kind="ExternalInput").ap()
    d["OUT"] = nc.dram_tensor("OUT", [C, 512], f32, kind="ExternalOutput").ap()

    with tile.TileContext(nc) as tc:
        _emit(nc, tc, d)
    nc.compile()
    return nc


def _emit(nc, tc, d):
    from contextlib import ExitStack

    with ExitStack() as ctx:
        # ---------------- long-lived tiles ----------------
        cpool = ctx.enter_context(tc.tile_pool(name="cpool", bufs=1))
        attnT = cpool.tile([P, 2, T], bf16, name="attnT")
        bqk = cpool.tile([P, 4], f32, name="bqk")
        bup = cpool.tile([P, 32], f32, name="bup")
        bdown = cpool.tile([P, 8], f32, name="bdown")
        diagm = cpool.tile([P, P], bf16, name="diagm")
        epsT = cpool.tile([P, 1], f32, name="epsT")
        onesb = cpool.tile([P, P], bf16, name="onesb")
        selp = cpool.tile([P, P], bf16, name="selp")
        gate = cpool.tile([P, 1], f32, name="gate")
        nc.vector.memset(epsT[:], EPS)
        nc.vector.memset(onesb[:], 1.0)
        nc.vector.memset(selp[:], 0.0)
        nc.vector.memset(selp[0:1, 0:64], 1.0)
        nc.vector.memset(selp[64:65, 64:128], 1.0)

        # DRAM scratch for the proj partials + RS outputs
        dramp = ctx.enter_context(tc.tile_pool(name="dramp", bufs=1,
                                               space="DRAM"))
        partials = [dramp.tile([1024, C], bf16, name=f"part{r}")
                    for r in range(2)]
        rsout = [dramp.tile([STRIP, C], bf16, name=f"rsout{r}")
                 for r in range(2)]

        # MLP weights / residual / strip tiles: allocated BEFORE the
        # attention pools so strip-0's MLP can run while attention finishes
        # (tiles in later pools would inherit waits on attention SBUF reuse).
        wmlp = ctx.enter_context(tc.tile_pool(name="wmlp", bufs=1))
        wd = wmlp.tile([P, NFT, C], bf16, name="wd")
        x1T = wmlp.tile([P, NCT, 512], f32, name="x1T")    # xb, then x1
        mep = ctx.enter_context(tc.tile_pool(name="mep", bufs=1))

        # attention operand tiles (freed after proj)
        bpool_cm = tc.tile_pool(name="bpool", bufs=1)
        bpool = bpool_cm.__enter__()
        qT = bpool.tile([P, 2, T], bf16, name="qT")
        kT = bpool.tile([P, 2, T], bf16, name="kT")
        v_aug = bpool.tile([P, HC, NTT, 65], bf16, name="v_aug")
        wp_sb = bpool.tile([P, 2, C], bf16, name="wp_sb")
        nc.vector.memset(v_aug[:, :, :, 64:65], 1.0)

        # ---------------- qkv projections ----------------
        with tc.tile_pool(name="gpool", bufs=2) as gpool, \
             tc.tile_pool(name="wqkp", bufs=1) as wqkp, \
             tc.tile_pool(name="qkps", bufs=3, space="PSUM") as qkps:
            wqk = wqkp.tile([P, NCT, 512], bf16, name="wqk")
            wv = wqkp.tile([P, NCT, 256], bf16, name="wv")
            wqksrc = d["WqkT"].rearrange("(ct p) o -> p ct o", p=P)
            gsrc = d["gT"].rearrange("(ct p) t -> p ct t", p=P)
            # startup-critical order: q weights, first g chunk, then the rest
            nc.sync.dma_start(wqk[:, 0:4, 0:256], wqksrc[:, 0:4, 0:256])
            g0 = gpool.tile([P, NCT, 256], bf16, name="g", tag="g")
            nc.sync.dma_start(g0[:, 0:4, :], gsrc[:, 0:4, 0:256])
            nc.sync.dma_start(wqk[:, 4:8, 0:256], wqksrc[:, 4:8, 0:256])
            nc.sync.dma_start(g0[:, 4:8, :], gsrc[:, 4:8, 0:256])
            nc.sync.dma_start(wqk[:, :, 256:512], wqksrc[:, :, 256:512])
            nc.sync.dma_start(wv[:],
                              d["WvT"].rearrange("(ct p) o -> p ct o", p=P))
            for t, key in [(bqk, "bqk"), (bup, "bup"), (bdown, "bdown"),
                           (diagm, "diagm")]:
                nc.sync.dma_start(t[:], d[key])
            nc.sync.dma_start(wp_sb[:],
                              d["WpT"].rearrange("(j p) o -> p j o", p=P))
            for hh in range(2 * NSPAN):      # half-spans of 256 tokens
                c0 = hh * 256
                if hh == 0:
                    g = g0
                else:
                    g = gpool.tile([P, NCT, 256], bf16, name="g", tag="g")
                    nc.sync.dma_start(g[:], gsrc[:, :, c0:c0 + 256])
                for ot in range(4):          # q0 q1 k0 k1
                    ps = qkps.tile([P, 256], f32, name="ps", tag="qk")
                    for ct in range(NCT):
                        nc.tensor.matmul(ps[:], wqk[:, ct, ot * P:(ot + 1) * P],
                                         g[:, ct, :],
                                         start=(ct == 0), stop=(ct == NCT - 1))
                    dstT = qT if ot < 2 else kT
                    nc.scalar.add(dstT[:, ot % 2, c0:c0 + 256],
                                  ps[:], bqk[:, ot:ot + 1])
                for tl in range(2):          # v, token tile kt = 2*hh+tl
                    kt = 2 * hh + tl
                    pv = qkps.tile([P, 256], f32, name="pv", tag="qk")
                    for ct in range(NCT):
                        nc.tensor.matmul(pv[:], g[:, ct, tl * P:(tl + 1) * P],
                                         wv[:, ct, :],
                                         start=(ct == 0), stop=(ct == NCT - 1))
                    nc.scalar.copy(
                        v_aug[:, :, kt, 0:64],
                        pv[:].rearrange("p (h dd) -> p h dd", dd=64))
            # stage the big loads (SP queue, after the startup-critical ones)
            wdsrc = d["WdownT"].rearrange("(cf p) o -> p cf o", p=P)
            for cc in range(8):
                nc.sync.dma_start(wd[:, cc * 4:(cc + 1) * 4, :],
                                  wdsrc[:, cc * 4:(cc + 1) * 4, :])
            nc.sync.dma_start(x1T[:],
                              d["xbT"].rearrange("(ct p) t -> p ct t", p=P))

        # ----- strip-MLP emission helpers -----
        wusrc = d["WupT"].rearrange("(ct p) f -> p ct f", p=P)
        outdst = d["OUT"].rearrange("(ot p) t -> p ot t", p=P)

        def emit_wuw(w):
            t = mep.tile([P, NCT, 1024], bf16, name="wuw", tag="wuw", bufs=2)
            nc.sync.dma_start(
                t[:], wusrc[:, :, (w % 4) * 1024:((w % 4) + 1) * 1024])
            return t

        def emit_rs_transpose(r):
            rsT = mep.tile([P, NCT, STRIP], bf16, name="rsT", tag="rsT")
            nc.sync.dma_start_transpose(rsT[:], rsout[r][:])
            return rsT

        def emit_strip_pre(r, rsT, upps, gate_ex=None):
            cs = r * STRIP
            if gate_ex is not None:
                # scheduler-proofing: root the chain on a late attention exp
                # so its long RS-wait cannot head-block the DVE queue ahead
                # of ready attention work (the scheduler's internal sim
                # underestimates collective latency)
                gate2 = mep.tile([P, 1], f32, name="gate2", tag="gate2")
                nc.vector.tensor_scalar(gate2[:], gate_ex[:, 511:512],
                                        0.0, 1.0, mybir.AluOpType.mult,
                                        mybir.AluOpType.add)
                rsTg = mep.tile([P, NCT, STRIP], bf16, name="rsTg",
                                tag="x1b")
                nc.vector.tensor_scalar(rsTg[:], rsT[:], gate2[:, 0:1], None,
                                        mybir.AluOpType.mult)
                rsT = rsTg
            x1b = mep.tile([P, NCT, STRIP], bf16, name="x1b", tag="x1b")
            if r == 1 and DBG == "":
                # post-attention strip: bf16 x1 first so the LN2 stat
                # matmuls start one DVE op earlier; fp32 residual update
                # follows off the critical path
                nc.vector.tensor_add(x1b[:], x1T[:, :, cs:cs + STRIP],
                                     rsT[:])
                nc.vector.tensor_add(x1T[:, :, cs:cs + STRIP],
                                     x1T[:, :, cs:cs + STRIP], rsT[:])
            else:
                nc.vector.tensor_add(x1T[:, :, cs:cs + STRIP],
                                     x1T[:, :, cs:cs + STRIP], rsT[:])
                if DBG == "x1":
                    nc.sync.dma_start(outdst[:, :, cs:cs + STRIP],
                                      x1T[:, :, cs:cs + STRIP])
                    return None
                nc.vector.tensor_copy(x1b[:], x1T[:, :, cs:cs + STRIP])
            sqb = mep.tile([P, NCT, STRIP], bf16, name="sqb", tag="g28")
            for ct in range(NCT):
                nc.vector.tensor_mul(sqb[:, ct, :], x1b[:, ct, :],
                                     x1b[:, ct, :])
            psmu = upps.tile([P, STRIP], f32, name="psmu", tag="pu")
            for ct in range(NCT):
                nc.tensor.matmul(psmu[:], onesb[:], x1b[:, ct, :],
                                 start=(ct == 0), stop=(ct == NCT - 1))
            pssq = upps.tile([P, STRIP], f32, name="pssq", tag="pu")
            for ct in range(NCT):
                nc.tensor.matmul(pssq[:], onesb[:], sqb[:, ct, :],
                                 start=(ct == 0), stop=(ct == NCT - 1))
            mu = mep.tile([P, STRIP], f32, name="mu", tag="mu")
            e2 = mep.tile([P, STRIP], f32, name="e2", tag="e2")
            std = mep.tile([P, STRIP], f32, name="std", tag="stdt")
            nc.scalar.mul(mu[:], psmu[:], 1.0 / C)
            nc.scalar.mul(e2[:], pssq[:], 1.0 / C)
            musq = mep.tile([P, STRIP], f32, name="musq", tag="tmpc", bufs=2)
            nc.vector.tensor_mul(musq[:], mu[:], mu[:])
            nc.vector.tensor_sub(e2[:], e2[:], musq[:])
            nc.scalar.activation(std[:], e2[:], AF.Sqrt, bias=epsT[:])
            nc.vector.reciprocal(std[:], std[:])
            g2 = mep.tile([P, NCT, STRIP], bf16, name="g2", tag="g28")
            for ct in range(NCT):
                tmpc = mep.tile([P, STRIP], f32, name="tmpc", tag="tmpc",
                                bufs=2)
                nc.vector.tensor_sub(tmpc[:], x1T[:, ct, cs:cs + STRIP],
                                     mu[:])
                nc.vector.tensor_mul(g2[:, ct, :], tmpc[:], std[:])
            if DBG == "g2":
                g2f = mep.tile([P, NCT, STRIP], f32, name="g2f", tag="g2f")
                nc.vector.tensor_copy(g2f[:], g2[:])
                nc.sync.dma_start(outdst[:, :, cs:cs + STRIP], g2f[:])
                return None
            hT = mep.tile([P, NFT, STRIP], bf16, name="hT", tag="hT")
            return dict(cs=cs, g2=g2, hT=hT)

        def emit_up_group(st, f, wt, defer_gelu=False):
            pu = upps_ref[0].tile([P, STRIP], f32, name="pu", tag="pu")
            fl = f % 8
            for ct in range(NCT):
                nc.tensor.matmul(pu[:], wt[:, ct, fl * P:(fl + 1) * P],
                                 st["g2"][:, ct, :],
                                 start=(ct == 0), stop=(ct == NCT - 1))
            if defer_gelu:
                # keep Gelu out of the attention window: its act table
                # does not share a set with Exp, so inline gelus thrash
                # 1.28us table loads per switch.  Stage raw pre-act.
                nc.vector.tensor_copy(st["hT"][:, f, :], pu[:])
            else:
                nc.scalar.activation(st["hT"][:, f, :], pu[:], AF.Gelu,
                                     bias=bup[:, f:f + 1])

        def emit_down_out(st, dnps):
            for ot in range(8):
                pd = dnps.tile([P, STRIP], f32, name="pd", tag="dn")
                for cf in range(NFT):
                    nc.tensor.matmul(pd[:], wd[:, cf, ot * P:(ot + 1) * P],
                                     st["hT"][:, cf, :],
                                     start=(cf == 0), stop=(cf == NFT - 1))
                td = mep.tile([P, STRIP], f32, name="td", tag="td", bufs=2)
                nc.scalar.add(td[:], pd[:], bdown[:, ot:ot + 1])
                ox = mep.tile([P, STRIP], f32, name="ox", tag="ox", bufs=2)
                nc.vector.tensor_add(ox[:], td[:],
                                     x1T[:, ot, st["cs"]:st["cs"] + STRIP])
                nc.gpsimd.dma_start(outdst[:, ot, st["cs"]:st["cs"] + STRIP],
                                    ox[:])

        # ---------------- attention + proj + RS (+ strip0 MLP fill) --------
        wuw = {}
        if not DBG:
            wuw[0] = emit_wuw(0)
            wuw[1] = emit_wuw(1)
        st0 = None
        rsT0 = None
        rsT1 = None
        last_ex = None
        upf = [0]        # next strip0 up f-group to emit

        with tc.tile_pool(name="avps", bufs=4, space="PSUM") as avps, \
             tc.tile_pool(name="expp", bufs=4) as expp, \
             tc.tile_pool(name="nrmp", bufs=2) as nrmp, \
             tc.tile_pool(name="prst", bufs=2) as prst:

            def span_tail(Q, avs, mkpsum):
                # normalize: attnT[:, pr, span] = av / den, then proj + RS
                q0 = Q * SPAN
                for pr in range(2):
                    denb = nrmp.tile([P, 512], bf16, name="denb", tag="denb")
                    nc.vector.memset(denb[0:65, :], 0.0)
                    with nc.allow_low_precision(reason="bf16 softmax denom"):
                        nc.vector.reciprocal(denb[0:1, :],
                                             avs[2 * pr][64:65, :])
                        nc.vector.reciprocal(denb[64:65, :],
                                             avs[2 * pr + 1][64:65, :])
                    bc = mkpsum()
                    nc.tensor.matmul(bc[:], selp[0:65, :], denb[0:65, :],
                                     start=True, stop=True)
                    bcs = nrmp.tile([P, 512], f32, name="bcs", tag="bcs")
                    nc.scalar.copy(bcs[:], bc[:])
                    nc.vector.tensor_mul(attnT[0:64, pr, q0:q0 + SPAN],
                                         avs[2 * pr][0:64, :], bcs[0:64, :])
                    nc.vector.tensor_mul(attnT[64:128, pr, q0:q0 + SPAN],
                                         avs[2 * pr + 1][0:64, :],
                                         bcs[64:128, :])
                for tl in range(4):
                    tt = 4 * Q + tl
                    stage = prst.tile([P, 1024], bf16, name="stage", tag="st")
                    for half in range(2):
                        pp = avps.tile([P, 512], f32, name="pp", tag="av")
                        for j in range(2):
                            nc.tensor.matmul(
                                pp[:],
                                attnT[:, j, tt * P:(tt + 1) * P],
                                wp_sb[:, j, half * 512:(half + 1) * 512],
                                start=(j == 0), stop=(j == 1))
                        nc.vector.tensor_copy(
                            stage[:, half * 512:(half + 1) * 512], pp[:])
                    row = (Q % 2) * 512 + tl * P
                    nc.gpsimd.dma_start(partials[Q // 2][row:row + P, :],
                                        stage[:])
                if Q % 2 == 1:
                    r = Q // 2
                    nc.gpsimd.collective_compute(
                        "ReduceScatter", mybir.AluOpType.add,
                        ins=[partials[r].opt()], outs=[rsout[r].opt()],
                        replica_groups=[[0, 1, 2, 3], [4, 5, 6, 7]])
                    return emit_rs_transpose(r)
                return None

            # ---- spans 0-2: paired scores, one exp per head pair ----
            scp_cm = tc.tile_pool(name="scp", bufs=2, space="PSUM")
            scp = scp_cm.__enter__()

            def mkpsum_pair():
                t = scp.tile([P, 2, 512], f32, name="sc2", tag="sc")
                return t[:, 0, :]

            for Q in range(3):
                q0 = Q * SPAN
                avs = []
                for h in range(HC):
                    av = avps.tile([P, 512], f32, name=f"av{h}", tag="av")
                    avs.append(av)
                nkt = 4 * Q + 4
                for kt in range(nkt):
                    p_ = kt - 4 * Q
                    c0 = 128 * p_ if p_ > 0 else 0
                    sps = []
                    for pr in range(2):
                        sc2 = scp.tile([P, 2, 512], f32, name="sc2", tag="sc")
                        for i in range(2):
                            h = 2 * pr + i
                            hb = (h % 2) * 64
                            j = h // 2
                            nc.tensor.matmul(
                                sc2[:, i, c0:512],
                                kT[hb:hb + 64, j, kt * P:(kt + 1) * P],
                                qT[hb:hb + 64, j, q0 + c0:q0 + 512],
                                start=True, stop=True)
                        sps.append(sc2)
                    for pr in range(2):
                        ex2 = expp.tile([P, 2, 512], bf16, name="ex2",
                                        tag="ex")
                        nc.scalar.activation(ex2[:, :, c0:512],
                                             sps[pr][:, :, c0:512], AF.Exp)
                        for i in range(2):
                            h = 2 * pr + i
                            if p_ >= 0:
                                nc.vector.tensor_mul(ex2[:, i, c0:c0 + 128],
                                                     ex2[:, i, c0:c0 + 128],
                                                     diagm[:])
                            nc.tensor.matmul(
                                avs[h][0:65, c0:512],
                                v_aug[:, h, kt, 0:65],
                                ex2[:, i, c0:512],
                                start=(kt == 0), stop=(kt == nkt - 1),
                                skip_group_check=True)
                rsT = span_tail(Q, avs, mkpsum_pair)
                if rsT is not None:
                    rsT0 = rsT
                    if not DBG:
                        wuw[2] = emit_wuw(2)
                        wuw[3] = emit_wuw(3)

            scp_cm.__exit__(None, None, None)

            # ---- span 3: single scores + strip0 MLP fill ----
            with tc.tile_pool(name="sc3p", bufs=2, space="PSUM") as scps, \
                 tc.tile_pool(name="upps", bufs=2, space="PSUM") as upps:
                upps_ref = [upps]

                def fill_slot(kt):
                    nonlocal st0
                    if DBG:
                        return
                    if kt == 4:
                        st0 = emit_strip_pre(0, rsT0, upps, gate_ex=last_ex)
                        return
                    if st0 is None or upf[0] >= NFT:
                        return
                    for _ in range(2):
                        if upf[0] >= NFT:
                            break
                        emit_up_group(st0, upf[0], wuw[upf[0] // 8],
                                      defer_gelu=True)
                        upf[0] += 1

                Q = 3
                q0 = Q * SPAN
                avs = []
                for h in range(HC):
                    av = avps.tile([P, 512], f32, name=f"av{h}", tag="av")
                    avs.append(av)
                nkt = 16
                for kt in range(nkt):
                    p_ = kt - 4 * Q
                    c0 = 128 * p_ if p_ > 0 else 0
                    scs = []
                    for h in range(HC):
                        hb = (h % 2) * 64
                        j = h // 2
                        sc = scps.tile([P, 512], f32, name="sc", tag="sc")
                        nc.tensor.matmul(
                            sc[:, c0:512],
                            kT[hb:hb + 64, j, kt * P:(kt + 1) * P],
                            qT[hb:hb + 64, j, q0 + c0:q0 + 512],
                            start=True, stop=True)
                        scs.append(sc)
                    for h in range(HC):
                        ex = expp.tile([P, 512], bf16, name="ex", tag="ex")
                        last_ex = ex
                        nc.scalar.activation(ex[:, c0:512], scs[h][:, c0:512],
                                             AF.Exp)
                        if p_ >= 0:
                            nc.vector.tensor_mul(ex[:, c0:c0 + 128],
                                                 ex[:, c0:c0 + 128], diagm[:])
                        nc.tensor.matmul(
                            avs[h][0:65, c0:512],
                            v_aug[:, h, kt, 0:65],
                            ex[:, c0:512],
                            start=(kt == 0), stop=(kt == nkt - 1),
                            skip_group_check=True)
                    fill_slot(kt)

                def mkpsum_single():
                    return scps.tile([P, 512], f32, name="pp", tag="sc")

                rsT1 = span_tail(3, avs, mkpsum_single)
                if not DBG:
                    for w in range(4, 8):
                        wuw[w] = emit_wuw(w)
                # rest of strip0's ups (overlap RS2 on the collective cores)
                if not DBG and st0 is not None:
                    while upf[0] < NFT:
                        emit_up_group(st0, upf[0], wuw[upf[0] // 8],
                                      defer_gelu=True)
                        upf[0] += 1
                # gate = 1.0, data-dependent on the last exp: ops scaled by
                # it cannot be scheduled into the attention exp stream
                nc.vector.tensor_scalar(gate[:], last_ex[:, 511:512], 0.0,
                                        1.0, mybir.AluOpType.mult,
                                        mybir.AluOpType.add)
                if DBG:
                    rsT0x = rsT0 if rsT0 is not None else emit_rs_transpose(0)
                    emit_strip_pre(0, rsT0x, upps)
                    emit_strip_pre(1, rsT1, upps)

        bpool_cm.__exit__(None, None, None)   # free qT / kT / v_aug / wp_sb

        # ---------------- strip0 down + full strip1 ----------------
        if not DBG:
            with tc.tile_pool(name="dnps", bufs=3, space="PSUM") as dnps, \
                 tc.tile_pool(name="up2", bufs=3, space="PSUM") as up2:
                upps_ref[0] = up2
                # bulk gelu for strip0's staged pre-activations (one table
                # switch, after all attention exps are done; gate enforces it)
                for f in range(NFT):
                    nc.scalar.activation(st0["hT"][:, f, :], st0["hT"][:, f, :],
                                         AF.Gelu, bias=bup[:, f:f + 1],
                                         scale=gate[:, 0:1])
                emit_down_out(st0, dnps)
                st1 = emit_strip_pre(1, rsT1, up2)
                for f in range(NFT):
                    emit_up_group(st1, f, wuw[4 + f // 8])
                emit_down_out(st1, dnps)


def _prep_inputs(x, ln1_w, ln1_b, c_attn_w, c_attn_b, c_proj_w, c_proj_b,
                 ln2_w, ln2_b, up_w, up_b, down_w, down_b):
    """Host-side preprocessing -> list of 8 per-core input dicts."""
    x = np.asarray(x, np.float32)
    f64 = np.float64
    mu = x.mean(-1, keepdims=True, dtype=f64)
    var = np.asarray(x, f64).var(-1, keepdims=True)
    g = ((x - mu) / np.sqrt(var + EPS)).astype(np.float32)     # [B, T, C]

    ln1_w = np.asarray(ln1_w, np.float32); ln1_b = np.asarray(ln1_b, np.float32)
    ln2_w = np.asarray(ln2_w, np.float32); ln2_b = np.asarray(ln2_b, np.float32)
    c_attn_w = np.asarray(c_attn_w, np.float32)
    c_attn_b = np.asarray(c_attn_b, np.float32)
    c_proj_w = np.asarray(c_proj_w, np.float32)
    c_proj_b = np.asarray(c_proj_b, np.float32)
    up_w = np.asarray(up_w, np.float32); up_b = np.asarray(up_b, np.float32)
    down_w = np.asarray(down_w, np.float32)
    down_b = np.asarray(down_b, np.float32)

    Wa = c_attn_w * ln1_w[None, :]                  # fold LN1 scale
    ba = c_attn_b + c_attn_w @ ln1_b                # fold LN1 shift
    Wq, Wk, Wv = Wa[:C], Wa[C:2 * C], Wa[2 * C:]
    bq, bk, bv = ba[:C], ba[C:2 * C], ba[2 * C:]
    s = 1.0 / np.sqrt(D)
    Wq = Wq * s; bq = bq * s                        # fold attention scale

    Wup = up_w * ln2_w[None, :]
    bupv = up_b + up_w @ ln2_b

    def b2t(v, n):   # per-partition bias layout [128, n]
        return np.ascontiguousarray(v.reshape(n, P).T.astype(np.float32))

    diagm = np.triu(np.ones((P, P), np.float32))    # kv row <= q col

    shared = {
        "WupT": np.ascontiguousarray(Wup.T).astype(BF),
        "WdownT": np.ascontiguousarray(down_w.T).astype(BF),
        "bup": b2t(bupv, 32), "bdown": b2t(down_b, 8),
        "diagm": diagm.astype(BF),
    }

    # residual with proj bias and (v-bias pushed through proj) folded in
    xb = x + (c_proj_b + c_proj_w @ bv)[None, None, :]

    in_maps, tok_slices = [], []
    for core in range(8):
        b, i = core // 4, core % 4
        ch = slice(i * 256, (i + 1) * 256)          # this core's attn channels
        wqk = np.concatenate([Wq[ch], Wk[ch]], axis=0)      # [512, 1024]
        m = dict(shared)
        m["WqkT"] = np.ascontiguousarray(wqk.T).astype(BF)
        m["WvT"] = np.ascontiguousarray(Wv[ch].T).astype(BF)
        m["WpT"] = np.ascontiguousarray(c_proj_w[:, ch].T).astype(BF)
        m["bqk"] = b2t(np.concatenate([bq[ch], bk[ch]]), 4)
        m["gT"] = np.ascontiguousarray(g[b].T).astype(BF)
        strips = [slice(1024 * r + STRIP * i, 1024 * r + STRIP * (i + 1))
                  for r in range(2)]
        xbT = np.concatenate([xb[b, st].T for st in strips], axis=1)
        m["xbT"] = np.ascontiguousarray(xbT).astype(np.float32)
        in_maps.append(m)
        tok_slices.append((b, strips))
    return in_maps, tok_slices


def kernel(**inputs):
    global _CACHED_NC
    if _CACHED_NC is None:
        _CACHED_NC = _build_nc()
    nc = _CACHED_NC
    in_maps, tok_slices = _prep_inputs(**inputs)
    try:
        res = run_bass_kernel_spmd(nc, in_maps, list(range(8)))
    except Exception:
        # one retry: transient NRT device faults are recoverable on re-run
        res = run_bass_kernel_spmd(nc, in_maps, list(range(8)))
    out = np.empty((B, T, C), np.float32)
    for core in range(8):
        o = res.results[core]["OUT"]                # [C, 512]
        b, strips = tok_slices[core]
        for r, st in enumerate(strips):
            out[b, st, :] = o[:, r * STRIP:(r + 1) * STRIP].T
    return out


# revision 8
# speedup vs baseline: 1.0649x; 1.0233x over previous
"""Trainium2 Bass kernel for a GPT-style transformer block (B=2, T=2048,
C=1024, 16 heads, MLP 4x), sharded across 8 NeuronCores.

Sharding: attention is HEAD-sharded. Core c = (b, i) with b = c//4,
i = c%4 owns heads [4i, 4i+4) of batch b for the whole 2048-token range:
it computes q/k/v for just those 256 channels (same FLOPs as a row shard
of all channels), runs causal attention with no cross-core kv exchange,
then produces token-major c_proj partials. Two ReduceScatters (one per
1024-token chunk) sum the partials over the 4-core batch group and hand
each core a contiguous 256-token strip per chunk; LN2+MLP then run
data-parallel on the core's two strips.

Host precomputes LN1 (inputs-only), folds LN scale/shift into the matmul
weights, folds 1/sqrt(D) into Wq, and folds the v-bias + proj-bias into
the residual (softmax weights sum to 1, so attn_out = av/den + bv and
proj(attn_out) = proj(av/den) + Wp@bv). Causality lives in the loop
bounds; only diagonal 128x128 tiles get a multiplicative triu mask.
"""
import numpy as np
import ml_dtypes

import concourse.bass as bass
import concourse.mybir as mybir
import concourse.tile as tile
import concourse.bacc as bacc
from concourse.bass_utils import run_bass_kernel_spmd

BF = ml_dtypes.bfloat16
P = 128
B, T, C, D, F = 2, 2048, 1024, 64, 4096
NCT = C // P            # 8   c-tiles
NFT = F // P            # 32  f-tiles
NTT = T // P            # 16  token tiles
HC = 4                  # heads per core
SPAN = 512              # q/token span
NSPAN = T // SPAN       # 4
STRIP = 256             # tokens owned per core per RS chunk
EPS = 1e-5
f32 = mybir.dt.float32
bf16 = mybir.dt.bfloat16
AF = mybir.ActivationFunctionType

_CACHED_NC = None
import os as _os
DBG = _os.environ.get("KDBG", "")


def _build_nc():
    nc = bacc.Bacc("TRN2", target_bir_lowering=False, debug=False)
    d = {}
    for name, shape, dt in [
        ("gT", [C, T], bf16),          # ln1(x) for the whole batch, ch-major
        ("WqkT", [C, 512], bf16),      # q(256) | k(256) out channels
        ("WvT", [C, 256], bf16),
        ("WpT", [256, C], bf16),       # [attn-ch, out-ch] for moving use
        ("WupT", [C, F], bf16),
        ("WdownT", [F, C], bf16),
        ("bqk", [P, 4], f32),
        ("bup", [P, 32], f32),
        ("bdown", [P, 8], f32),
        ("xbT", [C, 512], f32),        # residual for own tokens (biases folded)
        ("diagm", [P, P], bf16),       # triu causal mask for diagonal tiles
    ]:
        d[name] = nc.dram_tensor(name, shape, dt, kind="ExternalInput").ap()
    d["OUT"] = nc.dram_tensor("OUT", [C, 512], f32, kind="ExternalOutput").ap()

    with tile.TileContext(nc) as tc:
        _emit(nc, tc, d)
    nc.compile()
    return nc


def _emit(nc, tc, d):
    from contextlib import ExitStack

    with ExitStack() as ctx:
        # ---------------- long-lived tiles ----------------
        cpool = ctx.enter_context(tc.tile_pool(name="cpool", bufs=1))
        attnT = cpool.tile([P, 2, T], bf16, name="attnT")
        bqk = cpool.tile([P, 4], f32, name="bqk")
        bup = cpool.tile([P, 32], f32, name="bup")
        bdown = cpool.tile([P, 8], f32, name="bdown")
        diagm = cpool.tile([P, P], bf16, name="diagm")
        epsT = cpool.tile([P, 1], f32, name="epsT")
        onesb = cpool.tile([P, P], bf16, name="onesb")
        selp = cpool.tile([P, P], bf16, name="selp")
        gate = cpool.tile([P, 1], f32, name="gate")
        nc.vector.memset(epsT[:], EPS)
        nc.vector.memset(onesb[:], 1.0)
        nc.vector.memset(selp[:], 0.0)
        nc.vector.memset(selp[0:1, 0:64], 1.0)
        nc.vector.memset(selp[64:65, 64:128], 1.0)

        # DRAM scratch for the proj partials + RS outputs
        dramp = ctx.enter_context(tc.tile_pool(name="dramp", bufs=1,
                                               space="DRAM"))
        partials = [dramp.tile([1024, C], bf16, name=f"part{r}")
                    for r in range(2)]
        rsout = [dramp.tile([STRIP, C], bf16, name=f"rsout{r}")
                 for r in range(2)]

        # MLP weights / residual / strip tiles: allocated BEFORE the
        # attention pools so strip-0's MLP can run while attention finishes
        # (tiles in later pools would inherit waits on attention SBUF reuse).
        wmlp = ctx.enter_context(tc.tile_pool(name="wmlp", bufs=1))
        wd = wmlp.tile([P, NFT, C], bf16, name="wd")
        x1T = wmlp.tile([P, NCT, 512], f32, name="x1T")    # xb, then x1
        mep = ctx.enter_context(tc.tile_pool(name="mep", bufs=1))

        # attention operand tiles (freed after proj)
        bpool_cm = tc.tile_pool(name="bpool", bufs=1)
        bpool = bpool_cm.__enter__()
        qT = bpool.tile([P, 2, T], bf16, name="qT")
        kT = bpool.tile([P, 2, T], bf16, name="kT")
        v_aug = bpool.tile([P, HC, NTT, 65], bf16, name="v_aug")
        wp_sb = bpool.tile([P, 2, C], bf16, name="wp_sb")
        nc.vector.memset(v_aug[:, :, :, 64:65], 1.0)

        # ---------------- qkv projections ----------------
        with tc.tile_pool(name="gpool", bufs=2) as gpool, \
             tc.tile_pool(name="wqkp", bufs=1) as wqkp, \
             tc.tile_pool(name="qkps", bufs=4, space="PSUM") as qkps:
            wqk = wqkp.tile([P, NCT, 512], bf16, name="wqk")
            wv = wqkp.tile([P, NCT, 256], bf16, name="wv")
            wqksrc = d["WqkT"].rearrange("(ct p) o -> p ct o", p=P)
            gsrc = d["gT"].rearrange("(ct p) t -> p ct t", p=P)
            # startup-critical order: q weights, first g chunk, then the rest
            nc.sync.dma_start(wqk[:, 0:4, 0:256], wqksrc[:, 0:4, 0:256])
            g0 = gpool.tile([P, NCT, 256], bf16, name="g", tag="g")
            nc.sync.dma_start(g0[:, 0:4, :], gsrc[:, 0:4, 0:256])
            nc.sync.dma_start(wqk[:, 4:8, 0:256], wqksrc[:, 4:8, 0:256])
            nc.sync.dma_start(g0[:, 4:8, :], gsrc[:, 4:8, 0:256])
            nc.sync.dma_start(wqk[:, :, 256:512], wqksrc[:, :, 256:512])
            nc.sync.dma_start(wv[:],
                              d["WvT"].rearrange("(ct p) o -> p ct o", p=P))
            for t, key in [(bqk, "bqk"), (bup, "bup"), (bdown, "bdown"),
                           (diagm, "diagm")]:
                nc.sync.dma_start(t[:], d[key])
            nc.sync.dma_start(wp_sb[:],
                              d["WpT"].rearrange("(j p) o -> p j o", p=P))
            for hh in range(2 * NSPAN):      # half-spans of 256 tokens
                c0 = hh * 256
                if hh == 0:
                    g = g0
                else:
                    g = gpool.tile([P, NCT, 256], bf16, name="g", tag="g")
                    nc.sync.dma_start(g[:], gsrc[:, :, c0:c0 + 256])
                for ot in range(4):          # q0 q1 k0 k1
                    ps = qkps.tile([P, 256], f32, name="ps", tag="qk")
                    for ct in range(NCT):
                        nc.tensor.matmul(ps[:], wqk[:, ct, ot * P:(ot + 1) * P],
                                         g[:, ct, :],
                                         start=(ct == 0), stop=(ct == NCT - 1))
                    dstT = qT if ot < 2 else kT
                    nc.scalar.add(dstT[:, ot % 2, c0:c0 + 256],
                                  ps[:], bqk[:, ot:ot + 1])
                for tl in range(2):          # v, token tile kt = 2*hh+tl
                    kt = 2 * hh + tl
                    pv = qkps.tile([P, 256], f32, name="pv", tag="qk")
                    for ct in range(NCT):
                        nc.tensor.matmul(pv[:], g[:, ct, tl * P:(tl + 1) * P],
                                         wv[:, ct, :],
                                         start=(ct == 0), stop=(ct == NCT - 1))
                    nc.scalar.copy(
                        v_aug[:, :, kt, 0:64],
                        pv[:].rearrange("p (h dd) -> p h dd", dd=64))
            # stage the big loads (SP queue, after the startup-critical ones)
            wdsrc = d["WdownT"].rearrange("(cf p) o -> p cf o", p=P)
            for cc in range(8):
                nc.sync.dma_start(wd[:, cc * 4:(cc + 1) * 4, :],
                                  wdsrc[:, cc * 4:(cc + 1) * 4, :])
            nc.sync.dma_start(x1T[:],
                              d["xbT"].rearrange("(ct p) t -> p ct t", p=P))

        # ----- strip-MLP emission helpers -----
        wusrc = d["WupT"].rearrange("(ct p) f -> p ct f", p=P)
        outdst = d["OUT"].rearrange("(ot p) t -> p ot t", p=P)

        def emit_wuw(w):
            t = mep.tile([P, NCT, 1024], bf16, name="wuw", tag="wuw", bufs=2)
            nc.sync.dma_start(
                t[:], wusrc[:, :, (w % 4) * 1024:((w % 4) + 1) * 1024])
            return t

        def emit_rs_transpose(r):
            rsT = mep.tile([P, NCT, STRIP], bf16, name="rsT", tag="rsT")
            nc.sync.dma_start_transpose(rsT[:], rsout[r][:])
            return rsT

        def emit_strip_pre(r, rsT, upps, gate_ex=None):
            cs = r * STRIP
            if gate_ex is not None:
                # scheduler-proofing: root the chain on a late attention exp
                # so its long RS-wait cannot head-block the DVE queue ahead
                # of ready attention work (the scheduler's internal sim
                # underestimates collective latency)
                gate2 = mep.tile([P, 1], f32, name="gate2", tag="gate2")
                nc.vector.tensor_scalar(gate2[:], gate_ex[:, 511:512],
                                        0.0, 1.0, mybir.AluOpType.mult,
                                        mybir.AluOpType.add)
                rsTg = mep.tile([P, NCT, STRIP], bf16, name="rsTg",
                                tag="x1b")
                nc.vector.tensor_scalar(rsTg[:], rsT[:], gate2[:, 0:1], None,
                                        mybir.AluOpType.mult)
                rsT = rsTg
            x1b = mep.tile([P, NCT, STRIP], bf16, name="x1b", tag="x1b")
            if r == 1 and DBG == "":
                # post-attention strip: bf16 x1 first so the LN2 stat
                # matmuls start one DVE op earlier; fp32 residual update
                # follows off the critical path
                nc.vector.tensor_add(x1b[:], x1T[:, :, cs:cs + STRIP],
                                     rsT[:])
                nc.vector.tensor_add(x1T[:, :, cs:cs + STRIP],
                                     x1T[:, :, cs:cs + STRIP], rsT[:])
            else:
                nc.vector.tensor_add(x1T[:, :, cs:cs + STRIP],
                                     x1T[:, :, cs:cs + STRIP], rsT[:])
                if DBG == "x1":
                    nc.sync.dma_start(outdst[:, :, cs:cs + STRIP],
                                      x1T[:, :, cs:cs + STRIP])
                    return None
                nc.vector.tensor_copy(x1b[:], x1T[:, :, cs:cs + STRIP])
            sqb = mep.tile([P, NCT, STRIP], bf16, name="sqb", tag="g28")
            for ct in range(NCT):
                nc.vector.tensor_mul(sqb[:, ct, :], x1b[:, ct, :],
                                     x1b[:, ct, :])
            psmu = upps.tile([P, STRIP], f32, name="psmu", tag="pu")
            for ct in range(NCT):
                nc.tensor.matmul(psmu[:], onesb[:], x1b[:, ct, :],
                                 start=(ct == 0), stop=(ct == NCT - 1))
            pssq = upps.tile([P, STRIP], f32, name="pssq", tag="pu")
            for ct in range(NCT):
                nc.tensor.matmul(pssq[:], onesb[:], sqb[:, ct, :],
                                 start=(ct == 0), stop=(ct == NCT - 1))
            mu = mep.tile([P, STRIP], f32, name="mu", tag="mu")
            e2 = mep.tile([P, STRIP], f32, name="e2", tag="e2")
            std = mep.tile([P, STRIP], f32, name="std", tag="stdt")
            nc.scalar.mul(mu[:], psmu[:], 1.0 / C)
            nc.scalar.mul(e2[:], pssq[:], 1.0 / C)
            musq = mep.tile([P, STRIP], f32, name="musq", tag="tmpc", bufs=2)
            nc.vector.tensor_mul(musq[:], mu[:], mu[:])
            nc.vector.tensor_sub(e2[:], e2[:], musq[:])
            nc.scalar.activation(std[:], e2[:], AF.Sqrt, bias=epsT[:])
            nc.vector.reciprocal(std[:], std[:])
            g2 = mep.tile([P, NCT, STRIP], bf16, name="g2", tag="g28")
            for ct in range(NCT):
                tmpc = mep.tile([P, STRIP], f32, name="tmpc", tag="tmpc",
                                bufs=2)
                nc.vector.tensor_sub(tmpc[:], x1T[:, ct, cs:cs + STRIP],
                                     mu[:])
                nc.vector.tensor_mul(g2[:, ct, :], tmpc[:], std[:])
            if DBG == "g2":
                g2f = mep.tile([P, NCT, STRIP], f32, name="g2f", tag="g2f")
                nc.vector.tensor_copy(g2f[:], g2[:])
                nc.sync.dma_start(outdst[:, :, cs:cs + STRIP], g2f[:])
                return None
            hT = mep.tile([P, NFT, STRIP], bf16, name="hT", tag="hT")
            return dict(cs=cs, g2=g2, hT=hT)

        def emit_up_group(st, f, wt, defer_gelu=False):
            pu = upps_ref[0].tile([P, STRIP], f32, name="pu", tag="pu")
            fl = f % 8
            for ct in range(NCT):
                nc.tensor.matmul(pu[:], wt[:, ct, fl * P:(fl + 1) * P],
                                 st["g2"][:, ct, :],
                                 start=(ct == 0), stop=(ct == NCT - 1))
            if defer_gelu:
                # keep Gelu out of the attention window: its act table
                # does not share a set with Exp, so inline gelus thrash
                # 1.28us table loads per switch.  Stage raw pre-act.
                nc.vector.tensor_copy(st["hT"][:, f, :], pu[:])
            else:
                nc.scalar.activation(st["hT"][:, f, :], pu[:], AF.Gelu,
                                     bias=bup[:, f:f + 1])

        def emit_down_out(st, dnps):
            for ot in range(8):
                pd = dnps.tile([P, STRIP], f32, name="pd", tag="dn")
                for cf in range(NFT):
                    nc.tensor.matmul(pd[:], wd[:, cf, ot * P:(ot + 1) * P],
                                     st["hT"][:, cf, :],
                                     start=(cf == 0), stop=(cf == NFT - 1))
                td = mep.tile([P, STRIP], f32, name="td", tag="td", bufs=2)
                nc.scalar.add(td[:], pd[:], bdown[:, ot:ot + 1])
                ox = mep.tile([P, STRIP], f32, name="ox", tag="ox", bufs=2)
                nc.vector.tensor_add(ox[:], td[:],
                                     x1T[:, ot, st["cs"]:st["cs"] + STRIP])
                nc.gpsimd.dma_start(outdst[:, ot, st["cs"]:st["cs"] + STRIP],
                                    ox[:])

        # ---------------- attention + proj + RS (+ strip0 MLP fill) --------
        wuw = {}
        if not DBG:
            wuw[0] = emit_wuw(0)
            wuw[1] = emit_wuw(1)
        st0 = None
        rsT0 = None
        rsT1 = None
        last_ex = None
        upf = [0]        # next strip0 up f-group to emit

        with tc.tile_pool(name="avps", bufs=4, space="PSUM") as avps, \
             tc.tile_pool(name="expp", bufs=4) as expp, \
             tc.tile_pool(name="nrmp", bufs=3) as nrmp, \
             tc.tile_pool(name="prst", bufs=2) as prst:

            def span_tail(Q, avs, mkpsum):
                # normalize: attnT[:, pr, span] = av / den, then proj + RS
                q0 = Q * SPAN
                for pr in range(2):
                    denb = nrmp.tile([P, 512], bf16, name="denb", tag="denb")
                    nc.vector.memset(denb[0:65, :], 0.0)
                    with nc.allow_low_precision(reason="bf16 softmax denom"):
                        nc.vector.reciprocal(denb[0:1, :],
                                             avs[2 * pr][64:65, :])
                        nc.vector.reciprocal(denb[64:65, :],
                                             avs[2 * pr + 1][64:65, :])
                    bc = mkpsum()
                    nc.tensor.matmul(bc[:], selp[0:65, :], denb[0:65, :],
                                     start=True, stop=True)
                    bcs = nrmp.tile([P, 512], f32, name="bcs", tag="bcs")
                    nc.scalar.copy(bcs[:], bc[:])
                    nc.vector.tensor_mul(attnT[0:64, pr, q0:q0 + SPAN],
                                         avs[2 * pr][0:64, :], bcs[0:64, :])
                    nc.vector.tensor_mul(attnT[64:128, pr, q0:q0 + SPAN],
                                         avs[2 * pr + 1][0:64, :],
                                         bcs[64:128, :])
                for tl in range(4):
                    tt = 4 * Q + tl
                    stage = prst.tile([P, 1024], bf16, name="stage", tag="st")
                    for half in range(2):
                        pp = avps.tile([P, 512], f32, name="pp", tag="av")
                        for j in range(2):
                            nc.tensor.matmul(
                                pp[:],
                                attnT[:, j, tt * P:(tt + 1) * P],
                                wp_sb[:, j, half * 512:(half + 1) * 512],
                                start=(j == 0), stop=(j == 1))
                        nc.vector.tensor_copy(
                            stage[:, half * 512:(half + 1) * 512], pp[:])
                    row = (Q % 2) * 512 + tl * P
                    nc.gpsimd.dma_start(partials[Q // 2][row:row + P, :],
                                        stage[:])
                if Q % 2 == 1:
                    r = Q // 2
                    nc.gpsimd.collective_compute(
                        "ReduceScatter", mybir.AluOpType.add,
                        ins=[partials[r].opt()], outs=[rsout[r].opt()],
                        replica_groups=[[0, 1, 2, 3], [4, 5, 6, 7]])
                    return emit_rs_transpose(r)
                return None

            # ---- spans 0-2: paired scores, one exp per head pair ----
            scp_cm = tc.tile_pool(name="scp", bufs=2, space="PSUM")
            scp = scp_cm.__enter__()

            def mkpsum_pair():
                t = scp.tile([P, 2, 512], f32, name="sc2", tag="sc")
                return t[:, 0, :]

            for Q in range(3):
                q0 = Q * SPAN
                avs = []
                for h in range(HC):
                    av = avps.tile([P, 512], f32, name=f"av{h}", tag="av")
                    avs.append(av)
                nkt = 4 * Q + 4
                for kt in range(nkt):
                    p_ = kt - 4 * Q
                    c0 = 128 * p_ if p_ > 0 else 0
                    sps = []
                    for pr in range(2):
                        sc2 = scp.tile([P, 2, 512], f32, name="sc2", tag="sc")
                        for i in range(2):
                            h = 2 * pr + i
                            hb = (h % 2) * 64
                            j = h // 2
                            nc.tensor.matmul(
                                sc2[:, i, c0:512],
                                kT[hb:hb + 64, j, kt * P:(kt + 1) * P],
                                qT[hb:hb + 64, j, q0 + c0:q0 + 512],
                                start=True, stop=True)
                        sps.append(sc2)
                    for pr in range(2):
                        ex2 = expp.tile([P, 2, 512], bf16, name="ex2",
                                        tag="ex")
                        nc.scalar.activation(ex2[:, :, c0:512],
                                             sps[pr][:, :, c0:512], AF.Exp)
                        for i in range(2):
                            h = 2 * pr + i
                            if p_ >= 0:
                                nc.vector.tensor_mul(ex2[:, i, c0:c0 + 128],
                                                     ex2[:, i, c0:c0 + 128],
                                                     diagm[:])
                            nc.tensor.matmul(
                                avs[h][0:65, c0:512],
                                v_aug[:, h, kt, 0:65],
                                ex2[:, i, c0:512],
                                start=(kt == 0), stop=(kt == nkt - 1),
                                skip_group_check=True)
                rsT = span_tail(Q, avs, mkpsum_pair)
                if rsT is not None:
                    rsT0 = rsT
                    if not DBG:
                        wuw[2] = emit_wuw(2)
                        wuw[3] = emit_wuw(3)

            scp_cm.__exit__(None, None, None)

            # ---- span 3: single scores + strip0 MLP fill ----
            with tc.tile_pool(name="sc3p", bufs=2, space="PSUM") as scps, \
                 tc.tile_pool(name="upps", bufs=2, space="PSUM") as upps:
                upps_ref = [upps]

                def fill_slot(kt):
                    nonlocal st0
                    if DBG:
                        return
                    if kt == 4:
                        st0 = emit_strip_pre(0, rsT0, upps, gate_ex=last_ex)
                        return
                    if st0 is None or upf[0] >= NFT:
                        return
                    for _ in range(2):
                        if upf[0] >= NFT:
                            break
                        emit_up_group(st0, upf[0], wuw[upf[0] // 8],
                                      defer_gelu=True)
                        upf[0] += 1

                Q = 3
                q0 = Q * SPAN
                avs = []
                for h in range(HC):
                    av = avps.tile([P, 512], f32, name=f"av{h}", tag="av")
                    avs.append(av)
                nkt = 16
                for kt in range(nkt):
                    p_ = kt - 4 * Q
                    c0 = 128 * p_ if p_ > 0 else 0
                    scs = []
                    for h in range(HC):
                        hb = (h % 2) * 64
                        j = h // 2
                        sc = scps.tile([P, 512], f32, name="sc", tag="sc")
                        nc.tensor.matmul(
                            sc[:, c0:512],
                            kT[hb:hb + 64, j, kt * P:(kt + 1) * P],
                            qT[hb:hb + 64, j, q0 + c0:q0 + 512],
                            start=True, stop=True)
                        scs.append(sc)
                    for h in range(HC):
                        ex = expp.tile([P, 512], bf16, name="ex", tag="ex")
                        last_ex = ex
                        nc.scalar.activation(ex[:, c0:512], scs[h][:, c0:512],
                                             AF.Exp)
                        if p_ >= 0:
                            nc.vector.tensor_mul(ex[:, c0:c0 + 128],
                                                 ex[:, c0:c0 + 128], diagm[:])
                        nc.tensor.matmul(
                            avs[h][0:65, c0:512],
                            v_aug[:, h, kt, 0:65],
                            ex[:, c0:512],
                            start=(kt == 0), stop=(kt == nkt - 1),
                            skip_group_check=True)
                    fill_slot(kt)

                def mkpsum_single():
                    return scps.tile([P, 512], f32, name="pp", tag="sc")

                rsT1 = span_tail(3, avs, mkpsum_single)
                if not DBG:
                    for w in range(4, 8):
                        wuw[w] = emit_wuw(w)
                # rest of strip0's ups (overlap RS2 on the collective cores)
                if not DBG and st0 is not None:
                    while upf[0] < NFT:
                        emit_up_group(st0, upf[0], wuw[upf[0] // 8],
                                      defer_gelu=True)
                        upf[0] += 1
                # gate = 1.0, data-dependent on the last exp: ops scaled by
                # it cannot be scheduled into the attention exp stream
                nc.vector.tensor_scalar(gate[:], last_ex[:, 511:512], 0.0,
                                        1.0, mybir.AluOpType.mult,
                                        mybir.AluOpType.add)
                if DBG:
                    rsT0x = rsT0 if rsT0 is not None else emit_rs_transpose(0)
                    emit_strip_pre(0, rsT0x, upps)
                    emit_strip_pre(1, rsT1, upps)

        bpool_cm.__exit__(None, None, None)   # free qT / kT / v_aug / wp_sb

        # ---------------- strip0 down + full strip1 ----------------
        if not DBG:
            with tc.tile_pool(name="dnps", bufs=3, space="PSUM") as dnps, \
                 tc.tile_pool(name="up2", bufs=3, space="PSUM") as up2:
                upps_ref[0] = up2
                # bulk gelu for strip0's staged pre-activations (one table
                # switch, after all attention exps are done; gate enforces it)
                for f in range(NFT):
                    nc.scalar.activation(st0["hT"][:, f, :], st0["hT"][:, f, :],
                                         AF.Gelu, bias=bup[:, f:f + 1],
                                         scale=gate[:, 0:1])
                emit_down_out(st0, dnps)
                st1 = emit_strip_pre(1, rsT1, up2)
                for f in range(NFT):
                    emit_up_group(st1, f, wuw[4 + f // 8])
                emit_down_out(st1, dnps)


def _prep_inputs(x, ln1_w, ln1_b, c_attn_w, c_attn_b, c_proj_w, c_proj_b,
                 ln2_w, ln2_b, up_w, up_b, down_w, down_b):
    """Host-side preprocessing -> list of 8 per-core input dicts."""
    x = np.asarray(x, np.float32)
    f64 = np.float64
    mu = x.mean(-1, keepdims=True, dtype=f64)
    var = np.asarray(x, f64).var(-1, keepdims=True)
    g = ((x - mu) / np.sqrt(var + EPS)).astype(np.float32)     # [B, T, C]

    ln1_w = np.asarray(ln1_w, np.float32); ln1_b = np.asarray(ln1_b, np.float32)
    ln2_w = np.asarray(ln2_w, np.float32); ln2_b = np.asarray(ln2_b, np.float32)
    c_attn_w = np.asarray(c_attn_w, np.float32)
    c_attn_b = np.asarray(c_attn_b, np.float32)
    c_proj_w = np.asarray(c_proj_w, np.float32)
    c_proj_b = np.asarray(c_proj_b, np.float32)
    up_w = np.asarray(up_w, np.float32); up_b = np.asarray(up_b, np.float32)
    down_w = np.asarray(down_w, np.float32)
    down_b = np.asarray(down_b, np.float32)

    Wa = c_attn_w * ln1_w[None, :]                  # fold LN1 scale
    ba = c_attn_b + c_attn_w @ ln1_b                # fold LN1 shift
    Wq, Wk, Wv = Wa[:C], Wa[C:2 * C], Wa[2 * C:]
    bq, bk, bv = ba[:C], ba[C:2 * C], ba[2 * C:]
    s = 1.0 / np.sqrt(D)
    Wq = Wq * s; bq = bq * s                        # fold attention scale

    Wup = up_w * ln2_w[None, :]
    bupv = up_b + up_w @ ln2_b

    def b2t(v, n):   # per-partition bias layout [128, n]
        return np.ascontiguousarray(v.reshape(n, P).T.astype(np.float32))

    diagm = np.triu(np.ones((P, P), np.float32))    # kv row <= q col

    shared = {
        "WupT": np.ascontiguousarray(Wup.T).astype(BF),
        "WdownT": np.ascontiguousarray(down_w.T).astype(BF),
        "bup": b2t(bupv, 32), "bdown": b2t(down_b, 8),
        "diagm": diagm.astype(BF),
    }

    # residual with proj bias and (v-bias pushed through proj) folded in
    xb = x + (c_proj_b + c_proj_w @ bv)[None, None, :]

    in_maps, tok_slices = [], []
    for core in range(8):
        b, i = core // 4, core % 4
        ch = slice(i * 256, (i + 1) * 256)          # this core's attn channels
        wqk = np.concatenate([Wq[ch], Wk[ch]], axis=0)      # [512, 1024]
        m = dict(shared)
        m["WqkT"] = np.ascontiguousarray(wqk.T).astype(BF)
        m["WvT"] = np.ascontiguousarray(Wv[ch].T).astype(BF)
        m["WpT"] = np.ascontiguousarray(c_proj_w[:, ch].T).astype(BF)
        m["bqk"] = b2t(np.concatenate([bq[ch], bk[ch]]), 4)
        m["gT"] = np.ascontiguousarray(g[b].T).astype(BF)
        strips = [slice(1024 * r + STRIP * i, 1024 * r + STRIP * (i + 1))
                  for r in range(2)]
        xbT = np.concatenate([xb[b, st].T for st in strips], axis=1)
        m["xbT"] = np.ascontiguousarray(xbT).astype(np.float32)
        in_maps.append(m)
        tok_slices.append((b, strips))
    return in_maps, tok_slices


def kernel(**inputs):
    global _CACHED_NC
    if _CACHED_NC is None:
        _CACHED_NC = _build_nc()
    nc = _CACHED_NC
    in_maps, tok_slices = _prep_inputs(**inputs)
    try:
        res = run_bass_kernel_spmd(nc, in_maps, list(range(8)))
    except Exception:
        # one retry: transient NRT device faults are recoverable on re-run
        res = run_bass_kernel_spmd(nc, in_maps, list(range(8)))
    out = np.empty((B, T, C), np.float32)
    for core in range(8):
        o = res.results[core]["OUT"]                # [C, 512]
        b, strips = tok_slices[core]
        for r, st in enumerate(strips):
            out[b, st, :] = o[:, r * STRIP:(r + 1) * STRIP].T
    return out


# revision 9
# speedup vs baseline: 1.0649x; 1.0000x over previous
"""Trainium2 Bass kernel for a GPT-style transformer block (B=2, T=2048,
C=1024, 16 heads, MLP 4x), sharded across 8 NeuronCores.

Sharding: attention is HEAD-sharded. Core c = (b, i) with b = c//4,
i = c%4 owns heads [4i, 4i+4) of batch b for the whole 2048-token range:
it computes q/k/v for just those 256 channels (same FLOPs as a row shard
of all channels), runs causal attention with no cross-core kv exchange,
then produces token-major c_proj partials. Two ReduceScatters (one per
1024-token chunk) sum the partials over the 4-core batch group and hand
each core a contiguous 256-token strip per chunk; LN2+MLP then run
data-parallel on the core's two strips.

Host precomputes LN1 (inputs-only), folds LN scale/shift into the matmul
weights, folds 1/sqrt(D) into Wq, and folds the v-bias + proj-bias into
the residual (softmax weights sum to 1, so attn_out = av/den + bv and
proj(attn_out) = proj(av/den) + Wp@bv). Causality lives in the loop
bounds; only diagonal 128x128 tiles get a multiplicative triu mask.
"""
import numpy as np
import ml_dtypes

import concourse.bass as bass
import concourse.mybir as mybir
import concourse.tile as tile
import concourse.bacc as bacc
from concourse.bass_utils import run_bass_kernel_spmd

BF = ml_dtypes.bfloat16
P = 128
B, T, C, D, F = 2, 2048, 1024, 64, 4096
NCT = C // P            # 8   c-tiles
NFT = F // P            # 32  f-tiles
NTT = T // P            # 16  token tiles
HC = 4                  # heads per core
SPAN = 512              # q/token span
NSPAN = T // SPAN       # 4
STRIP = 256             # tokens owned per core per RS chunk
EPS = 1e-5
f32 = mybir.dt.float32
bf16 = mybir.dt.bfloat16
AF = mybir.ActivationFunctionType

_CACHED_NC = None
import os as _os
DBG = _os.environ.get("KDBG", "")


def _build_nc():
    nc = bacc.Bacc("TRN2", target_bir_lowering=False, debug=False)
    d = {}
    for name, shape, dt in [
        ("gT", [C, T], bf16),          # ln1(x) for the whole batch, ch-major
        ("WqkT", [C, 512], bf16),      # q(256) | k(256) out channels
        ("WvT", [C, 256], bf16),
        ("WpT", [256, C], bf16),       # [attn-ch, out-ch] for moving use
        ("WupT", [C, F], bf16),
        ("WdownT", [F, C], bf16),
        ("bqk", [P, 4], f32),
        ("bup", [P, 32], f32),
        ("bdown", [P, 8], f32),
        ("xbT", [C, 512], f32),        # residual for own tokens (biases folded)
        ("diagm", [P, P], bf16),       # triu causal mask for diagonal tiles
    ]:
        d[name] = nc.dram_tensor(name, shape, dt, kind="ExternalInput").ap()
    d["OUT"] = nc.dram_tensor("OUT", [C, 512], f32, kind="ExternalOutput").ap()

    with tile.TileContext(nc) as tc:
        _emit(nc, tc, d)
    nc.compile()
    return nc


def _emit(nc, tc, d):
    from contextlib import ExitStack

    with ExitStack() as ctx:
        # ---------------- long-lived tiles ----------------
        cpool = ctx.enter_context(tc.tile_pool(name="cpool", bufs=1))
        attnT = cpool.tile([P, 2, T], bf16, name="attnT")
        bqk = cpool.tile([P, 4], f32, name="bqk")
        bup = cpool.tile([P, 32], f32, name="bup")
        bdown = cpool.tile([P, 8], f32, name="bdown")
        diagm = cpool.tile([P, P], bf16, name="diagm")
        epsT = cpool.tile([P, 1], f32, name="epsT")
        onesb = cpool.tile([P, P], bf16, name="onesb")
        selp = cpool.tile([P, P], bf16, name="selp")
        gate = cpool.tile([P, 1], f32, name="gate")
        nc.vector.memset(epsT[:], EPS)
        nc.vector.memset(onesb[:], 1.0)
        nc.vector.memset(selp[:], 0.0)
        nc.vector.memset(selp[0:1, 0:64], 1.0)
        nc.vector.memset(selp[64:65, 64:128], 1.0)

        # DRAM scratch for the proj partials + RS outputs
        dramp = ctx.enter_context(tc.tile_pool(name="dramp", bufs=1,
                                               space="DRAM"))
        partials = [dramp.tile([1024, C], bf16, name=f"part{r}")
                    for r in range(2)]
        rsout = [dramp.tile([STRIP, C], bf16, name=f"rsout{r}")
                 for r in range(2)]

        # MLP weights / residual / strip tiles: allocated BEFORE the
        # attention pools so strip-0's MLP can run while attention finishes
        # (tiles in later pools would inherit waits on attention SBUF reuse).
        wmlp = ctx.enter_context(tc.tile_pool(name="wmlp", bufs=1))
        wd = wmlp.tile([P, NFT, C], bf16, name="wd")
        x1T = wmlp.tile([P, NCT, 512], f32, name="x1T")    # xb, then x1
        mep = ctx.enter_context(tc.tile_pool(name="mep", bufs=1))

        # attention operand tiles (freed after proj)
        bpool_cm = tc.tile_pool(name="bpool", bufs=1)
        bpool = bpool_cm.__enter__()
        qT = bpool.tile([P, 2, T], bf16, name="qT")
        kT = bpool.tile([P, 2, T], bf16, name="kT")
        v_aug = bpool.tile([P, HC, NTT, 65], bf16, name="v_aug")
        wp_sb = bpool.tile([P, 2, C], bf16, name="wp_sb")
        nc.vector.memset(v_aug[:, :, :, 64:65], 1.0)

        # ---------------- qkv projections ----------------
        with tc.tile_pool(name="gpool", bufs=2) as gpool, \
             tc.tile_pool(name="wqkp", bufs=1) as wqkp, \
             tc.tile_pool(name="qkps", bufs=4, space="PSUM") as qkps:
            wqk = wqkp.tile([P, NCT, 512], bf16, name="wqk")
            wv = wqkp.tile([P, NCT, 256], bf16, name="wv")
            wqksrc = d["WqkT"].rearrange("(ct p) o -> p ct o", p=P)
            gsrc = d["gT"].rearrange("(ct p) t -> p ct t", p=P)
            # startup-critical order: q weights, first g chunk, then the rest
            nc.sync.dma_start(wqk[:, 0:4, 0:256], wqksrc[:, 0:4, 0:256])
            g0 = gpool.tile([P, NCT, 256], bf16, name="g", tag="g")
            nc.sync.dma_start(g0[:, 0:4, :], gsrc[:, 0:4, 0:256])
            nc.sync.dma_start(wqk[:, 4:8, 0:256], wqksrc[:, 4:8, 0:256])
            nc.sync.dma_start(g0[:, 4:8, :], gsrc[:, 4:8, 0:256])
            nc.sync.dma_start(wqk[:, :, 256:512], wqksrc[:, :, 256:512])
            nc.sync.dma_start(wv[:],
                              d["WvT"].rearrange("(ct p) o -> p ct o", p=P))
            for t, key in [(bqk, "bqk"), (bup, "bup"), (bdown, "bdown"),
                           (diagm, "diagm")]:
                nc.sync.dma_start(t[:], d[key])
            nc.sync.dma_start(wp_sb[:],
                              d["WpT"].rearrange("(j p) o -> p j o", p=P))
            for hh in range(2 * NSPAN):      # half-spans of 256 tokens
                c0 = hh * 256
                if hh == 0:
                    g = g0
                else:
                    g = gpool.tile([P, NCT, 256], bf16, name="g", tag="g")
                    nc.sync.dma_start(g[:], gsrc[:, :, c0:c0 + 256])
                for ot in range(4):          # q0 q1 k0 k1
                    ps = qkps.tile([P, 256], f32, name="ps", tag="qk")
                    for ct in range(NCT):
                        nc.tensor.matmul(ps[:], wqk[:, ct, ot * P:(ot + 1) * P],
                                         g[:, ct, :],
                                         start=(ct == 0), stop=(ct == NCT - 1))
                    dstT = qT if ot < 2 else kT
                    nc.scalar.add(dstT[:, ot % 2, c0:c0 + 256],
                                  ps[:], bqk[:, ot:ot + 1])
                for tl in range(2):          # v, token tile kt = 2*hh+tl
                    kt = 2 * hh + tl
                    pv = qkps.tile([P, 256], f32, name="pv", tag="qk")
                    for ct in range(NCT):
                        nc.tensor.matmul(pv[:], g[:, ct, tl * P:(tl + 1) * P],
                                         wv[:, ct, :],
                                         start=(ct == 0), stop=(ct == NCT - 1))
                    nc.scalar.copy(
                        v_aug[:, :, kt, 0:64],
                        pv[:].rearrange("p (h dd) -> p h dd", dd=64))
            # stage the big loads (SP queue, after the startup-critical ones)
            wdsrc = d["WdownT"].rearrange("(cf p) o -> p cf o", p=P)
            for cc in range(8):
                nc.sync.dma_start(wd[:, cc * 4:(cc + 1) * 4, :],
                                  wdsrc[:, cc * 4:(cc + 1) * 4, :])
            nc.sync.dma_start(x1T[:],
                              d["xbT"].rearrange("(ct p) t -> p ct t", p=P))

        # ----- strip-MLP emission helpers -----
        wusrc = d["WupT"].rearrange("(ct p) f -> p ct f", p=P)
        outdst = d["OUT"].rearrange("(ot p) t -> p ot t", p=P)

        def emit_wuw(w):
            t = mep.tile([P, NCT, 1024], bf16, name="wuw", tag="wuw", bufs=2)
            nc.sync.dma_start(
                t[:], wusrc[:, :, (w % 4) * 1024:((w % 4) + 1) * 1024])
            return t

        def emit_rs_transpose(r):
            rsT = mep.tile([P, NCT, STRIP], bf16, name="rsT", tag="rsT")
            nc.sync.dma_start_transpose(rsT[:], rsout[r][:])
            return rsT

        def emit_strip_pre(r, rsT, upps, gate_ex=None):
            cs = r * STRIP
            if gate_ex is not None:
                # scheduler-proofing: root the chain on a late attention exp
                # so its long RS-wait cannot head-block the DVE queue ahead
                # of ready attention work (the scheduler's internal sim
                # underestimates collective latency)
                gate2 = mep.tile([P, 1], f32, name="gate2", tag="gate2")
                nc.vector.tensor_scalar(gate2[:], gate_ex[:, 511:512],
                                        0.0, 1.0, mybir.AluOpType.mult,
                                        mybir.AluOpType.add)
                rsTg = mep.tile([P, NCT, STRIP], bf16, name="rsTg",
                                tag="x1b")
                nc.vector.tensor_scalar(rsTg[:], rsT[:], gate2[:, 0:1], None,
                                        mybir.AluOpType.mult)
                rsT = rsTg
            x1b = mep.tile([P, NCT, STRIP], bf16, name="x1b", tag="x1b")
            if r == 1 and DBG == "":
                # post-attention strip: bf16 x1 first so the LN2 stat
                # matmuls start one DVE op earlier; fp32 residual update
                # follows off the critical path
                nc.vector.tensor_add(x1b[:], x1T[:, :, cs:cs + STRIP],
                                     rsT[:])
                nc.vector.tensor_add(x1T[:, :, cs:cs + STRIP],
                                     x1T[:, :, cs:cs + STRIP], rsT[:])
            else:
                nc.vector.tensor_add(x1T[:, :, cs:cs + STRIP],
                                     x1T[:, :, cs:cs + STRIP], rsT[:])
                if DBG == "x1":
                    nc.sync.dma_start(outdst[:, :, cs:cs + STRIP],
                                      x1T[:, :, cs:cs + STRIP])
                    return None
                nc.vector.tensor_copy(x1b[:], x1T[:, :, cs:cs + STRIP])
            sqb = mep.tile([P, NCT, STRIP], bf16, name="sqb", tag="g28")
            for ct in range(NCT):
                nc.vector.tensor_mul(sqb[:, ct, :], x1b[:, ct, :],
                                     x1b[:, ct, :])
            psmu = upps.tile([P, STRIP], f32, name="psmu", tag="pu")
            for ct in range(NCT):
                nc.tensor.matmul(psmu[:], onesb[:], x1b[:, ct, :],
                                 start=(ct == 0), stop=(ct == NCT - 1))
            pssq = upps.tile([P, STRIP], f32, name="pssq", tag="pu")
            for ct in range(NCT):
                nc.tensor.matmul(pssq[:], onesb[:], sqb[:, ct, :],
                                 start=(ct == 0), stop=(ct == NCT - 1))
            mu = mep.tile([P, STRIP], f32, name="mu", tag="mu")
            e2 = mep.tile([P, STRIP], f32, name="e2", tag="e2")
            std = mep.tile([P, STRIP], f32, name="std", tag="stdt")
            nc.scalar.mul(mu[:], psmu[:], 1.0 / C)
            nc.scalar.mul(e2[:], pssq[:], 1.0 / C)
            musq = mep.tile([P, STRIP], f32, name="musq", tag="tmpc", bufs=2)
            nc.vector.tensor_mul(musq[:], mu[:], mu[:])
            nc.vector.tensor_sub(e2[:], e2[:], musq[:])
            nc.scalar.activation(std[:], e2[:], AF.Sqrt, bias=epsT[:])
            nc.vector.reciprocal(std[:], std[:])
            g2 = mep.tile([P, NCT, STRIP], bf16, name="g2", tag="g28")
            for ct in range(NCT):
                tmpc = mep.tile([P, STRIP], f32, name="tmpc", tag="tmpc",
                                bufs=2)
                nc.vector.tensor_sub(tmpc[:], x1T[:, ct, cs:cs + STRIP],
                                     mu[:])
                nc.vector.tensor_mul(g2[:, ct, :], tmpc[:], std[:])
            if DBG == "g2":
                g2f = mep.tile([P, NCT, STRIP], f32, name="g2f", tag="g2f")
                nc.vector.tensor_copy(g2f[:], g2[:])
                nc.sync.dma_start(outdst[:, :, cs:cs + STRIP], g2f[:])
                return None
            hT = mep.tile([P, NFT, STRIP], bf16, name="hT", tag="hT")
            return dict(cs=cs, g2=g2, hT=hT)

        def emit_up_group(st, f, wt, defer_gelu=False):
            pu = upps_ref[0].tile([P, STRIP], f32, name="pu", tag="pu")
            fl = f % 8
            for ct in range(NCT):
                nc.tensor.matmul(pu[:], wt[:, ct, fl * P:(fl + 1) * P],
                                 st["g2"][:, ct, :],
                                 start=(ct == 0), stop=(ct == NCT - 1))
            if defer_gelu:
                # keep Gelu out of the attention window: its act table
                # does not share a set with Exp, so inline gelus thrash
                # 1.28us table loads per switch.  Stage raw pre-act.
                nc.vector.tensor_copy(st["hT"][:, f, :], pu[:])
            else:
                nc.scalar.activation(st["hT"][:, f, :], pu[:], AF.Gelu,
                                     bias=bup[:, f:f + 1])

        def emit_down_out(st, dnps):
            for ot in range(8):
                pd = dnps.tile([P, STRIP], f32, name="pd", tag="dn")
                for cf in range(NFT):
                    nc.tensor.matmul(pd[:], wd[:, cf, ot * P:(ot + 1) * P],
                                     st["hT"][:, cf, :],
                                     start=(cf == 0), stop=(cf == NFT - 1))
                td = mep.tile([P, STRIP], f32, name="td", tag="td", bufs=2)
                nc.scalar.add(td[:], pd[:], bdown[:, ot:ot + 1])
                ox = mep.tile([P, STRIP], f32, name="ox", tag="ox", bufs=2)
                nc.vector.tensor_add(ox[:], td[:],
                                     x1T[:, ot, st["cs"]:st["cs"] + STRIP])
                nc.gpsimd.dma_start(outdst[:, ot, st["cs"]:st["cs"] + STRIP],
                                    ox[:])

        # ---------------- attention + proj + RS (+ strip0 MLP fill) --------
        wuw = {}
        if not DBG:
            wuw[0] = emit_wuw(0)
            wuw[1] = emit_wuw(1)
        st0 = None
        rsT0 = None
        rsT1 = None
        last_ex = None
        upf = [0]        # next strip0 up f-group to emit

        with tc.tile_pool(name="avps", bufs=4, space="PSUM") as avps, \
             tc.tile_pool(name="expp", bufs=4) as expp, \
             tc.tile_pool(name="nrmp", bufs=3) as nrmp, \
             tc.tile_pool(name="prst", bufs=2) as prst:

            def span_tail(Q, avs, mkpsum):
                # normalize: attnT[:, pr, span] = av / den, then proj + RS
                q0 = Q * SPAN
                for pr in range(2):
                    denb = nrmp.tile([P, 512], bf16, name="denb", tag="denb")
                    nc.vector.memset(denb[0:65, :], 0.0)
                    with nc.allow_low_precision(reason="bf16 softmax denom"):
                        nc.vector.reciprocal(denb[0:1, :],
                                             avs[2 * pr][64:65, :])
                        nc.vector.reciprocal(denb[64:65, :],
                                             avs[2 * pr + 1][64:65, :])
                    bc = mkpsum()
                    nc.tensor.matmul(bc[:], selp[0:65, :], denb[0:65, :],
                                     start=True, stop=True)
                    bcs = nrmp.tile([P, 512], f32, name="bcs", tag="bcs")
                    nc.scalar.copy(bcs[:], bc[:])
                    nc.vector.tensor_mul(attnT[0:64, pr, q0:q0 + SPAN],
                                         avs[2 * pr][0:64, :], bcs[0:64, :])
                    nc.vector.tensor_mul(attnT[64:128, pr, q0:q0 + SPAN],
                                         avs[2 * pr + 1][0:64, :],
                                         bcs[64:128, :])
                for tl in range(4):
                    tt = 4 * Q + tl
                    stage = prst.tile([P, 1024], bf16, name="stage", tag="st")
                    for half in range(2):
                        pp = avps.tile([P, 512], f32, name="pp", tag="av")
                        for j in range(2):
                            nc.tensor.matmul(
                                pp[:],
                                attnT[:, j, tt * P:(tt + 1) * P],
                                wp_sb[:, j, half * 512:(half + 1) * 512],
                                start=(j == 0), stop=(j == 1))
                        nc.vector.tensor_copy(
                            stage[:, half * 512:(half + 1) * 512], pp[:])
                    row = (Q % 2) * 512 + tl * P
                    nc.gpsimd.dma_start(partials[Q // 2][row:row + P, :],
                                        stage[:])
                if Q % 2 == 1:
                    r = Q // 2
                    nc.gpsimd.collective_compute(
                        "ReduceScatter", mybir.AluOpType.add,
                        ins=[partials[r].opt()], outs=[rsout[r].opt()],
                        replica_groups=[[0, 1, 2, 3], [4, 5, 6, 7]])
                    return emit_rs_transpose(r)
                return None

            # ---- spans 0-2: paired scores, one exp per head pair ----
            scp_cm = tc.tile_pool(name="scp", bufs=2, space="PSUM")
            scp = scp_cm.__enter__()

            def mkpsum_pair():
                t = scp.tile([P, 2, 512], f32, name="sc2", tag="sc")
                return t[:, 0, :]

            for Q in range(3):
                q0 = Q * SPAN
                avs = []
                for h in range(HC):
                    av = avps.tile([P, 512], f32, name=f"av{h}", tag="av")
                    avs.append(av)
                nkt = 4 * Q + 4
                for kt in range(nkt):
                    p_ = kt - 4 * Q
                    c0 = 128 * p_ if p_ > 0 else 0
                    sps = []
                    for pr in range(2):
                        sc2 = scp.tile([P, 2, 512], f32, name="sc2", tag="sc")
                        for i in range(2):
                            h = 2 * pr + i
                            hb = (h % 2) * 64
                            j = h // 2
                            nc.tensor.matmul(
                                sc2[:, i, c0:512],
                                kT[hb:hb + 64, j, kt * P:(kt + 1) * P],
                                qT[hb:hb + 64, j, q0 + c0:q0 + 512],
                                start=True, stop=True)
                        sps.append(sc2)
                    for pr in range(2):
                        ex2 = expp.tile([P, 2, 512], bf16, name="ex2",
                                        tag="ex")
                        nc.scalar.activation(ex2[:, :, c0:512],
                                             sps[pr][:, :, c0:512], AF.Exp)
                        for i in range(2):
                            h = 2 * pr + i
                            if p_ >= 0:
                                nc.vector.tensor_mul(ex2[:, i, c0:c0 + 128],
                                                     ex2[:, i, c0:c0 + 128],
                                                     diagm[:])
                            nc.tensor.matmul(
                                avs[h][0:65, c0:512],
                                v_aug[:, h, kt, 0:65],
                                ex2[:, i, c0:512],
                                start=(kt == 0), stop=(kt == nkt - 1),
                                skip_group_check=True)
                rsT = span_tail(Q, avs, mkpsum_pair)
                if rsT is not None:
                    rsT0 = rsT
                    if not DBG:
                        wuw[2] = emit_wuw(2)
                        wuw[3] = emit_wuw(3)

            scp_cm.__exit__(None, None, None)

            # ---- span 3: single scores + strip0 MLP fill ----
            with tc.tile_pool(name="sc3p", bufs=3, space="PSUM") as scps, \
                 tc.tile_pool(name="upps", bufs=1, space="PSUM") as upps:
                upps_ref = [upps]

                def fill_slot(kt):
                    nonlocal st0
                    if DBG:
                        return
                    if kt == 4:
                        st0 = emit_strip_pre(0, rsT0, upps, gate_ex=last_ex)
                        return
                    if st0 is None or upf[0] >= NFT:
                        return
                    for _ in range(2):
                        if upf[0] >= NFT:
                            break
                        emit_up_group(st0, upf[0], wuw[upf[0] // 8],
                                      defer_gelu=True)
                        upf[0] += 1

                Q = 3
                q0 = Q * SPAN
                avs = []
                for h in range(HC):
                    av = avps.tile([P, 512], f32, name=f"av{h}", tag="av")
                    avs.append(av)
                nkt = 16
                for kt in range(nkt):
                    p_ = kt - 4 * Q
                    c0 = 128 * p_ if p_ > 0 else 0
                    scs = []
                    for h in range(HC):
                        hb = (h % 2) * 64
                        j = h // 2
                        sc = scps.tile([P, 512], f32, name="sc", tag="sc")
                        nc.tensor.matmul(
                            sc[:, c0:512],
                            kT[hb:hb + 64, j, kt * P:(kt + 1) * P],
                            qT[hb:hb + 64, j, q0 + c0:q0 + 512],
                            start=True, stop=True)
                        scs.append(sc)
                    for h in range(HC):
                        ex = expp.tile([P, 512], bf16, name="ex", tag="ex")
                        last_ex = ex
                        nc.scalar.activation(ex[:, c0:512], scs[h][:, c0:512],
                                             AF.Exp)
                        if p_ >= 0:
                            nc.vector.tensor_mul(ex[:, c0:c0 + 128],
                                                 ex[:, c0:c0 + 128], diagm[:])
                        nc.tensor.matmul(
                            avs[h][0:65, c0:512],
                            v_aug[:, h, kt, 0:65],
                            ex[:, c0:512],
                            start=(kt == 0), stop=(kt == nkt - 1),
                            skip_group_check=True)
                    fill_slot(kt)

                def mkpsum_single():
                    return scps.tile([P, 512], f32, name="pp", tag="sc")

                rsT1 = span_tail(3, avs, mkpsum_single)
                if not DBG:
                    for w in range(4, 8):
                        wuw[w] = emit_wuw(w)
                # rest of strip0's ups (overlap RS2 on the collective cores)
                if not DBG and st0 is not None:
                    while upf[0] < NFT:
                        emit_up_group(st0, upf[0], wuw[upf[0] // 8],
                                      defer_gelu=True)
                        upf[0] += 1
                # gate = 1.0, data-dependent on the last exp: ops scaled by
                # it cannot be scheduled into the attention exp stream
                nc.vector.tensor_scalar(gate[:], last_ex[:, 511:512], 0.0,
                                        1.0, mybir.AluOpType.mult,
                                        mybir.AluOpType.add)
                if DBG:
                    rsT0x = rsT0 if rsT0 is not None else emit_rs_transpose(0)
                    emit_strip_pre(0, rsT0x, upps)
                    emit_strip_pre(1, rsT1, upps)

        bpool_cm.__exit__(None, None, None)   # free qT / kT / v_aug / wp_sb

        # ---------------- strip0 down + full strip1 ----------------
        if not DBG:
            with tc.tile_pool(name="dnps", bufs=3, space="PSUM") as dnps, \
                 tc.tile_pool(name="up2", bufs=3, space="PSUM") as up2:
                upps_ref[0] = up2
                # bulk gelu for strip0's staged pre-activations (one table
                # switch, after all attention exps are done; gate enforces it)
                for f in range(NFT):
                    nc.scalar.activation(st0["hT"][:, f, :], st0["hT"][:, f, :],
                                         AF.Gelu, bias=bup[:, f:f + 1],
                                         scale=gate[:, 0:1])
                emit_down_out(st0, dnps)
                st1 = emit_strip_pre(1, rsT1, up2)
                for f in range(NFT):
                    emit_up_group(st1, f, wuw[4 + f // 8])
                emit_down_out(st1, dnps)


def _prep_inputs(x, ln1_w, ln1_b, c_attn_w, c_attn_b, c_proj_w, c_proj_b,
                 ln2_w, ln2_b, up_w, up_b, down_w, down_b):
    """Host-side preprocessing -> list of 8 per-core input dicts."""
    x = np.asarray(x, np.float32)
    f64 = np.float64
    mu = x.mean(-1, keepdims=True, dtype=f64)
    var = np.asarray(x, f64).var(-1, keepdims=True)
    g = ((x - mu) / np.sqrt(var + EPS)).astype(np.float32)     # [B, T, C]

    ln1_w = np.asarray(ln1_w, np.float32); ln1_b = np.asarray(ln1_b, np.float32)
    ln2_w = np.asarray(ln2_w, np.float32); ln2_b = np.asarray(ln2_b, np.float32)
    c_attn_w = np.asarray(c_attn_w, np.float32)
    c_attn_b = np.asarray(c_attn_b, np.float32)
    c_proj_w = np.asarray(c_proj_w, np.float32)
    c_proj_b = np.asarray(c_proj_b, np.float32)
    up_w = np.asarray(up_w, np.float32); up_b = np.asarray(up_b, np.float32)
    down_w = np.asarray(down_w, np.float32)
    down_b = np.asarray(down_b, np.float32)

    Wa = c_attn_w * ln1_w[None, :]                  # fold LN1 scale
    ba = c_attn_b + c_attn_w @ ln1_b                # fold LN1 shift
    Wq, Wk, Wv = Wa[:C], Wa[C:2 * C], Wa[2 * C:]
    bq, bk, bv = ba[:C], ba[C:2 * C], ba[2 * C:]
    s = 1.0 / np.sqrt(D)
    Wq = Wq * s; bq = bq * s                        # fold attention scale

    Wup = up_w * ln2_w[None, :]
    bupv = up_b + up_w @ ln2_b

    def b2t(v, n):   # per-partition bias layout [128, n]
        return np.ascontiguousarray(v.reshape(n, P).T.astype(np.float32))

    diagm = np.triu(np.ones((P, P), np.float32))    # kv row <= q col

    shared = {
        "WupT": np.ascontiguousarray(Wup.T).astype(BF),
        "WdownT": np.ascontiguousarray(down_w.T).astype(BF),
        "bup": b2t(bupv, 32), "bdown": b2t(down_b, 8),
        "diagm": diagm.astype(BF),
    }

    # residual with proj bias and (v-bias pushed through proj) folded in
    xb = x + (c_proj_b + c_proj_w @ bv)[None, None, :]

    in_maps, tok_slices = [], []
    for core in range(8):
        b, i = core // 4, core % 4
        ch = slice(i * 256, (i + 1) * 256)          # this core's attn channels
        wqk = np.concatenate([Wq[ch], Wk[ch]], axis=0)      # [512, 1024]
        m = dict(shared)
        m["WqkT"] = np.ascontiguousarray(wqk.T).astype(BF)
        m["WvT"] = np.ascontiguousarray(Wv[ch].T).astype(BF)
        m["WpT"] = np.ascontiguousarray(c_proj_w[:, ch].T).astype(BF)
        m["bqk"] = b2t(np.concatenate([bq[ch], bk[ch]]), 4)
        m["gT"] = np.ascontiguousarray(g[b].T).astype(BF)
        strips = [slice(1024 * r + STRIP * i, 1024 * r + STRIP * (i + 1))
                  for r in range(2)]
        xbT = np.concatenate([xb[b, st].T for st in strips], axis=1)
        m["xbT"] = np.ascontiguousarray(xbT).astype(np.float32)
        in_maps.append(m)
        tok_slices.append((b, strips))
    return in_maps, tok_slices


def kernel(**inputs):
    global _CACHED_NC
    if _CACHED_NC is None:
        _CACHED_NC = _build_nc()
    nc = _CACHED_NC
    in_maps, tok_slices = _prep_inputs(**inputs)
    try:
        res = run_bass_kernel_spmd(nc, in_maps, list(range(8)))
    except Exception:
        # one retry: transient NRT device faults are recoverable on re-run
        res = run_bass_kernel_spmd(nc, in_maps, list(range(8)))
    out = np.empty((B, T, C), np.float32)
    for core in range(8):
        o = res.results[core]["OUT"]                # [C, 512]
        b, strips = tok_slices[core]
        for r, st in enumerate(strips):
            out[b, st, :] = o[:, r * STRIP:(r + 1) * STRIP].T
    return out
